# revision 54
# baseline (speedup 1.0000x reference)
"""GAT (2-layer, PyG-style) + MLP + graph-LN + global mean pool on 8 Trainium2 cores.

Strategy (sharding_hint): nodes partitioned contiguously across the 8 cores;
edges partitioned by destination node (1-D graph partition, host-sorted by dst);
the per-layer node-feature table [h | a_src-logit] is AllGathered so each core
gathers h[src] rows for its local edges with indirect DMA; per-destination
softmax + weighted aggregation is done with one-hot scatter matmuls
accumulating in PSUM per 128-node block; graph-LayerNorm statistics and are
combined with a tiny AllReduce; the final global_mean_pool partial sums per
core are assembled on the host (unshard step).
"""

import os
import sys

sys.path.insert(0, "/opt/trn_rl_repo")

import math
from contextlib import ExitStack

QSPLIT = os.environ.get("QSPLIT", "0") == "1"  # alternate SWDGE queues for gathers

import numpy as np
import ml_dtypes

BF = ml_dtypes.bfloat16

import concourse.bass as bass
import concourse.bacc as bacc
import concourse.tile as tile
import concourse.mybir as mybir
from concourse import bass_utils
from concourse.bass import IndirectOffsetOnAxis
from concourse.bass_interp import get_hw_module

F32 = mybir.dt.float32
BF16 = mybir.dt.bfloat16
I32 = mybir.dt.int32
I16 = mybir.dt.int16
ALU = mybir.AluOpType
ACTF = mybir.ActivationFunctionType

# ---- problem constants (hardcoded per spec) ----
N = 100000
E_RAW = 1600000
D_IN = 9
HID = 16
HEADS = 8
D = 128
G = 1000
NCORES = 8
NEG_SLOPE = 0.2
LN_EPS = 1e-5
TBLW = 144  # table row width in bf16 (288B rows: h 128 | al_s f32-pairs)
WIN = 32  # edge tiles per gather window
P = 128

PAD_DREL = 300.0  # one-hot never matches (iota is 0..127)
PAD_SLOT = 300.0


# ======================================================================
# host-side prep: edge sort / partition / padding, index layouts, weights
# ======================================================================
def _prep(x, edge_index, batch, n=N, e_raw=E_RAW, ncores=NCORES, g=G):
    nl = n // ncores  # owned nodes per core
    npd = ((nl + P - 1) // P) * P
    nb = npd // P  # 128-node blocks per core

    # self-loops are handled by an on-chip diagonal fast path (h and al
    # are core-local), so only the raw edges go through the gather.
    src = edge_index[0].astype(np.int64)
    dst = edge_index[1].astype(np.int64)
    order = np.argsort(dst, kind="stable")
    srcs = src[order]
    dsts = dst[order]

    bounds = np.searchsorted(dsts, np.arange(ncores + 1) * nl)
    per_core = []
    cnt = np.zeros((ncores, nb), dtype=np.int64)
    for c in range(ncores):
        s_c = srcs[bounds[c] : bounds[c + 1]]
        d_c = dsts[bounds[c] : bounds[c + 1]] - c * nl
        per_core.append((s_c, d_c))
        cnt[c] = np.bincount(d_c // P, minlength=nb)
    maxe = cnt.max(axis=0)
    cap = ((maxe + P - 1) // P) * P  # edge slots per block
    cap = np.maximum(cap, P)
    ktiles = (cap // P).astype(np.int64)
    t_total = int(ktiles.sum())
    blk_starts = np.concatenate([[0], np.cumsum(ktiles)])
    tile2blk = []
    mcap = []  # valid gather rows per tile (max over cores, pads skipped)
    for b in range(nb):
        tile2blk += [b] * int(ktiles[b])
        for i in range(int(ktiles[b])):
            mcap.append(int(min(P, max(1, maxe[b] - i * P))))
    blk_first = {b: int(blk_starts[b]) for b in range(nb)}
    blk_last = {b: int(blk_starts[b + 1]) - 1 for b in range(nb)}

    core_arrays = []
    for c in range(ncores):
        s_c, d_c = per_core[c]
        gidx = np.zeros((t_total * P,), dtype=np.int32)
        drel = np.full((t_total * P,), PAD_DREL, dtype=np.float32)
        blk = d_c // P
        cstart = np.concatenate([[0], np.cumsum(np.bincount(blk, minlength=nb))])
        for b in range(nb):
            e0, e1 = cstart[b], cstart[b + 1]
            o0 = int(blk_starts[b]) * P
            m = e1 - e0
            sc = s_c[e0:e1]
            gidx[o0 : o0 + m] = ((sc // nl) * npd + (sc % nl)).astype(np.int32)
            drel[o0 : o0 + m] = (d_c[e0:e1] % P).astype(np.float32)
        core_arrays.append(
            (
                gidx.reshape(t_total, P).T.copy(),
                drel.reshape(t_total, P).T.copy(),
            )
        )

    # pool slots
    bsort = np.asarray(batch, dtype=np.int64)
    slots = []
    g0s = []
    counts = np.bincount(bsort, minlength=g).astype(np.float64)
    for c in range(ncores):
        bs = bsort[c * nl : (c + 1) * nl]
        g0 = int(bs[0])
        sl = np.full((npd,), PAD_SLOT, dtype=np.float32)
        sl[:nl] = (bs - g0).astype(np.float32)
        assert sl[:nl].max() < 256, "graph-slot overflow"
        slots.append(sl.reshape(nb, P).T.copy())
        g0s.append(g0)

    meta = dict(
        n=n, nl=nl, npd=npd, nb=nb, t=t_total, mcap=mcap,
        tile2blk=tile2blk, blk_first=blk_first, blk_last=blk_last,
        ncores=ncores, g=g,
    )
    return meta, core_arrays, slots, g0s, counts


def _blockdiag(a):  # a [H, C] -> [H*C, H]
    h, c = a.shape
    out = np.zeros((h * c, h), dtype=np.float32)
    for i in range(h):
        out[i * c : (i + 1) * c, i] = a[i]
    return out


def _headmap(heads, ch):  # [H, H*C] one-hot expansion map
    out = np.zeros((heads, heads * ch), dtype=np.float32)
    for i in range(heads):
        out[i, i * ch : (i + 1) * ch] = 1.0
    return out


def _consts(weights, meta):
    """Replicated (same every core) input arrays."""
    w = weights
    c = {}
    c["W1"] = w["W1"].astype(np.float32)  # [9, 128]
    c["a1blk"] = np.concatenate(
        [_blockdiag(w["a_src1"]), _blockdiag(w["a_dst1"])], axis=1
    ).astype(BF)  # [128, 16]
    c["b1c"] = w["b1"].reshape(D, 1).astype(np.float32)
    c["W2"] = w["W2"].astype(BF)  # [128,128] lhsT
    c["a2blk"] = np.concatenate(
        [w["a_src2"].reshape(D, 1), w["a_dst2"].reshape(D, 1)], axis=1
    ).astype(BF)  # [128, 2]
    c["b2c"] = w["b2"].reshape(D, 1).astype(np.float32)
    for i in (1, 2, 3, 4):
        c[f"fc{i}w"] = w[f"fc{i}_w"].astype(BF)
        c[f"fc{i}b"] = w[f"fc{i}_b"].reshape(D, 1).astype(np.float32)
    c["g1c"] = w["g1"].reshape(D, 1).astype(np.float32)
    c["be1c"] = w["beta1"].reshape(D, 1).astype(np.float32)
    c["g2c"] = w["g2"].reshape(D, 1).astype(np.float32)
    c["be2c"] = w["beta2"].reshape(D, 1).astype(np.float32)
    c["fcfw"] = w["fcf_w"].reshape(D, 1).astype(BF)
    # layer-2 attention on raw (pre-LN) h: zals = h_pre @ (g1*(W2@a_src2)),
    # als2 = rstd1*zals + ofs1@(W2@a_src2); same for dst
    a2s = w["a_src2"].reshape(D).astype(np.float32)
    a2d = w["a_dst2"].reshape(D).astype(np.float32)
    W2f = w["W2"].astype(np.float32)
    g1f = w["g1"].astype(np.float32)
    c["zvec"] = np.stack(
        [g1f * (W2f @ a2s), g1f * (W2f @ a2d)], axis=1
    ).astype(BF)  # [D, 2]
    c["w2asum"] = (W2f @ (a2s + a2d)).reshape(D, 1).astype(np.float32)
    c["g2fcf"] = (
        w["g2"].astype(np.float32) * w["fcf_w"].reshape(D).astype(np.float32)
    ).reshape(D, 1).astype(BF)
    c["iota256"] = np.tile(np.arange(256, dtype=np.float32), (P, 1)).astype(BF)
    c["iota128"] = np.tile(np.arange(P, dtype=np.float32), (P, 1)).astype(BF)
    c["eye128b"] = np.eye(P, dtype=np.float32).astype(BF)
    c["eye16f"] = np.eye(16, dtype=np.float32)
    c["hmap1"] = _headmap(HEADS, HID)  # [8, 128] f32
    c["ones1r"] = np.ones((1, P), dtype=np.float32)
    c["ones128c"] = np.ones((P, 1), dtype=np.float32)
    return c


# ======================================================================
# device program
# ======================================================================
def _chunks(total, width):
    out = []
    o = 0
    while o < total:
        w = min(width, total - o)
        out.append((o, w))
        o += w
    return out


def build_program(meta, debug=False):
    npd, nb, t = meta["npd"], meta["nb"], meta["t"]
    ncores = meta["ncores"]
    mcap = meta["mcap"]
    tile2blk = meta["tile2blk"]
    blk_first = meta["blk_first"]
    blk_last = meta["blk_last"]
    n_glob = meta["n"]
    nl = meta["nl"]
    rg = [list(range(ncores))]
    ch512 = _chunks(npd, 512)
    ln_cnt = float(n_glob * D)  # real elements for graph-LN stats

    nc = bacc.Bacc(
        "TRN2",
        target_bir_lowering=False,
        debug=False,
        enable_asserts=False,
        num_devices=ncores,
        num_swdge_queues=2 if QSPLIT else 1,
    )

    def inp(name, shape, dt):
        return nc.dram_tensor(name, shape, dt, kind="ExternalInput").ap()

    xT = inp("xT", [D_IN, npd], F32)
    gidx = inp("gidx", [P, t], I32)
    drel = inp("drel", [P, t], F32)
    slot = inp("slot", [P, nb], F32)
    iota128 = inp("iota128", [P, P], BF16)
    W1 = inp("W1", [D_IN, D], F32)
    a1blk = inp("a1blk", [D, 2 * HEADS], BF16)
    b1c = inp("b1c", [D, 1], F32)
    W2 = inp("W2", [D, D], BF16)
    a2blk = inp("a2blk", [D, 2], BF16)
    b2c = inp("b2c", [D, 1], F32)
    zvec = inp("zvec", [D, 2], BF16)
    w2asum = inp("w2asum", [D, 1], F32)
    g2fcf = inp("g2fcf", [D, 1], BF16)
    fcw = {i: inp(f"fc{i}w", [D, D], BF16) for i in (1, 2, 3, 4)}
    fcb = {i: inp(f"fc{i}b", [D, 1], F32) for i in (1, 2, 3, 4)}
    g1c = inp("g1c", [D, 1], F32)
    be1c = inp("be1c", [D, 1], F32)
    g2c = inp("g2c", [D, 1], F32)
    be2c = inp("be2c", [D, 1], F32)
    fcfw = inp("fcfw", [D, 1], BF16)
    iota256 = inp("iota256", [P, 256], BF16)
    eye128b = inp("eye128b", [P, P], BF16)
    eye16f = inp("eye16f", [16, 16], F32)
    hmap1 = inp("hmap1", [HEADS, D], F32)
    ones1r = inp("ones1r", [1, P], F32)
    ones128c = inp("ones128c", [P, 1], F32)

    pool_out = nc.dram_tensor("pool_out", [256, 1], F32, kind="ExternalOutput").ap()
    ln2sums = nc.dram_tensor("ln2sums", [1, 2], F32, kind="ExternalOutput").ap()
    dbg_out = None
    dbg1_out = None
    if debug:
        dbg_out = nc.dram_tensor("dbg_out", [D, npd], F32, kind="ExternalOutput").ap()
        dbg1_out = nc.dram_tensor("dbg1_out", [D, npd], F32, kind="ExternalOutput").ap()

    TW = TBLW  # 256 bf16 = 512B rows: h(128) | al_s f32-pairs | pad

    with tile.TileContext(nc) as tc, ExitStack() as top:
        dram = top.enter_context(tc.tile_pool(name="dram", bufs=1, space="DRAM"))
        persist = top.enter_context(tc.tile_pool(name="persist", bufs=1))
        cpool = top.enter_context(tc.tile_pool(name="consts", bufs=1))
        cbp = top.enter_context(tc.tile_pool(name="cbp", bufs=2, space="PSUM"))
        cbs = top.enter_context(tc.tile_pool(name="cbs", bufs=2))

        def scr():  # one-bank PSUM scratch, callers slice columns
            return cbp.tile([P, 512], F32, tag="scr", name="scr")

        tbl1_loc = dram.tile([npd, TW], BF16, tag="tbl1_loc")
        tbl1_full = dram.tile([ncores * npd, TW], BF16, tag="tbl1_full", addr_space="Shared")
        tbl2_loc = dram.tile([npd, TW], BF16, tag="tbl2_loc")
        tbl2_full = dram.tile([ncores * npd, TW], BF16, tag="tbl2_full", addr_space="Shared")

        ln_in = dram.tile([1, 2], F32, tag="ln_in")
        ln_out = dram.tile([1, 2], F32, tag="ln_out", addr_space="Shared")
        ln_in2 = dram.tile([1, 2], F32, tag="ln_in2")
        ln_out2 = dram.tile([1, 2], F32, tag="ln_out2", addr_space="Shared")

        # persistent activations (transposed [feat, node]) and edge-index tables
        hc_sb = persist.tile([P, npd], BF16, tag="hc_sb")  # residual
        hf_sb = persist.tile([P, npd], BF16, tag="hf_sb")  # working activation
        drel_sb = persist.tile([P, t], F32, tag="drel_sb")
        gidx_sb = persist.tile([P, t], I32, tag="gidx_sb")
        ald1_sb = persist.tile([P, nb * HEADS], BF16, tag="ald1_sb")
        ald2_sb = persist.tile([P, nb], BF16, tag="ald2_sb")
        als1_sb = persist.tile([P, nb * HEADS], BF16, tag="als1_sb")
        als2_sb = persist.tile([P, nb], BF16, tag="als2_sb")
        hnm_sb = persist.tile([P, npd], BF16, tag="hnm_sb")  # node-major h
        # LN1-derived affine terms (filled post-E1, consumed in E2)
        sc1_sb = persist.tile([P, 1], F32, tag="sc1_sb")
        ofs1_sb = persist.tile([P, 1], F32, tag="ofs1_sb")
        rstd1_sb = persist.tile([P, 1], F32, tag="rstd1_sb")
        cc_sb = persist.tile([P, 1], F32, tag="cc_sb")
        c2_sb = persist.tile([P, 1], F32, tag="c2_sb")
        ofsrow_sb = persist.tile([1, P], F32, tag="ofsrow_sb")
        poolacc_sb = persist.tile([P, 2], F32, tag="poolacc_sb")
        nc.gpsimd.memset(poolacc_sb[:], 0.0)

        # constants in SBUF
        def cload(ap_in, shape, dt, tag):
            s = cpool.tile(shape, dt, tag=tag)
            nc.sync.dma_start(out=s[:], in_=ap_in)
            return s

        W1_s = cload(W1, [D_IN, D], F32, "W1")
        a1_s = cload(a1blk, [D, 2 * HEADS], BF16, "a1")
        a2_s = cload(a2blk, [D, 2], BF16, "a2")
        b1_s = cload(b1c, [D, 1], F32, "b1")
        b2_s = cload(b2c, [D, 1], F32, "b2")
        zvec_s = cload(zvec, [D, 2], BF16, "zvec")
        w2asum_s = cload(w2asum, [D, 1], F32, "w2asum")
        g2fcf_s = cload(g2fcf, [D, 1], BF16, "g2fcf")
        fcw_s = {i: cload(fcw[i], [D, D], BF16, f"fw{i}") for i in (1, 2, 3, 4)}
        fcb_s = {i: cload(fcb[i], [D, 1], F32, f"fb{i}") for i in (1, 2, 3, 4)}
        g1_s = cload(g1c, [D, 1], F32, "g1")
        be1_s = cload(be1c, [D, 1], F32, "be1")
        g2_s = cload(g2c, [D, 1], F32, "g2")
        be2_s = cload(be2c, [D, 1], F32, "be2")
        W2_s = cload(W2, [D, D], BF16, "W2")
        fcf_s = cload(fcfw, [D, 1], BF16, "fcf")
        io256_s = cload(iota256, [P, 256], BF16, "io256")
        io128_s = cload(iota128, [P, P], BF16, "io128")
        eye_s = cload(eye128b, [P, P], BF16, "eye")
        eye16_s = cload(eye16f, [16, 16], F32, "eye16")
        hmap1_s = cload(hmap1, [HEADS, D], F32, "hmap1")
        ones1r_s = cload(ones1r, [1, P], F32, "ones1r")
        ones128c_s = cload(ones128c, [P, 1], F32, "ones128c")
        slot_s = cload(slot, [P, nb], F32, "slot")

        nc.sync.dma_start(out=drel_sb[:], in_=drel)
        nc.sync.dma_start(out=gidx_sb[:], in_=gidx)

        # ---------------- stage A for a GAT layer: build tables ----------------
        def stage_a(layer, src_hT_fn):
            """Write tbl{layer}_loc rows [h bf16 | al_s f32] and ald table; then AllGather.
            src_hT_fn(c0, w, ps): fills psum tile [128, w] with this layer's hT chunk."""
            heads = HEADS if layer == 1 else 1
            a_s = a1_s if layer == 1 else a2_s
            tbl_loc = tbl1_loc if layer == 1 else tbl2_loc
            tbl_full = tbl1_full if layer == 1 else tbl2_full
            ald_sb = ald1_sb if layer == 1 else ald2_sb
            als_sb = als1_sb if layer == 1 else als2_sb
            with (
                tc.tile_pool(name=f"sa{layer}", bufs=5) as sp,
                tc.tile_pool(name=f"sap{layer}", bufs=2, space="PSUM") as pp,
            ):
                for c0, w in ch512:
                    ps = pp.tile([P, 512], F32, tag="hps")
                    src_hT_fn(c0, w, ps)
                    # keep transposed activation for downstream dense chain
                    nc.vector.tensor_copy(out=hf_sb[:, c0 : c0 + w], in_=ps[:, :w])
                for k in range(nb):
                    c0 = k * P
                    s_ = scr()
                    # al_s/al_d for this chunk: [2*heads, 128] = a^T @ hT
                    alps = s_[0 : 2 * heads, 0:P]
                    nc.tensor.matmul(
                        out=alps, lhsT=a_s[:], rhs=hf_sb[:, c0 : c0 + P],
                        start=True, stop=True,
                    )
                    asb = sp.tile([2 * heads, P], F32, tag="asb")
                    nc.vector.tensor_copy(out=asb[:], in_=alps)
                    # transpose -> [128, 2*heads]
                    atp = s_[:, P : P + 2 * heads]
                    nc.tensor.transpose(
                        out=atp, in_=asb[:], identity=eye16_s[: 2 * heads, : 2 * heads]
                    )
                    ats = sp.tile([P, 2 * heads], F32, tag="ats")
                    nc.vector.tensor_copy(out=ats[:], in_=atp)
                    # transpose h chunk -> node-major [128n, 128f]
                    htp = s_[:].bitcast(BF16)[:, 320:448]
                    nc.tensor.transpose(
                        out=htp, in_=hf_sb[:, c0 : c0 + P], identity=eye_s[:]
                    )
                    nc.vector.tensor_copy(
                        out=hnm_sb[:, c0 : c0 + P], in_=htp
                    )
                    # table writes
                    nc.sync.dma_start(
                        out=tbl_loc[c0 : c0 + P, 0:D], in_=hnm_sb[:, c0 : c0 + P]
                    )
                    nc.sync.dma_start(
                        out=tbl_loc[c0 : c0 + P, D : D + 2 * heads],
                        in_=ats[:].bitcast(BF16)[:, 0 : 2 * heads],
                    )
                    nc.vector.tensor_copy(
                        out=ald_sb[:, k * heads : (k + 1) * heads],
                        in_=ats[:, heads : 2 * heads],
                    )
                    nc.vector.tensor_copy(
                        out=als_sb[:, k * heads : (k + 1) * heads],
                        in_=ats[:, 0:heads],
                    )
            nc.gpsimd.collective_compute(
                "AllGather",
                ALU.bypass,
                replica_groups=rg,
                ins=[tbl_loc[:].opt()],
                outs=[tbl_full[:].opt()],
            )

        # ---------------- edge phase for a GAT layer ----------------
        def edge_phase(layer, out_cb):
            """GAT aggregation; out_cb(b, osb) gets [128f, 128n] f32 sbuf tile.

            Per tile: one indirect row-gather (h|al_s), one-hot S on DVE,
            St = transpose(S) on PE feeds the al_d expansion matmul; agg/den
            matmuls accumulate into quad-packed PSUM banks (4 blocks/bank).
            """
            heads = HEADS if layer == 1 else 1
            tbl_full = tbl1_full if layer == 1 else tbl2_full
            ald_sb = ald1_sb if layer == 1 else ald2_sb
            als_sb = als1_sb if layer == 1 else als2_sb
            nwin = (t + WIN - 1) // WIN
            agg_banks = {}
            den_banks = {}
            with (
                tc.tile_pool(name=f"eg{layer}", bufs=3) as gp,
                tc.tile_pool(name=f"eb{layer}", bufs=2) as bigp,
                tc.tile_pool(name=f"es{layer}", bufs=3) as sp,
                tc.tile_pool(name=f"est{layer}", bufs=WIN + 4) as stpool,
                tc.tile_pool(name=f"ef{layer}", bufs=2) as fin,
                tc.tile_pool(name=f"ep{layer}", bufs=2, space="PSUM") as pp,
                tc.tile_pool(name=f"ed{layer}", bufs=1, space="PSUM") as dp,
                tc.tile_pool(name=f"ea{layer}", bufs=1, space="PSUM") as ap_,
                tc.tile_pool(name=f"et{layer}", bufs=1, space="PSUM") as tp_,
                tc.tile_pool(name=f"er{layer}", bufs=1, space="PSUM") as rp,
            ):
                # first-touch memset so never-gathered pad partitions stay finite
                for _ in range(3):
                    gm = gp.tile([P, WIN * TW], BF16, tag="gath")
                    nc.gpsimd.memset(gm[:], 0.0)
                for wi in range(nwin):
                    t0 = wi * WIN
                    L = min(WIN, t - t0)
                    gath = gp.tile([P, WIN * TW], BF16, tag="gath")
                    for i in range(L):
                        m = mcap[t0 + i]
                        bi = nc.gpsimd.indirect_dma_start(
                            out=gath[0:m, i * TW : (i + 1) * TW],
                            out_offset=None,
                            in_=tbl_full[:],
                            in_offset=IndirectOffsetOnAxis(
                                ap=gidx_sb[0:m, t0 + i : t0 + i + 1], axis=0
                            ),
                        )
                        if QSPLIT and (t0 + i) % 2 == 1:
                            bi.ins.queue = "qPoolDynamic1"
                    aldw = ap_.tile([P, WIN * heads], F32, tag="aldw")
                    st_list = []
                    for i in range(L):
                        ti = t0 + i
                        s_t = stpool.tile([P, P], BF16, tag="s_t")
                        nc.vector.tensor_scalar(
                            out=s_t[:], in0=io128_s[:],
                            scalar1=drel_sb[:, ti : ti + 1], scalar2=None,
                            op0=ALU.is_equal,
                        )
                        st_list.append(s_t)
                    # batched St: 4 transposes share one PSUM bank, one ACT copy
                    for g0 in range(0, L, 4):
                        gl = min(4, L - g0)
                        stp = tp_.tile([P, 4 * P], BF16, tag="stp")
                        for j in range(gl):
                            nc.tensor.matmul(
                                out=stp[:, j * P : (j + 1) * P],
                                lhsT=st_list[g0 + j][:],
                                rhs=eye_s[:],
                                is_transpose=True,
                                start=(j == 0),
                                stop=(j == gl - 1),
                                skip_group_check=True,
                            )
                        sts = sp.tile([P, 4 * P], BF16, tag="sts")
                        nc.scalar.activation(
                            out=sts[:, 0 : gl * P], in_=stp[:, 0 : gl * P],
                            func=ACTF.Copy,
                        )
                        for j in range(gl):
                            i = g0 + j
                            b = tile2blk[t0 + i]
                            nc.tensor.matmul(
                                out=aldw[:, i * heads : (i + 1) * heads],
                                lhsT=sts[:, j * P : (j + 1) * P],
                                rhs=ald_sb[:, b * heads : (b + 1) * heads],
                                start=(i == 0),
                                stop=(i == L - 1),
                                skip_group_check=True,
                            )
                    # e = al_s[src] + al_d[dst]
                    als_v = (
                        gath[:]
                        .bitcast(F32)
                        .rearrange("p (t w) -> p t w", w=TW // 2)[
                            :, 0:L, D // 2 : D // 2 + heads
                        ]
                    )
                    e_sb = sp.tile([P, WIN * heads], F32, tag="e_sb")
                    nc.vector.tensor_tensor(
                        out=e_sb[:, 0 : L * heads].rearrange(
                            "p (t h) -> p t h", h=heads
                        ),
                        in0=als_v,
                        in1=aldw[:, 0 : L * heads].rearrange(
                            "p (t h) -> p t h", h=heads
                        ),
                        op=ALU.add,
                    )
                    if layer == 2:
                        # raw z-logits -> true logits: e = rstd1*e + CC
                        nc.vector.tensor_scalar(
                            out=e_sb[:, 0 : L * heads],
                            in0=e_sb[:, 0 : L * heads],
                            scalar1=rstd1_sb[:], scalar2=cc_sb[:],
                            op0=ALU.mult, op1=ALU.add,
                        )
                    t02 = sp.tile([P, WIN * heads], F32, tag="t02")
                    nc.vector.tensor_scalar(
                        out=t02[:, 0 : L * heads], in0=e_sb[:, 0 : L * heads],
                        scalar1=NEG_SLOPE, scalar2=None, op0=ALU.mult,
                    )
                    lr_sb = sp.tile([P, WIN * heads], F32, tag="lr_sb")
                    nc.vector.tensor_tensor(
                        out=lr_sb[:, 0 : L * heads], in0=e_sb[:, 0 : L * heads],
                        in1=t02[:, 0 : L * heads], op=ALU.max,
                    )
                    pexp = bigp.tile([P, WIN * D], BF16, tag="pexp")
                    nc.scalar.activation(
                        out=pexp[:, 0 : L * D].rearrange(
                            "p (t h c) -> p t h c", h=heads, c=D // heads
                        ),
                        in_=lr_sb[:, 0 : L * heads]
                        .rearrange("p (t h) -> p t h", h=heads)
                        .broadcast_to((P, L, heads, D // heads)),
                        func=ACTF.Exp,
                    )
                    p_sb = sp.tile([P, WIN * heads], BF16, tag="p_sb")
                    nc.scalar.activation(
                        out=p_sb[:, 0 : L * heads], in_=lr_sb[:, 0 : L * heads],
                        func=ACTF.Exp,
                    )
                    wh = bigp.tile([P, WIN * D], BF16, tag="wh")
                    nc.vector.tensor_tensor(
                        out=wh[:, 0 : L * D].rearrange("p (t c) -> p t c", c=D),
                        in0=gath[:].rearrange("p (t w) -> p t w", w=TW)[:, 0:L, 0:D],
                        in1=pexp[:, 0 : L * D].rearrange("p (t c) -> p t c", c=D),
                        op=ALU.mult,
                    )
                    for i in range(L):
                        ti = t0 + i
                        b = tile2blk[ti]
                        q = b // 4  # quad id
                        qi = b % 4
                        if q not in agg_banks:
                            qblocks = [bb for bb in range(4 * q, min(4 * q + 4, nb))]
                            ab = pp.tile([P, 512], F32, tag="aggq", name="aggq")
                            db = dp.tile([8, 512], F32, tag="denq", name="denq")
                            agg_banks[q] = (
                                ab, blk_first[qblocks[0]], blk_last[qblocks[-1]]
                            )
                            den_banks[q] = (
                                db, blk_first[qblocks[0]], blk_last[qblocks[-1]]
                            )
                        ab, qf, ql = agg_banks[q]
                        db, _, _ = den_banks[q]
                        nc.tensor.matmul(
                            out=ab[:, qi * P : (qi + 1) * P],
                            lhsT=wh[:, i * D : (i + 1) * D],
                            rhs=st_list[i][:],
                            start=(ti == qf),
                            stop=False,
                            skip_group_check=True,
                        )
                        nc.tensor.matmul(
                            out=db[0:heads, qi * P : (qi + 1) * P],
                            lhsT=p_sb[:, i * heads : (i + 1) * heads],
                            rhs=st_list[i][:],
                            start=(ti == qf),
                            stop=False,
                            skip_group_check=True,
                        )
                        if ti == blk_last[b]:
                            # diagonal (self-loop) contribution: h and al are
                            # local, injected as one pseudo-tile (S = identity)
                            esf = fin.tile([P, HEADS], F32, tag="esf")
                            nc.vector.tensor_tensor(
                                out=esf[:, 0:heads],
                                in0=als_sb[:, b * heads : (b + 1) * heads],
                                in1=ald_sb[:, b * heads : (b + 1) * heads],
                                op=ALU.add,
                            )
                            if layer == 2:
                                nc.vector.tensor_scalar(
                                    out=esf[:, 0:heads], in0=esf[:, 0:heads],
                                    scalar1=rstd1_sb[:], scalar2=cc_sb[:],
                                    op0=ALU.mult, op1=ALU.add,
                                )
                            tsf = fin.tile([P, HEADS], F32, tag="tsf")
                            nc.vector.tensor_scalar(
                                out=tsf[:, 0:heads], in0=esf[:, 0:heads],
                                scalar1=NEG_SLOPE, scalar2=None, op0=ALU.mult,
                            )
                            lrs = fin.tile([P, HEADS], F32, tag="lrs")
                            nc.vector.tensor_tensor(
                                out=lrs[:, 0:heads], in0=esf[:, 0:heads],
                                in1=tsf[:, 0:heads], op=ALU.max,
                            )
                            pxs = fin.tile([P, D], BF16, tag="pxs")
                            nc.scalar.activation(
                                out=pxs[:].rearrange("p (h c) -> p h c", h=heads),
                                in_=lrs[:, 0:heads].broadcast_to(
                                    (P, heads, D // heads)
                                ),
                                func=ACTF.Exp,
                            )
                            psbs = fin.tile([P, HEADS], BF16, tag="psbs")
                            nc.scalar.activation(
                                out=psbs[:, 0:heads], in_=lrs[:, 0:heads],
                                func=ACTF.Exp,
                            )
                            whs = fin.tile([P, D], BF16, tag="whs")
                            nc.vector.tensor_tensor(
                                out=whs[:], in0=hnm_sb[:, b * P : (b + 1) * P],
                                in1=pxs[:], op=ALU.mult,
                            )
                            nc.tensor.matmul(
                                out=ab[:, qi * P : (qi + 1) * P],
                                lhsT=whs[:], rhs=eye_s[:],
                                start=False, stop=(ti == ql),
                                skip_group_check=True,
                            )
                            nc.tensor.matmul(
                                out=db[0:heads, qi * P : (qi + 1) * P],
                                lhsT=psbs[:, 0:heads], rhs=eye_s[:],
                                start=False, stop=(ti == ql),
                                skip_group_check=True,
                            )
                            dsb = fin.tile([heads, P], F32, tag="dsb")
                            nc.vector.tensor_scalar(
                                out=dsb[:], in0=db[0:heads, qi * P : (qi + 1) * P],
                                scalar1=1e-16, scalar2=None, op0=ALU.add,
                            )
                            rsb = fin.tile([heads, P], F32, tag="rsb")
                            nc.vector.reciprocal(out=rsb[:], in_=dsb[:])
                            rp_t = rp.tile([P, 512], F32, tag="rscr")
                            rex = rp_t[:, 0:P]
                            nc.tensor.matmul(
                                out=rex,
                                lhsT=(ones1r_s if heads == 1 else hmap1_s)[:],
                                rhs=rsb[:], start=True, stop=True,
                            )
                            rxs = fin.tile([P, P], F32, tag="rxs")
                            nc.scalar.activation(out=rxs[:], in_=rex, func=ACTF.Copy)
                            osb = fin.tile([P, P], F32, tag="osb")
                            if layer == 2:
                                # out2 = [sc1*ab + ofs1*den] @ W2 / den
                                tmul = fin.tile([P, P], F32, tag="tmul")
                                nc.vector.tensor_scalar(
                                    out=tmul[:],
                                    in0=ab[:, qi * P : (qi + 1) * P],
                                    scalar1=sc1_sb[:], scalar2=None,
                                    op0=ALU.mult,
                                )
                                t2l = fin.tile([P, P], BF16, tag="t2l")
                                nc.vector.tensor_copy(out=t2l[:], in_=tmul[:])
                                ps2 = rp_t[:, 2 * P : 3 * P]
                                nc.tensor.matmul(
                                    out=ps2, lhsT=W2_s[:], rhs=t2l[:],
                                    start=True, stop=True,
                                )
                                nc.vector.tensor_tensor(
                                    out=osb[:], in0=ps2, in1=rxs[:],
                                    op=ALU.mult,
                                )
                            else:
                                nc.vector.tensor_tensor(
                                    out=osb[:], in0=ab[:, qi * P : (qi + 1) * P],
                                    in1=rxs[:], op=ALU.mult,
                                )
                            out_cb(b, osb)
                            if b == 4 * q + 3 or b == nb - 1:
                                del agg_banks[q], den_banks[q]

        # ---------------- graph-LN over h_sb -> dst_sb (bf16) ----------------
        def graph_ln(src_sb, g_s, be_s, ln_i, ln_o, dst_sb, stash=False):
            with (
                tc.tile_pool(name="ln", bufs=2) as sp,
                tc.tile_pool(name="lnp", bufs=2, space="PSUM") as pp,
            ):
                nchunk = len(ch512)
                stats = sp.tile([P, nchunk * 6], F32, tag="stats")
                for ci, (c0, w) in enumerate(ch512):
                    nc.vector.bn_stats(
                        out=stats[:, ci * 6 : (ci + 1) * 6], in_=src_sb[:, c0 : c0 + w]
                    )
                mv = sp.tile([P, 2], F32, tag="mv")
                nc.vector.bn_aggr(
                    out=mv[:], in_=stats[:].rearrange("p (c s) -> p c s", s=6)
                )
                # per-partition sums: [sx, sxx] = [m, (v+m^2)] * npd
                sums = sp.tile([P, 2], F32, tag="sums")
                nc.vector.tensor_scalar(
                    out=sums[:, 0:1], in0=mv[:, 0:1], scalar1=float(npd),
                    scalar2=None, op0=ALU.mult,
                )
                m2 = sp.tile([P, 1], F32, tag="m2")
                nc.vector.tensor_tensor(
                    out=m2[:], in0=mv[:, 0:1], in1=mv[:, 0:1], op=ALU.mult
                )
                nc.vector.tensor_tensor(
                    out=sums[:, 1:2], in0=mv[:, 1:2], in1=m2[:], op=ALU.add
                )
                nc.vector.tensor_scalar(
                    out=sums[:, 1:2], in0=sums[:, 1:2], scalar1=float(npd),
                    scalar2=None, op0=ALU.mult,
                )
                red = pp.tile([1, 2], F32, tag="red")
                nc.tensor.matmul(
                    out=red[:], lhsT=ones128c_s[:], rhs=sums[:], start=True, stop=True
                )
                rsb = sp.tile([1, 2], F32, tag="rsb2")
                nc.vector.tensor_copy(out=rsb[:], in_=red[:])
                nc.gpsimd.dma_start(out=ln_i[:], in_=rsb[:])
                nc.gpsimd.collective_compute(
                    "AllReduce", ALU.add, replica_groups=rg,
                    ins=[ln_i[:].opt()], outs=[ln_o[:].opt()],
                )
                ar = sp.tile([1, 2], F32, tag="ar")
                nc.gpsimd.dma_start(out=ar[:], in_=ln_o[:])
                bc = pp.tile([P, 2], F32, tag="bc")
                nc.tensor.matmul(
                    out=bc[:], lhsT=ones1r_s[:], rhs=ar[:], start=True, stop=True
                )
                # mu = s1/cnt ; var = s2/cnt - mu^2 ; s = g * rsqrt(var+eps) ; b = be - mu*s
                mu = sp.tile([P, 1], F32, tag="mu")
                nc.vector.tensor_scalar(
                    out=mu[:], in0=bc[:, 0:1], scalar1=1.0 / ln_cnt, scalar2=None,
                    op0=ALU.mult,
                )
                var = sp.tile([P, 1], F32, tag="var")
                nc.vector.tensor_scalar(
                    out=var[:], in0=bc[:, 1:2], scalar1=1.0 / ln_cnt, scalar2=None,
                    op0=ALU.mult,
                )
                mu2 = sp.tile([P, 1], F32, tag="mu2")
                nc.vector.tensor_tensor(out=mu2[:], in0=mu[:], in1=mu[:], op=ALU.mult)
                nc.vector.tensor_tensor(
                    out=var[:], in0=var[:], in1=mu2[:], op=ALU.subtract
                )
                nc.vector.tensor_scalar(
                    out=var[:], in0=var[:], scalar1=LN_EPS, scalar2=None, op0=ALU.add
                )
                sd = sp.tile([P, 1], F32, tag="sd")
                nc.scalar.activation(out=sd[:], in_=var[:], func=ACTF.Sqrt)
                rstd = sp.tile([P, 1], F32, tag="rstd")
                nc.vector.reciprocal(out=rstd[:], in_=sd[:])
                sc = sp.tile([P, 1], F32, tag="sc")
                nc.vector.tensor_tensor(out=sc[:], in0=g_s[:], in1=rstd[:], op=ALU.mult)
                ofs = sp.tile([P, 1], F32, tag="ofs")
                nc.vector.tensor_tensor(out=ofs[:], in0=mu[:], in1=sc[:], op=ALU.mult)
                nc.vector.tensor_tensor(
                    out=ofs[:], in0=be_s[:], in1=ofs[:], op=ALU.subtract
                )
                if stash:
                    nc.vector.tensor_copy(out=rstd1_sb[:], in_=rstd[:])
                    nc.vector.tensor_copy(out=sc1_sb[:], in_=sc[:])
                    nc.vector.tensor_copy(out=ofs1_sb[:], in_=ofs[:])
                for c0, w in ch512:
                    nc.scalar.activation(
                        out=dst_sb[:, c0 : c0 + w], in_=src_sb[:, c0 : c0 + w],
                        func=ACTF.Identity, bias=ofs[:], scale=sc[:],
                    )
                # zero the padded node columns so they don't pollute later stats
                if npd > nl:
                    nc.gpsimd.memset(dst_sb[:, nl:npd], 0.0)

        # =========================== the network ===========================
        # ---- GAT layer 1 stage A: h1 = x @ W1 (transposed) ----
        xin_pool = top.enter_context(tc.tile_pool(name="xin", bufs=2))

        def h1_fn(c0, w, ps):
            xt = xin_pool.tile([D_IN, 512], F32, tag="xt")
            nc.sync.dma_start(out=xt[:, :w], in_=xT[:, c0 : c0 + w])
            nc.tensor.matmul(
                out=ps[:, :w], lhsT=W1_s[:], rhs=xt[:, :w], start=True, stop=True
            )

        stage_a(1, h1_fn)

        # ---- GAT layer 1 edge phase; per-block callback also runs fc1/fc2
        # and builds the raw (pre-LN) layer-2 table [h_pre | zals], so AG2
        # can start immediately when E1 drains and LN1 runs under it. ----
        def out1_cb(b, osb):
            c0 = b * P
            h1o = hc_sb[:, c0 : c0 + P]
            nc.scalar.activation(
                out=h1o, in_=osb[:], func=ACTF.Relu, bias=b1_s[:], scale=1.0,
            )
            s_ = scr()
            ps1 = s_[:, 0:P]
            nc.tensor.matmul(
                out=ps1, lhsT=fcw_s[1][:], rhs=h1o, start=True, stop=True
            )
            t1 = cbs.tile([P, P], BF16, tag="t1")
            nc.scalar.activation(
                out=t1[:], in_=ps1, func=ACTF.Relu, bias=fcb_s[1][:], scale=1.0
            )
            psf = s_[:, P : 2 * P]
            nc.tensor.matmul(
                out=psf, lhsT=fcw_s[2][:], rhs=t1[:], start=True, stop=True
            )
            t2f = cbs.tile([P, P], F32, tag="t2f")
            nc.vector.tensor_tensor(out=t2f[:], in0=psf, in1=h1o, op=ALU.add)
            nc.scalar.activation(
                out=hf_sb[:, c0 : c0 + P], in_=t2f[:], func=ACTF.Relu,
                bias=fcb_s[2][:], scale=1.0,
            )
            # raw layer-2 attention z-values
            zz = s_[0:2, 2 * P : 3 * P]
            nc.tensor.matmul(
                out=zz, lhsT=zvec_s[:], rhs=hf_sb[:, c0 : c0 + P],
                start=True, stop=True,
            )
            zzs = cbs.tile([2, P], F32, tag="zzs")
            nc.vector.tensor_copy(out=zzs[:], in_=zz)
            atp2 = s_[:, 3 * P : 3 * P + 2]
            nc.tensor.transpose(out=atp2, in_=zzs[:], identity=eye16_s[0:2, 0:2])
            ats2 = cbs.tile([P, 2], F32, tag="ats2")
            nc.vector.tensor_copy(out=ats2[:], in_=atp2)
            nc.vector.tensor_copy(out=ald2_sb[:, b : b + 1], in_=ats2[:, 1:2])
            nc.vector.tensor_copy(out=als2_sb[:, b : b + 1], in_=ats2[:, 0:1])
            htp2 = s_[:].bitcast(BF16)[:, 772:900]
            nc.tensor.transpose(
                out=htp2, in_=hf_sb[:, c0 : c0 + P], identity=eye_s[:]
            )
            nc.vector.tensor_copy(out=hnm_sb[:, c0 : c0 + P], in_=htp2)
            nc.sync.dma_start(
                out=tbl2_loc[c0 : c0 + P, 0:D], in_=hnm_sb[:, c0 : c0 + P]
            )
            nc.sync.dma_start(
                out=tbl2_loc[c0 : c0 + P, D : D + 2],
                in_=ats2[:].bitcast(BF16)[:, 0:2],
            )

        edge_phase(1, out1_cb)
        nc.gpsimd.collective_compute(
            "AllGather",
            ALU.bypass,
            replica_groups=rg,
            ins=[tbl2_loc[:].opt()],
            outs=[tbl2_full[:].opt()],
        )
        if debug:
            with tc.tile_pool(name="dbg1p", bufs=2) as dbp1:
                for c0, w in ch512:
                    dt1 = dbp1.tile([P, 512], F32, tag="dbg1")
                    nc.vector.tensor_copy(out=dt1[:, :w], in_=hc_sb[:, c0 : c0 + w])
                    nc.sync.dma_start(out=dbg1_out[:, c0 : c0 + w], in_=dt1[:, :w])

        # ---- LN1 (stats + AR + affine; runs under AG2), then the E2
        # affine constants derived from (rstd1, ofs1) ----
        graph_ln(hf_sb, g1_s, be1_s, ln_in, ln_out, hc_sb, stash=True)
        with (
            tc.tile_pool(name="epl", bufs=1) as epp,
            tc.tile_pool(name="eplp", bufs=1, space="PSUM") as epps,
        ):
            m1 = epps.tile([1, 1], F32, tag="m1")
            nc.tensor.matmul(
                out=m1[:], lhsT=ofs1_sb[:], rhs=w2asum_s[:], start=True, stop=True
            )
            m1s = epp.tile([1, 1], F32, tag="m1s")
            nc.vector.tensor_copy(out=m1s[:], in_=m1[:])
            ccp = epps.tile([P, 1], F32, tag="ccp")
            nc.tensor.matmul(
                out=ccp[:], lhsT=ones1r_s[:], rhs=m1s[:], start=True, stop=True
            )
            nc.vector.tensor_copy(out=cc_sb[:], in_=ccp[:])
            ofsb = epp.tile([P, 1], BF16, tag="ofsb")
            nc.vector.tensor_copy(out=ofsb[:], in_=ofs1_sb[:])
            c2p = epps.tile([P, 1], F32, tag="c2p")
            nc.tensor.matmul(
                out=c2p[:], lhsT=W2_s[:], rhs=ofsb[:], start=True, stop=True
            )
            c2t = epp.tile([P, 1], F32, tag="c2t")
            nc.vector.tensor_copy(out=c2t[:], in_=c2p[:])
            nc.vector.tensor_tensor(
                out=c2_sb[:], in0=c2t[:], in1=b2_s[:], op=ALU.add
            )
            orp = epps.tile([1, P], F32, tag="orp")
            nc.tensor.matmul(
                out=orp[:], lhsT=ofsb[:], rhs=eye_s[:], start=True, stop=True
            )
            nc.vector.tensor_copy(out=ofsrow_sb[:], in_=orp[:])

        # ---- GAT layer 2 edge phase (raw table; LN+W2 folded into the
        # finalize). The callback also runs fc3/fc4 (+h_ln1 residual),
        # fcf on raw h4, and the slot-pool partials; LN2's affine moves
        # to the host via raw sums. ----
        def out2_cb(b, osb):
            c0 = b * P
            h2o = cbs.tile([P, P], BF16, tag="h2o")
            nc.scalar.activation(
                out=h2o[:], in_=osb[:], func=ACTF.Identity,
                bias=c2_sb[:], scale=1.0,
            )
            s_ = scr()
            ps3 = s_[:, 0:P]
            nc.tensor.matmul(
                out=ps3, lhsT=fcw_s[3][:], rhs=h2o[:], start=True, stop=True
            )
            t3 = cbs.tile([P, P], BF16, tag="t3")
            nc.scalar.activation(
                out=t3[:], in_=ps3, func=ACTF.Relu, bias=fcb_s[3][:], scale=1.0
            )
            ps4 = s_[:, P : 2 * P]
            nc.tensor.matmul(
                out=ps4, lhsT=fcw_s[4][:], rhs=t3[:], start=True, stop=True
            )
            t4 = cbs.tile([P, P], F32, tag="t4")
            nc.vector.tensor_tensor(
                out=t4[:], in0=ps4, in1=hc_sb[:, c0 : c0 + P], op=ALU.add
            )
            nc.scalar.activation(
                out=hc_sb[:, c0 : c0 + P], in_=t4[:], func=ACTF.Identity,
                bias=fcb_s[4][:], scale=1.0,
            )
            hv = s_[:, 2 * P : 2 * P + 1]
            nc.tensor.matmul(
                out=hv, lhsT=hc_sb[:, c0 : c0 + P], rhs=g2fcf_s[:],
                start=True, stop=True,
            )
            hvs = cbs.tile([P, 1], F32, tag="hvs")
            nc.vector.tensor_copy(out=hvs[:], in_=hv)
            oh = cbs.tile([P, 256], BF16, tag="oh")
            nc.vector.tensor_scalar(
                out=oh[:], in0=io256_s[:], scalar1=slot_s[:, b : b + 1],
                scalar2=None, op0=ALU.is_equal,
            )
            ohf = cbs.tile([P, 256], F32, tag="ohf")
            nc.vector.tensor_copy(out=ohf[:], in_=oh[:])
            pqa = s_[:, 2 * P + 1 : 2 * P + 2]
            nc.tensor.matmul(
                out=pqa, lhsT=ohf[:, 0:P], rhs=hvs[:], start=True, stop=True
            )
            pqb = s_[:, 2 * P + 2 : 2 * P + 3]
            nc.tensor.matmul(
                out=pqb, lhsT=ohf[:, P:256], rhs=hvs[:], start=True, stop=True
            )
            nc.vector.tensor_tensor(
                out=poolacc_sb[:, 0:1], in0=poolacc_sb[:, 0:1], in1=pqa,
                op=ALU.add,
            )
            nc.vector.tensor_tensor(
                out=poolacc_sb[:, 1:2], in0=poolacc_sb[:, 1:2], in1=pqb,
                op=ALU.add,
            )

        edge_phase(2, out2_cb)

        # ---- tail: LN2 raw sums -> host; pool partials -> pool_out ----
        with (
            tc.tile_pool(name="l2s", bufs=2) as sp2,
            tc.tile_pool(name="l2sp", bufs=1, space="PSUM") as pq2,
        ):
            if npd > nl:
                nc.gpsimd.memset(hc_sb[:, nl:npd], 0.0)
            nchunk = len(ch512)
            stats = sp2.tile([P, nchunk * 6], F32, tag="stats2")
            for ci, (c0, w) in enumerate(ch512):
                nc.vector.bn_stats(
                    out=stats[:, ci * 6 : (ci + 1) * 6], in_=hc_sb[:, c0 : c0 + w]
                )
            mv = sp2.tile([P, 2], F32, tag="mv2")
            nc.vector.bn_aggr(
                out=mv[:], in_=stats[:].rearrange("p (c s) -> p c s", s=6)
            )
            sums = sp2.tile([P, 2], F32, tag="sums2")
            nc.vector.tensor_scalar(
                out=sums[:, 0:1], in0=mv[:, 0:1], scalar1=float(npd),
                scalar2=None, op0=ALU.mult,
            )
            m2 = sp2.tile([P, 1], F32, tag="m2b")
            nc.vector.tensor_tensor(
                out=m2[:], in0=mv[:, 0:1], in1=mv[:, 0:1], op=ALU.mult
            )
            nc.vector.tensor_tensor(
                out=sums[:, 1:2], in0=mv[:, 1:2], in1=m2[:], op=ALU.add
            )
            nc.vector.tensor_scalar(
                out=sums[:, 1:2], in0=sums[:, 1:2], scalar1=float(npd),
                scalar2=None, op0=ALU.mult,
            )
            red = pq2.tile([1, 2], F32, tag="red2")
            nc.tensor.matmul(
                out=red[:], lhsT=ones128c_s[:], rhs=sums[:], start=True, stop=True
            )
            rsb2 = sp2.tile([1, 2], F32, tag="rsb2b")
            nc.vector.tensor_copy(out=rsb2[:], in_=red[:])
            nc.sync.dma_start(out=ln2sums, in_=rsb2[:])
            nc.sync.dma_start(out=pool_out[0:P, 0:1], in_=poolacc_sb[:, 0:1])
            nc.sync.dma_start(out=pool_out[P:256, 0:1], in_=poolacc_sb[:, 1:2])

    nc.compile()
    return nc


# ======================================================================
# driver
# ======================================================================
def _in_maps(meta, core_arrays, slots, consts, x):
    nl, npd = meta["nl"], meta["npd"]
    maps = []
    for c in range(meta["ncores"]):
        gidx_pi, drel_pi = core_arrays[c]
        xT = np.zeros((D_IN, npd), dtype=np.float32)
        xT[:, :nl] = np.asarray(x[c * nl : (c + 1) * nl], dtype=np.float32).T
        m = dict(
            xT=xT, gidx=gidx_pi, drel=drel_pi, slot=slots[c]
        )
        m.update(consts)
        maps.append(m)
    return maps


def _install_ntff_shim():
    """Provide antenv.axon_hooks via direct ctypes into libaxon_pjrt.so."""
    import types, contextlib, ctypes

    try:
        import antenv.axon_hooks  # noqa: F401

        return True
    except ImportError:
        pass
    so_path = "/opt/axon/libaxon_pjrt.so"
    try:
        lib = ctypes.CDLL(so_path)
    except OSError:
        return False
    if not hasattr(lib, "axon_start_nrt_profile"):
        return False
    lib.axon_start_nrt_profile.argtypes = [
        ctypes.POINTER(ctypes.c_int64),
        ctypes.c_size_t,
    ]
    lib.axon_start_nrt_profile.restype = ctypes.c_int64
    lib.axon_stop_nrt_profile.argtypes = [ctypes.c_char_p]
    lib.axon_stop_nrt_profile.restype = ctypes.c_int64

    @contextlib.contextmanager
    def _hook(output_dir, device_ids):
        import jax

        jax.devices()
        if device_ids:
            ids = (ctypes.c_int64 * len(device_ids))(*device_ids)
            rc = lib.axon_start_nrt_profile(ids, len(device_ids))
        else:
            rc = lib.axon_start_nrt_profile(None, 0)
        if rc != 0:
            raise RuntimeError(f"axon_start_nrt_profile rc={rc}")
        try:
            yield
        finally:
            nfiles = lib.axon_stop_nrt_profile(str(output_dir).encode())
            print(f"ntff profile: {nfiles} file(s) -> {output_dir}", file=sys.stderr)

    mod = types.ModuleType("antenv.axon_hooks")
    mod.get_axon_ntff_profile_hook = lambda: _hook
    mod.set_axon_ntff_profile_hook = lambda h: None
    import antenv

    antenv.axon_hooks = mod
    sys.modules["antenv.axon_hooks"] = mod
    return True


def run(inputs, debug=False, trace=False):
    if trace:
        trace = _install_ntff_shim()
    x = np.asarray(inputs["x"])
    edge_index = np.asarray(inputs["edge_index"])
    batch = np.asarray(inputs["batch"])
    meta, core_arrays, slots, g0s, counts = _prep(x, edge_index, batch)
    weights = {
        k: np.asarray(v)
        for k, v in inputs.items()
        if k not in ("x", "edge_index", "batch")
    }
    consts = _consts(weights, meta)
    nc = build_program(meta, debug=debug)
    maps = _in_maps(meta, core_arrays, slots, consts, x)

    hw = get_hw_module(nc.m)
    old = nc.m
    nc.m = hw
    try:
        res = bass_utils.run_bass_kernel_spmd(
            nc, maps, core_ids=list(range(meta["ncores"])), trace=trace
        )
    finally:
        nc.m = old

    # host unshard: assemble per-graph q-sums from per-core slot partials,
    # then apply the (linear) graph-LN2 + fcf affine: per node
    # LN2(h4)@fcf = rstd2*q + ofs2@fcf with q = h4@(g2*fcf).
    sums = np.zeros((G,), dtype=np.float64)
    sx = sxx = 0.0
    for c in range(meta["ncores"]):
        part = np.asarray(res.results[c]["pool_out"], dtype=np.float64).reshape(256)
        g0 = g0s[c]
        hi = min(256, G - g0)
        sums[g0 : g0 + hi] += part[:hi]
        ls = np.asarray(res.results[c]["ln2sums"], dtype=np.float64).reshape(2)
        sx += ls[0]
        sxx += ls[1]
    cnt_ln = float(N * D)
    mu2 = sx / cnt_ln
    var2 = sxx / cnt_ln - mu2 * mu2
    rstd2 = 1.0 / math.sqrt(var2 + LN_EPS)
    g2 = np.asarray(inputs["g2"], dtype=np.float64)
    beta2 = np.asarray(inputs["beta2"], dtype=np.float64)
    fcfw = np.asarray(inputs["fcf_w"], dtype=np.float64).reshape(D)
    c2f = float(((beta2 - mu2 * rstd2 * g2) * fcfw).sum())
    fcf_b = float(np.asarray(inputs["fcf_b"]).reshape(-1)[0])
    out = (rstd2 * sums + c2f * counts) / np.maximum(counts, 1.0) + fcf_b
    return out.astype(np.float32).reshape(G, 1), res


def kernel(**inputs):
    out, _ = run(inputs)
    return out



# revision 55
# speedup vs baseline: 1.1351x; 1.1351x over previous
"""GAT (2-layer, PyG-style) + MLP + graph-LN + global mean pool on 8 Trainium2 cores.

Strategy (sharding_hint): nodes partitioned contiguously across the 8 cores;
edges partitioned by destination node (1-D graph partition, host-sorted by dst);
the per-layer node-feature table [h | a_src-logit] is AllGathered so each core
gathers h[src] rows for its local edges with indirect DMA; per-destination
softmax + weighted aggregation is done with one-hot scatter matmuls
accumulating in PSUM per 128-node block; graph-LayerNorm statistics and are
combined with a tiny AllReduce; the final global_mean_pool partial sums per
core are assembled on the host (unshard step).
"""

import os
import sys

sys.path.insert(0, "/opt/trn_rl_repo")

import math
from contextlib import ExitStack

QSPLIT = os.environ.get("QSPLIT", "0") == "1"  # alternate SWDGE queues for gathers

import numpy as np
import ml_dtypes

BF = ml_dtypes.bfloat16

import concourse.bass as bass
import concourse.bacc as bacc
import concourse.tile as tile
import concourse.mybir as mybir
from concourse import bass_utils
from concourse.bass import IndirectOffsetOnAxis
from concourse.bass_interp import get_hw_module

F32 = mybir.dt.float32
BF16 = mybir.dt.bfloat16
I32 = mybir.dt.int32
I16 = mybir.dt.int16
ALU = mybir.AluOpType
ACTF = mybir.ActivationFunctionType

# ---- problem constants (hardcoded per spec) ----
N = 100000
E_RAW = 1600000
D_IN = 9
HID = 16
HEADS = 8
D = 128
G = 1000
NCORES = 8
NEG_SLOPE = 0.2
LN_EPS = 1e-5
TBLW = 144  # table row width in bf16 (288B rows: h 128 | al_s f32-pairs)
WIN = 32  # edge tiles per gather window
P = 128

PAD_DREL = 300.0  # one-hot never matches (iota is 0..127)
PAD_SLOT = 300.0


# ======================================================================
# host-side prep: edge sort / partition / padding, index layouts, weights
# ======================================================================
def _prep(x, edge_index, batch, n=N, e_raw=E_RAW, ncores=NCORES, g=G):
    nl = n // ncores  # owned nodes per core
    npd = ((nl + P - 1) // P) * P
    nb = npd // P  # 128-node blocks per core

    # self-loops are handled by an on-chip diagonal fast path (h and al
    # are core-local), so only the raw edges go through the gather.
    src = edge_index[0].astype(np.int64)
    dst = edge_index[1].astype(np.int64)
    order = np.argsort(dst, kind="stable")
    srcs = src[order]
    dsts = dst[order]

    bounds = np.searchsorted(dsts, np.arange(ncores + 1) * nl)
    per_core = []
    cnt = np.zeros((ncores, nb), dtype=np.int64)
    for c in range(ncores):
        s_c = srcs[bounds[c] : bounds[c + 1]]
        d_c = dsts[bounds[c] : bounds[c + 1]] - c * nl
        per_core.append((s_c, d_c))
        cnt[c] = np.bincount(d_c // P, minlength=nb)
    maxe = cnt.max(axis=0)
    cap = ((maxe + P - 1) // P) * P  # edge slots per block
    cap = np.maximum(cap, P)
    ktiles = (cap // P).astype(np.int64)
    t_total = int(ktiles.sum())
    blk_starts = np.concatenate([[0], np.cumsum(ktiles)])
    tile2blk = []
    mcap = []  # valid gather rows per tile (max over cores, pads skipped)
    for b in range(nb):
        tile2blk += [b] * int(ktiles[b])
        for i in range(int(ktiles[b])):
            mcap.append(int(min(P, max(1, maxe[b] - i * P))))
    blk_first = {b: int(blk_starts[b]) for b in range(nb)}
    blk_last = {b: int(blk_starts[b + 1]) - 1 for b in range(nb)}

    core_arrays = []
    for c in range(ncores):
        s_c, d_c = per_core[c]
        gidx = np.zeros((t_total * P,), dtype=np.int32)
        drel = np.full((t_total * P,), PAD_DREL, dtype=np.float32)
        blk = d_c // P
        cstart = np.concatenate([[0], np.cumsum(np.bincount(blk, minlength=nb))])
        for b in range(nb):
            e0, e1 = cstart[b], cstart[b + 1]
            o0 = int(blk_starts[b]) * P
            m = e1 - e0
            sc = s_c[e0:e1]
            gidx[o0 : o0 + m] = ((sc // nl) * npd + (sc % nl)).astype(np.int32)
            drel[o0 : o0 + m] = (d_c[e0:e1] % P).astype(np.float32)
        core_arrays.append(
            (
                gidx.reshape(t_total, P).T.copy(),
                drel.reshape(t_total, P).T.copy(),
            )
        )

    # pool slots
    bsort = np.asarray(batch, dtype=np.int64)
    slots = []
    g0s = []
    counts = np.bincount(bsort, minlength=g).astype(np.float64)
    for c in range(ncores):
        bs = bsort[c * nl : (c + 1) * nl]
        g0 = int(bs[0])
        sl = np.full((npd,), PAD_SLOT, dtype=np.float32)
        sl[:nl] = (bs - g0).astype(np.float32)
        assert sl[:nl].max() < 256, "graph-slot overflow"
        slots.append(sl.reshape(nb, P).T.copy())
        g0s.append(g0)

    meta = dict(
        n=n, nl=nl, npd=npd, nb=nb, t=t_total, mcap=mcap,
        tile2blk=tile2blk, blk_first=blk_first, blk_last=blk_last,
        ncores=ncores, g=g,
    )
    return meta, core_arrays, slots, g0s, counts


def _blockdiag(a):  # a [H, C] -> [H*C, H]
    h, c = a.shape
    out = np.zeros((h * c, h), dtype=np.float32)
    for i in range(h):
        out[i * c : (i + 1) * c, i] = a[i]
    return out


def _headmap(heads, ch):  # [H, H*C] one-hot expansion map
    out = np.zeros((heads, heads * ch), dtype=np.float32)
    for i in range(heads):
        out[i, i * ch : (i + 1) * ch] = 1.0
    return out


def _consts(weights, meta):
    """Replicated (same every core) input arrays."""
    w = weights
    c = {}
    c["W1"] = w["W1"].astype(np.float32)  # [9, 128]
    c["a1blk"] = np.concatenate(
        [_blockdiag(w["a_src1"]), _blockdiag(w["a_dst1"])], axis=1
    ).astype(BF)  # [128, 16]
    c["b1c"] = w["b1"].reshape(D, 1).astype(np.float32)
    c["W2"] = w["W2"].astype(BF)  # [128,128] lhsT
    c["a2blk"] = np.concatenate(
        [w["a_src2"].reshape(D, 1), w["a_dst2"].reshape(D, 1)], axis=1
    ).astype(BF)  # [128, 2]
    c["b2c"] = w["b2"].reshape(D, 1).astype(np.float32)
    for i in (1, 2, 3, 4):
        c[f"fc{i}w"] = w[f"fc{i}_w"].astype(BF)
        c[f"fc{i}b"] = w[f"fc{i}_b"].reshape(D, 1).astype(np.float32)
    c["g1c"] = w["g1"].reshape(D, 1).astype(np.float32)
    c["be1c"] = w["beta1"].reshape(D, 1).astype(np.float32)
    c["g2c"] = w["g2"].reshape(D, 1).astype(np.float32)
    c["be2c"] = w["beta2"].reshape(D, 1).astype(np.float32)
    c["fcfw"] = w["fcf_w"].reshape(D, 1).astype(BF)
    # layer-2 attention on raw (pre-LN) h: zals = h_pre @ (g1*(W2@a_src2)),
    # als2 = rstd1*zals + ofs1@(W2@a_src2); same for dst
    a2s = w["a_src2"].reshape(D).astype(np.float32)
    a2d = w["a_dst2"].reshape(D).astype(np.float32)
    W2f = w["W2"].astype(np.float32)
    g1f = w["g1"].astype(np.float32)
    c["zvec"] = np.stack(
        [g1f * (W2f @ a2s), g1f * (W2f @ a2d)], axis=1
    ).astype(BF)  # [D, 2]
    c["w2asum"] = (W2f @ (a2s + a2d)).reshape(D, 1).astype(np.float32)
    c["iota256"] = np.tile(np.arange(256, dtype=np.float32), (P, 1)).astype(BF)
    c["iota128"] = np.tile(np.arange(P, dtype=np.float32), (P, 1)).astype(BF)
    c["eye128b"] = np.eye(P, dtype=np.float32).astype(BF)
    c["eye16f"] = np.eye(16, dtype=np.float32)
    c["hmap1"] = _headmap(HEADS, HID)  # [8, 128] f32
    c["ones1r"] = np.ones((1, P), dtype=np.float32)
    c["ones128c"] = np.ones((P, 1), dtype=np.float32)
    return c


# ======================================================================
# device program
# ======================================================================
def _chunks(total, width):
    out = []
    o = 0
    while o < total:
        w = min(width, total - o)
        out.append((o, w))
        o += w
    return out


def build_program(meta, debug=False):
    npd, nb, t = meta["npd"], meta["nb"], meta["t"]
    ncores = meta["ncores"]
    mcap = meta["mcap"]
    tile2blk = meta["tile2blk"]
    blk_first = meta["blk_first"]
    blk_last = meta["blk_last"]
    n_glob = meta["n"]
    nl = meta["nl"]
    rg = [list(range(ncores))]
    ch512 = _chunks(npd, 512)
    ln_cnt = float(n_glob * D)  # real elements for graph-LN stats

    nc = bacc.Bacc(
        "TRN2",
        target_bir_lowering=False,
        debug=False,
        enable_asserts=False,
        num_devices=ncores,
        num_swdge_queues=2 if QSPLIT else 1,
    )

    def inp(name, shape, dt):
        return nc.dram_tensor(name, shape, dt, kind="ExternalInput").ap()

    xT = inp("xT", [D_IN, npd], F32)
    gidx = inp("gidx", [P, t], I32)
    drel = inp("drel", [P, t], F32)
    slot = inp("slot", [P, nb], F32)
    iota128 = inp("iota128", [P, P], BF16)
    W1 = inp("W1", [D_IN, D], F32)
    a1blk = inp("a1blk", [D, 2 * HEADS], BF16)
    b1c = inp("b1c", [D, 1], F32)
    W2 = inp("W2", [D, D], BF16)
    a2blk = inp("a2blk", [D, 2], BF16)
    b2c = inp("b2c", [D, 1], F32)
    zvec = inp("zvec", [D, 2], BF16)
    w2asum = inp("w2asum", [D, 1], F32)
    fcw = {i: inp(f"fc{i}w", [D, D], BF16) for i in (1, 2, 3, 4)}
    fcb = {i: inp(f"fc{i}b", [D, 1], F32) for i in (1, 2, 3, 4)}
    g1c = inp("g1c", [D, 1], F32)
    be1c = inp("be1c", [D, 1], F32)
    g2c = inp("g2c", [D, 1], F32)
    be2c = inp("be2c", [D, 1], F32)
    fcfw = inp("fcfw", [D, 1], BF16)
    iota256 = inp("iota256", [P, 256], BF16)
    eye128b = inp("eye128b", [P, P], BF16)
    eye16f = inp("eye16f", [16, 16], F32)
    hmap1 = inp("hmap1", [HEADS, D], F32)
    ones1r = inp("ones1r", [1, P], F32)
    ones128c = inp("ones128c", [P, 1], F32)

    pool_out = nc.dram_tensor("pool_out", [256, 1], F32, kind="ExternalOutput").ap()
    dbg_out = None
    dbg1_out = None
    if debug:
        dbg_out = nc.dram_tensor("dbg_out", [D, npd], F32, kind="ExternalOutput").ap()
        dbg1_out = nc.dram_tensor("dbg1_out", [D, npd], F32, kind="ExternalOutput").ap()

    TW = TBLW  # 256 bf16 = 512B rows: h(128) | al_s f32-pairs | pad

    with tile.TileContext(nc) as tc, ExitStack() as top:
        dram = top.enter_context(tc.tile_pool(name="dram", bufs=1, space="DRAM"))
        persist = top.enter_context(tc.tile_pool(name="persist", bufs=1))
        cpool = top.enter_context(tc.tile_pool(name="consts", bufs=1))
        cbp = top.enter_context(tc.tile_pool(name="cbp", bufs=2, space="PSUM"))
        cbs = top.enter_context(tc.tile_pool(name="cbs", bufs=2))

        def scr():  # one-bank PSUM scratch, callers slice columns
            return cbp.tile([P, 512], F32, tag="scr", name="scr")

        tbl1_loc = dram.tile([npd, TW], BF16, tag="tbl1_loc")
        tbl1_full = dram.tile([ncores * npd, TW], BF16, tag="tbl1_full", addr_space="Shared")
        tbl2_loc = dram.tile([npd, TW], BF16, tag="tbl2_loc")
        tbl2_full = dram.tile([ncores * npd, TW], BF16, tag="tbl2_full", addr_space="Shared")

        ln_in = dram.tile([1, 2], F32, tag="ln_in")
        ln_out = dram.tile([1, 2], F32, tag="ln_out", addr_space="Shared")
        ln_in2 = dram.tile([1, 2], F32, tag="ln_in2")
        ln_out2 = dram.tile([1, 2], F32, tag="ln_out2", addr_space="Shared")

        # persistent activations (transposed [feat, node]) and edge-index tables
        hc_sb = persist.tile([P, npd], BF16, tag="hc_sb")  # residual
        hf_sb = persist.tile([P, npd], BF16, tag="hf_sb")  # working activation
        drel_sb = persist.tile([P, t], F32, tag="drel_sb")
        gidx_sb = persist.tile([P, t], I32, tag="gidx_sb")
        ald1_sb = persist.tile([P, nb * HEADS], BF16, tag="ald1_sb")
        ald2_sb = persist.tile([P, nb], BF16, tag="ald2_sb")
        als1_sb = persist.tile([P, nb * HEADS], BF16, tag="als1_sb")
        als2_sb = persist.tile([P, nb], BF16, tag="als2_sb")
        hnm_sb = persist.tile([P, npd], BF16, tag="hnm_sb")  # node-major h
        # LN1-derived affine terms (filled post-E1, consumed in E2)
        sc1_sb = persist.tile([P, 1], F32, tag="sc1_sb")
        ofs1_sb = persist.tile([P, 1], F32, tag="ofs1_sb")
        rstd1_sb = persist.tile([P, 1], F32, tag="rstd1_sb")
        cc_sb = persist.tile([P, 1], F32, tag="cc_sb")
        c2_sb = persist.tile([P, 1], F32, tag="c2_sb")
        ofsrow_sb = persist.tile([1, P], F32, tag="ofsrow_sb")

        # constants in SBUF
        def cload(ap_in, shape, dt, tag):
            s = cpool.tile(shape, dt, tag=tag)
            nc.sync.dma_start(out=s[:], in_=ap_in)
            return s

        W1_s = cload(W1, [D_IN, D], F32, "W1")
        a1_s = cload(a1blk, [D, 2 * HEADS], BF16, "a1")
        a2_s = cload(a2blk, [D, 2], BF16, "a2")
        b1_s = cload(b1c, [D, 1], F32, "b1")
        b2_s = cload(b2c, [D, 1], F32, "b2")
        zvec_s = cload(zvec, [D, 2], BF16, "zvec")
        w2asum_s = cload(w2asum, [D, 1], F32, "w2asum")
        fcw_s = {i: cload(fcw[i], [D, D], BF16, f"fw{i}") for i in (1, 2, 3, 4)}
        fcb_s = {i: cload(fcb[i], [D, 1], F32, f"fb{i}") for i in (1, 2, 3, 4)}
        g1_s = cload(g1c, [D, 1], F32, "g1")
        be1_s = cload(be1c, [D, 1], F32, "be1")
        g2_s = cload(g2c, [D, 1], F32, "g2")
        be2_s = cload(be2c, [D, 1], F32, "be2")
        W2_s = cload(W2, [D, D], BF16, "W2")
        fcf_s = cload(fcfw, [D, 1], BF16, "fcf")
        io256_s = cload(iota256, [P, 256], BF16, "io256")
        io128_s = cload(iota128, [P, P], BF16, "io128")
        eye_s = cload(eye128b, [P, P], BF16, "eye")
        eye16_s = cload(eye16f, [16, 16], F32, "eye16")
        hmap1_s = cload(hmap1, [HEADS, D], F32, "hmap1")
        ones1r_s = cload(ones1r, [1, P], F32, "ones1r")
        ones128c_s = cload(ones128c, [P, 1], F32, "ones128c")
        slot_s = cload(slot, [P, nb], F32, "slot")

        nc.sync.dma_start(out=drel_sb[:], in_=drel)
        nc.sync.dma_start(out=gidx_sb[:], in_=gidx)

        # ---------------- stage A for a GAT layer: build tables ----------------
        def stage_a(layer, src_hT_fn):
            """Write tbl{layer}_loc rows [h bf16 | al_s f32] and ald table; then AllGather.
            src_hT_fn(c0, w, ps): fills psum tile [128, w] with this layer's hT chunk."""
            heads = HEADS if layer == 1 else 1
            a_s = a1_s if layer == 1 else a2_s
            tbl_loc = tbl1_loc if layer == 1 else tbl2_loc
            tbl_full = tbl1_full if layer == 1 else tbl2_full
            ald_sb = ald1_sb if layer == 1 else ald2_sb
            als_sb = als1_sb if layer == 1 else als2_sb
            with (
                tc.tile_pool(name=f"sa{layer}", bufs=5) as sp,
                tc.tile_pool(name=f"sap{layer}", bufs=2, space="PSUM") as pp,
            ):
                for c0, w in ch512:
                    ps = pp.tile([P, 512], F32, tag="hps")
                    src_hT_fn(c0, w, ps)
                    # keep transposed activation for downstream dense chain
                    nc.vector.tensor_copy(out=hf_sb[:, c0 : c0 + w], in_=ps[:, :w])
                for k in range(nb):
                    c0 = k * P
                    s_ = scr()
                    # al_s/al_d for this chunk: [2*heads, 128] = a^T @ hT
                    alps = s_[0 : 2 * heads, 0:P]
                    nc.tensor.matmul(
                        out=alps, lhsT=a_s[:], rhs=hf_sb[:, c0 : c0 + P],
                        start=True, stop=True,
                    )
                    asb = sp.tile([2 * heads, P], F32, tag="asb")
                    nc.vector.tensor_copy(out=asb[:], in_=alps)
                    # transpose -> [128, 2*heads]
                    atp = s_[:, P : P + 2 * heads]
                    nc.tensor.transpose(
                        out=atp, in_=asb[:], identity=eye16_s[: 2 * heads, : 2 * heads]
                    )
                    ats = sp.tile([P, 2 * heads], F32, tag="ats")
                    nc.vector.tensor_copy(out=ats[:], in_=atp)
                    # transpose h chunk -> node-major [128n, 128f]
                    htp = s_[:].bitcast(BF16)[:, 320:448]
                    nc.tensor.transpose(
                        out=htp, in_=hf_sb[:, c0 : c0 + P], identity=eye_s[:]
                    )
                    nc.vector.tensor_copy(
                        out=hnm_sb[:, c0 : c0 + P], in_=htp
                    )
                    # table writes
                    nc.sync.dma_start(
                        out=tbl_loc[c0 : c0 + P, 0:D], in_=hnm_sb[:, c0 : c0 + P]
                    )
                    nc.sync.dma_start(
                        out=tbl_loc[c0 : c0 + P, D : D + 2 * heads],
                        in_=ats[:].bitcast(BF16)[:, 0 : 2 * heads],
                    )
                    nc.vector.tensor_copy(
                        out=ald_sb[:, k * heads : (k + 1) * heads],
                        in_=ats[:, heads : 2 * heads],
                    )
                    nc.vector.tensor_copy(
                        out=als_sb[:, k * heads : (k + 1) * heads],
                        in_=ats[:, 0:heads],
                    )
            nc.gpsimd.collective_compute(
                "AllGather",
                ALU.bypass,
                replica_groups=rg,
                ins=[tbl_loc[:].opt()],
                outs=[tbl_full[:].opt()],
            )

        # ---------------- edge phase for a GAT layer ----------------
        def edge_phase(layer, out_cb):
            """GAT aggregation; out_cb(b, osb) gets [128f, 128n] f32 sbuf tile.

            Per tile: one indirect row-gather (h|al_s), one-hot S on DVE,
            St = transpose(S) on PE feeds the al_d expansion matmul; agg/den
            matmuls accumulate into quad-packed PSUM banks (4 blocks/bank).
            """
            heads = HEADS if layer == 1 else 1
            tbl_full = tbl1_full if layer == 1 else tbl2_full
            ald_sb = ald1_sb if layer == 1 else ald2_sb
            als_sb = als1_sb if layer == 1 else als2_sb
            nwin = (t + WIN - 1) // WIN
            agg_banks = {}
            den_banks = {}
            with (
                tc.tile_pool(name=f"eg{layer}", bufs=3) as gp,
                tc.tile_pool(name=f"eb{layer}", bufs=2) as bigp,
                tc.tile_pool(name=f"es{layer}", bufs=3) as sp,
                tc.tile_pool(name=f"est{layer}", bufs=WIN + 4) as stpool,
                tc.tile_pool(name=f"ef{layer}", bufs=2) as fin,
                tc.tile_pool(name=f"ep{layer}", bufs=2, space="PSUM") as pp,
                tc.tile_pool(name=f"ed{layer}", bufs=1, space="PSUM") as dp,
                tc.tile_pool(name=f"ea{layer}", bufs=1, space="PSUM") as ap_,
                tc.tile_pool(name=f"et{layer}", bufs=1, space="PSUM") as tp_,
                tc.tile_pool(name=f"er{layer}", bufs=1, space="PSUM") as rp,
            ):
                # first-touch memset so never-gathered pad partitions stay finite
                for _ in range(3):
                    gm = gp.tile([P, WIN * TW], BF16, tag="gath")
                    nc.gpsimd.memset(gm[:], 0.0)
                for wi in range(nwin):
                    t0 = wi * WIN
                    L = min(WIN, t - t0)
                    gath = gp.tile([P, WIN * TW], BF16, tag="gath")
                    for i in range(L):
                        m = mcap[t0 + i]
                        bi = nc.gpsimd.indirect_dma_start(
                            out=gath[0:m, i * TW : (i + 1) * TW],
                            out_offset=None,
                            in_=tbl_full[:],
                            in_offset=IndirectOffsetOnAxis(
                                ap=gidx_sb[0:m, t0 + i : t0 + i + 1], axis=0
                            ),
                        )
                        if QSPLIT and (t0 + i) % 2 == 1:
                            bi.ins.queue = "qPoolDynamic1"
                    aldw = ap_.tile([P, WIN * heads], F32, tag="aldw")
                    st_list = []
                    for i in range(L):
                        ti = t0 + i
                        s_t = stpool.tile([P, P], BF16, tag="s_t")
                        nc.vector.tensor_scalar(
                            out=s_t[:], in0=io128_s[:],
                            scalar1=drel_sb[:, ti : ti + 1], scalar2=None,
                            op0=ALU.is_equal,
                        )
                        st_list.append(s_t)
                    # batched St: 4 transposes share one PSUM bank, one ACT copy
                    for g0 in range(0, L, 4):
                        gl = min(4, L - g0)
                        stp = tp_.tile([P, 4 * P], BF16, tag="stp")
                        for j in range(gl):
                            nc.tensor.matmul(
                                out=stp[:, j * P : (j + 1) * P],
                                lhsT=st_list[g0 + j][:],
                                rhs=eye_s[:],
                                is_transpose=True,
                                start=(j == 0),
                                stop=(j == gl - 1),
                                skip_group_check=True,
                            )
                        sts = sp.tile([P, 4 * P], BF16, tag="sts")
                        nc.scalar.activation(
                            out=sts[:, 0 : gl * P], in_=stp[:, 0 : gl * P],
                            func=ACTF.Copy,
                        )
                        for j in range(gl):
                            i = g0 + j
                            b = tile2blk[t0 + i]
                            nc.tensor.matmul(
                                out=aldw[:, i * heads : (i + 1) * heads],
                                lhsT=sts[:, j * P : (j + 1) * P],
                                rhs=ald_sb[:, b * heads : (b + 1) * heads],
                                start=(i == 0),
                                stop=(i == L - 1),
                                skip_group_check=True,
                            )
                    # e = al_s[src] + al_d[dst]
                    als_v = (
                        gath[:]
                        .bitcast(F32)
                        .rearrange("p (t w) -> p t w", w=TW // 2)[
                            :, 0:L, D // 2 : D // 2 + heads
                        ]
                    )
                    e_sb = sp.tile([P, WIN * heads], F32, tag="e_sb")
                    nc.vector.tensor_tensor(
                        out=e_sb[:, 0 : L * heads].rearrange(
                            "p (t h) -> p t h", h=heads
                        ),
                        in0=als_v,
                        in1=aldw[:, 0 : L * heads].rearrange(
                            "p (t h) -> p t h", h=heads
                        ),
                        op=ALU.add,
                    )
                    if layer == 2:
                        # raw z-logits -> true logits: e = rstd1*e + CC
                        nc.vector.tensor_scalar(
                            out=e_sb[:, 0 : L * heads],
                            in0=e_sb[:, 0 : L * heads],
                            scalar1=rstd1_sb[:], scalar2=cc_sb[:],
                            op0=ALU.mult, op1=ALU.add,
                        )
                    t02 = sp.tile([P, WIN * heads], F32, tag="t02")
                    nc.vector.tensor_scalar(
                        out=t02[:, 0 : L * heads], in0=e_sb[:, 0 : L * heads],
                        scalar1=NEG_SLOPE, scalar2=None, op0=ALU.mult,
                    )
                    lr_sb = sp.tile([P, WIN * heads], F32, tag="lr_sb")
                    nc.vector.tensor_tensor(
                        out=lr_sb[:, 0 : L * heads], in0=e_sb[:, 0 : L * heads],
                        in1=t02[:, 0 : L * heads], op=ALU.max,
                    )
                    pexp = bigp.tile([P, WIN * D], BF16, tag="pexp")
                    nc.scalar.activation(
                        out=pexp[:, 0 : L * D].rearrange(
                            "p (t h c) -> p t h c", h=heads, c=D // heads
                        ),
                        in_=lr_sb[:, 0 : L * heads]
                        .rearrange("p (t h) -> p t h", h=heads)
                        .broadcast_to((P, L, heads, D // heads)),
                        func=ACTF.Exp,
                    )
                    p_sb = sp.tile([P, WIN * heads], BF16, tag="p_sb")
                    nc.scalar.activation(
                        out=p_sb[:, 0 : L * heads], in_=lr_sb[:, 0 : L * heads],
                        func=ACTF.Exp,
                    )
                    wh = bigp.tile([P, WIN * D], BF16, tag="wh")
                    nc.vector.tensor_tensor(
                        out=wh[:, 0 : L * D].rearrange("p (t c) -> p t c", c=D),
                        in0=gath[:].rearrange("p (t w) -> p t w", w=TW)[:, 0:L, 0:D],
                        in1=pexp[:, 0 : L * D].rearrange("p (t c) -> p t c", c=D),
                        op=ALU.mult,
                    )
                    for i in range(L):
                        ti = t0 + i
                        b = tile2blk[ti]
                        q = b // 4  # quad id
                        qi = b % 4
                        if q not in agg_banks:
                            qblocks = [bb for bb in range(4 * q, min(4 * q + 4, nb))]
                            ab = pp.tile([P, 512], F32, tag="aggq", name="aggq")
                            db = dp.tile([8, 512], F32, tag="denq", name="denq")
                            agg_banks[q] = (
                                ab, blk_first[qblocks[0]], blk_last[qblocks[-1]]
                            )
                            den_banks[q] = (
                                db, blk_first[qblocks[0]], blk_last[qblocks[-1]]
                            )
                        ab, qf, ql = agg_banks[q]
                        db, _, _ = den_banks[q]
                        nc.tensor.matmul(
                            out=ab[:, qi * P : (qi + 1) * P],
                            lhsT=wh[:, i * D : (i + 1) * D],
                            rhs=st_list[i][:],
                            start=(ti == qf),
                            stop=False,
                            skip_group_check=True,
                        )
                        nc.tensor.matmul(
                            out=db[0:heads, qi * P : (qi + 1) * P],
                            lhsT=p_sb[:, i * heads : (i + 1) * heads],
                            rhs=st_list[i][:],
                            start=(ti == qf),
                            stop=False,
                            skip_group_check=True,
                        )
                        if ti == blk_last[b]:
                            # diagonal (self-loop) contribution: h and al are
                            # local, injected as one pseudo-tile (S = identity)
                            esf = fin.tile([P, HEADS], F32, tag="esf")
                            nc.vector.tensor_tensor(
                                out=esf[:, 0:heads],
                                in0=als_sb[:, b * heads : (b + 1) * heads],
                                in1=ald_sb[:, b * heads : (b + 1) * heads],
                                op=ALU.add,
                            )
                            if layer == 2:
                                nc.vector.tensor_scalar(
                                    out=esf[:, 0:heads], in0=esf[:, 0:heads],
                                    scalar1=rstd1_sb[:], scalar2=cc_sb[:],
                                    op0=ALU.mult, op1=ALU.add,
                                )
                            tsf = fin.tile([P, HEADS], F32, tag="tsf")
                            nc.vector.tensor_scalar(
                                out=tsf[:, 0:heads], in0=esf[:, 0:heads],
                                scalar1=NEG_SLOPE, scalar2=None, op0=ALU.mult,
                            )
                            lrs = fin.tile([P, HEADS], F32, tag="lrs")
                            nc.vector.tensor_tensor(
                                out=lrs[:, 0:heads], in0=esf[:, 0:heads],
                                in1=tsf[:, 0:heads], op=ALU.max,
                            )
                            pxs = fin.tile([P, D], BF16, tag="pxs")
                            nc.scalar.activation(
                                out=pxs[:].rearrange("p (h c) -> p h c", h=heads),
                                in_=lrs[:, 0:heads].broadcast_to(
                                    (P, heads, D // heads)
                                ),
                                func=ACTF.Exp,
                            )
                            psbs = fin.tile([P, HEADS], BF16, tag="psbs")
                            nc.scalar.activation(
                                out=psbs[:, 0:heads], in_=lrs[:, 0:heads],
                                func=ACTF.Exp,
                            )
                            whs = fin.tile([P, D], BF16, tag="whs")
                            nc.vector.tensor_tensor(
                                out=whs[:], in0=hnm_sb[:, b * P : (b + 1) * P],
                                in1=pxs[:], op=ALU.mult,
                            )
                            nc.tensor.matmul(
                                out=ab[:, qi * P : (qi + 1) * P],
                                lhsT=whs[:], rhs=eye_s[:],
                                start=False, stop=(ti == ql),
                                skip_group_check=True,
                            )
                            nc.tensor.matmul(
                                out=db[0:heads, qi * P : (qi + 1) * P],
                                lhsT=psbs[:, 0:heads], rhs=eye_s[:],
                                start=False, stop=(ti == ql),
                                skip_group_check=True,
                            )
                            dsb = fin.tile([heads, P], F32, tag="dsb")
                            nc.vector.tensor_scalar(
                                out=dsb[:], in0=db[0:heads, qi * P : (qi + 1) * P],
                                scalar1=1e-16, scalar2=None, op0=ALU.add,
                            )
                            rsb = fin.tile([heads, P], F32, tag="rsb")
                            nc.vector.reciprocal(out=rsb[:], in_=dsb[:])
                            rp_t = rp.tile([P, 512], F32, tag="rscr")
                            rex = rp_t[:, 0:P]
                            nc.tensor.matmul(
                                out=rex,
                                lhsT=(ones1r_s if heads == 1 else hmap1_s)[:],
                                rhs=rsb[:], start=True, stop=True,
                            )
                            rxs = fin.tile([P, P], F32, tag="rxs")
                            nc.scalar.activation(out=rxs[:], in_=rex, func=ACTF.Copy)
                            osb = fin.tile([P, P], F32, tag="osb")
                            if layer == 2:
                                # out2 = [sc1*ab + ofs1*den] @ W2 / den
                                tmul = fin.tile([P, P], F32, tag="tmul")
                                nc.vector.tensor_scalar(
                                    out=tmul[:],
                                    in0=ab[:, qi * P : (qi + 1) * P],
                                    scalar1=sc1_sb[:], scalar2=None,
                                    op0=ALU.mult,
                                )
                                t2l = fin.tile([P, P], BF16, tag="t2l")
                                nc.vector.tensor_copy(out=t2l[:], in_=tmul[:])
                                ps2 = rp_t[:, 2 * P : 3 * P]
                                nc.tensor.matmul(
                                    out=ps2, lhsT=W2_s[:], rhs=t2l[:],
                                    start=True, stop=True,
                                )
                                nc.vector.tensor_tensor(
                                    out=osb[:], in0=ps2, in1=rxs[:],
                                    op=ALU.mult,
                                )
                            else:
                                nc.vector.tensor_tensor(
                                    out=osb[:], in0=ab[:, qi * P : (qi + 1) * P],
                                    in1=rxs[:], op=ALU.mult,
                                )
                            out_cb(b, osb)
                            if b == 4 * q + 3 or b == nb - 1:
                                del agg_banks[q], den_banks[q]

        # ---------------- graph-LN over h_sb -> dst_sb (bf16) ----------------
        def graph_ln(src_sb, g_s, be_s, ln_i, ln_o, dst_sb, stash=False):
            with (
                tc.tile_pool(name="ln", bufs=2) as sp,
                tc.tile_pool(name="lnp", bufs=2, space="PSUM") as pp,
            ):
                nchunk = len(ch512)
                stats = sp.tile([P, nchunk * 6], F32, tag="stats")
                for ci, (c0, w) in enumerate(ch512):
                    nc.vector.bn_stats(
                        out=stats[:, ci * 6 : (ci + 1) * 6], in_=src_sb[:, c0 : c0 + w]
                    )
                mv = sp.tile([P, 2], F32, tag="mv")
                nc.vector.bn_aggr(
                    out=mv[:], in_=stats[:].rearrange("p (c s) -> p c s", s=6)
                )
                # per-partition sums: [sx, sxx] = [m, (v+m^2)] * npd
                sums = sp.tile([P, 2], F32, tag="sums")
                nc.vector.tensor_scalar(
                    out=sums[:, 0:1], in0=mv[:, 0:1], scalar1=float(npd),
                    scalar2=None, op0=ALU.mult,
                )
                m2 = sp.tile([P, 1], F32, tag="m2")
                nc.vector.tensor_tensor(
                    out=m2[:], in0=mv[:, 0:1], in1=mv[:, 0:1], op=ALU.mult
                )
                nc.vector.tensor_tensor(
                    out=sums[:, 1:2], in0=mv[:, 1:2], in1=m2[:], op=ALU.add
                )
                nc.vector.tensor_scalar(
                    out=sums[:, 1:2], in0=sums[:, 1:2], scalar1=float(npd),
                    scalar2=None, op0=ALU.mult,
                )
                red = pp.tile([1, 2], F32, tag="red")
                nc.tensor.matmul(
                    out=red[:], lhsT=ones128c_s[:], rhs=sums[:], start=True, stop=True
                )
                rsb = sp.tile([1, 2], F32, tag="rsb2")
                nc.vector.tensor_copy(out=rsb[:], in_=red[:])
                nc.gpsimd.dma_start(out=ln_i[:], in_=rsb[:])
                nc.gpsimd.collective_compute(
                    "AllReduce", ALU.add, replica_groups=rg,
                    ins=[ln_i[:].opt()], outs=[ln_o[:].opt()],
                )
                ar = sp.tile([1, 2], F32, tag="ar")
                nc.gpsimd.dma_start(out=ar[:], in_=ln_o[:])
                bc = pp.tile([P, 2], F32, tag="bc")
                nc.tensor.matmul(
                    out=bc[:], lhsT=ones1r_s[:], rhs=ar[:], start=True, stop=True
                )
                # mu = s1/cnt ; var = s2/cnt - mu^2 ; s = g * rsqrt(var+eps) ; b = be - mu*s
                mu = sp.tile([P, 1], F32, tag="mu")
                nc.vector.tensor_scalar(
                    out=mu[:], in0=bc[:, 0:1], scalar1=1.0 / ln_cnt, scalar2=None,
                    op0=ALU.mult,
                )
                var = sp.tile([P, 1], F32, tag="var")
                nc.vector.tensor_scalar(
                    out=var[:], in0=bc[:, 1:2], scalar1=1.0 / ln_cnt, scalar2=None,
                    op0=ALU.mult,
                )
                mu2 = sp.tile([P, 1], F32, tag="mu2")
                nc.vector.tensor_tensor(out=mu2[:], in0=mu[:], in1=mu[:], op=ALU.mult)
                nc.vector.tensor_tensor(
                    out=var[:], in0=var[:], in1=mu2[:], op=ALU.subtract
                )
                nc.vector.tensor_scalar(
                    out=var[:], in0=var[:], scalar1=LN_EPS, scalar2=None, op0=ALU.add
                )
                sd = sp.tile([P, 1], F32, tag="sd")
                nc.scalar.activation(out=sd[:], in_=var[:], func=ACTF.Sqrt)
                rstd = sp.tile([P, 1], F32, tag="rstd")
                nc.vector.reciprocal(out=rstd[:], in_=sd[:])
                sc = sp.tile([P, 1], F32, tag="sc")
                nc.vector.tensor_tensor(out=sc[:], in0=g_s[:], in1=rstd[:], op=ALU.mult)
                ofs = sp.tile([P, 1], F32, tag="ofs")
                nc.vector.tensor_tensor(out=ofs[:], in0=mu[:], in1=sc[:], op=ALU.mult)
                nc.vector.tensor_tensor(
                    out=ofs[:], in0=be_s[:], in1=ofs[:], op=ALU.subtract
                )
                if stash:
                    nc.vector.tensor_copy(out=rstd1_sb[:], in_=rstd[:])
                    nc.vector.tensor_copy(out=sc1_sb[:], in_=sc[:])
                    nc.vector.tensor_copy(out=ofs1_sb[:], in_=ofs[:])
                for c0, w in ch512:
                    nc.scalar.activation(
                        out=dst_sb[:, c0 : c0 + w], in_=src_sb[:, c0 : c0 + w],
                        func=ACTF.Identity, bias=ofs[:], scale=sc[:],
                    )
                # zero the padded node columns so they don't pollute later stats
                if npd > nl:
                    nc.gpsimd.memset(dst_sb[:, nl:npd], 0.0)

        # =========================== the network ===========================
        # ---- GAT layer 1 stage A: h1 = x @ W1 (transposed) ----
        xin_pool = top.enter_context(tc.tile_pool(name="xin", bufs=2))

        def h1_fn(c0, w, ps):
            xt = xin_pool.tile([D_IN, 512], F32, tag="xt")
            nc.sync.dma_start(out=xt[:, :w], in_=xT[:, c0 : c0 + w])
            nc.tensor.matmul(
                out=ps[:, :w], lhsT=W1_s[:], rhs=xt[:, :w], start=True, stop=True
            )

        stage_a(1, h1_fn)

        # ---- GAT layer 1 edge phase; per-block callback also runs fc1/fc2
        # and builds the raw (pre-LN) layer-2 table [h_pre | zals], so AG2
        # can start immediately when E1 drains and LN1 runs under it. ----
        def out1_cb(b, osb):
            c0 = b * P
            h1o = hc_sb[:, c0 : c0 + P]
            nc.scalar.activation(
                out=h1o, in_=osb[:], func=ACTF.Relu, bias=b1_s[:], scale=1.0,
            )
            s_ = scr()
            ps1 = s_[:, 0:P]
            nc.tensor.matmul(
                out=ps1, lhsT=fcw_s[1][:], rhs=h1o, start=True, stop=True
            )
            t1 = cbs.tile([P, P], BF16, tag="t1")
            nc.scalar.activation(
                out=t1[:], in_=ps1, func=ACTF.Relu, bias=fcb_s[1][:], scale=1.0
            )
            psf = s_[:, P : 2 * P]
            nc.tensor.matmul(
                out=psf, lhsT=fcw_s[2][:], rhs=t1[:], start=True, stop=True
            )
            t2f = cbs.tile([P, P], F32, tag="t2f")
            nc.vector.tensor_tensor(out=t2f[:], in0=psf, in1=h1o, op=ALU.add)
            nc.scalar.activation(
                out=hf_sb[:, c0 : c0 + P], in_=t2f[:], func=ACTF.Relu,
                bias=fcb_s[2][:], scale=1.0,
            )
            # raw layer-2 attention z-values
            zz = s_[0:2, 2 * P : 3 * P]
            nc.tensor.matmul(
                out=zz, lhsT=zvec_s[:], rhs=hf_sb[:, c0 : c0 + P],
                start=True, stop=True,
            )
            zzs = cbs.tile([2, P], F32, tag="zzs")
            nc.vector.tensor_copy(out=zzs[:], in_=zz)
            atp2 = s_[:, 3 * P : 3 * P + 2]
            nc.tensor.transpose(out=atp2, in_=zzs[:], identity=eye16_s[0:2, 0:2])
            ats2 = cbs.tile([P, 2], F32, tag="ats2")
            nc.vector.tensor_copy(out=ats2[:], in_=atp2)
            nc.vector.tensor_copy(out=ald2_sb[:, b : b + 1], in_=ats2[:, 1:2])
            nc.vector.tensor_copy(out=als2_sb[:, b : b + 1], in_=ats2[:, 0:1])
            htp2 = s_[:].bitcast(BF16)[:, 772:900]
            nc.tensor.transpose(
                out=htp2, in_=hf_sb[:, c0 : c0 + P], identity=eye_s[:]
            )
            nc.vector.tensor_copy(out=hnm_sb[:, c0 : c0 + P], in_=htp2)
            nc.sync.dma_start(
                out=tbl2_loc[c0 : c0 + P, 0:D], in_=hnm_sb[:, c0 : c0 + P]
            )
            nc.sync.dma_start(
                out=tbl2_loc[c0 : c0 + P, D : D + 2],
                in_=ats2[:].bitcast(BF16)[:, 0:2],
            )

        edge_phase(1, out1_cb)
        nc.gpsimd.collective_compute(
            "AllGather",
            ALU.bypass,
            replica_groups=rg,
            ins=[tbl2_loc[:].opt()],
            outs=[tbl2_full[:].opt()],
        )
        if debug:
            with tc.tile_pool(name="dbg1p", bufs=2) as dbp1:
                for c0, w in ch512:
                    dt1 = dbp1.tile([P, 512], F32, tag="dbg1")
                    nc.vector.tensor_copy(out=dt1[:, :w], in_=hc_sb[:, c0 : c0 + w])
                    nc.sync.dma_start(out=dbg1_out[:, c0 : c0 + w], in_=dt1[:, :w])

        # ---- LN1 (stats + AR + affine; runs under AG2), then the E2
        # affine constants derived from (rstd1, ofs1) ----
        graph_ln(hf_sb, g1_s, be1_s, ln_in, ln_out, hc_sb, stash=True)
        with (
            tc.tile_pool(name="epl", bufs=1) as epp,
            tc.tile_pool(name="eplp", bufs=1, space="PSUM") as epps,
        ):
            m1 = epps.tile([1, 1], F32, tag="m1")
            nc.tensor.matmul(
                out=m1[:], lhsT=ofs1_sb[:], rhs=w2asum_s[:], start=True, stop=True
            )
            m1s = epp.tile([1, 1], F32, tag="m1s")
            nc.vector.tensor_copy(out=m1s[:], in_=m1[:])
            ccp = epps.tile([P, 1], F32, tag="ccp")
            nc.tensor.matmul(
                out=ccp[:], lhsT=ones1r_s[:], rhs=m1s[:], start=True, stop=True
            )
            nc.vector.tensor_copy(out=cc_sb[:], in_=ccp[:])
            ofsb = epp.tile([P, 1], BF16, tag="ofsb")
            nc.vector.tensor_copy(out=ofsb[:], in_=ofs1_sb[:])
            c2p = epps.tile([P, 1], F32, tag="c2p")
            nc.tensor.matmul(
                out=c2p[:], lhsT=W2_s[:], rhs=ofsb[:], start=True, stop=True
            )
            c2t = epp.tile([P, 1], F32, tag="c2t")
            nc.vector.tensor_copy(out=c2t[:], in_=c2p[:])
            nc.vector.tensor_tensor(
                out=c2_sb[:], in0=c2t[:], in1=b2_s[:], op=ALU.add
            )
            orp = epps.tile([1, P], F32, tag="orp")
            nc.tensor.matmul(
                out=orp[:], lhsT=ofsb[:], rhs=eye_s[:], start=True, stop=True
            )
            nc.vector.tensor_copy(out=ofsrow_sb[:], in_=orp[:])

        # ---- GAT layer 2 edge phase (raw table; LN+W2 folded into the
        # finalize) -> hf_sb = out2 + (W2^T ofs1 + b2) ----
        def out2_cb(b, osb):
            nc.scalar.activation(
                out=hf_sb[:, b * P : (b + 1) * P], in_=osb[:], func=ACTF.Identity,
                bias=c2_sb[:], scale=1.0,
            )

        edge_phase(2, out2_cb)

        # ---- fc3 (relu), fc4 (+residual h_ln1), LN2, fcf, pool ----
        with (
            tc.tile_pool(name="fc2", bufs=3) as fp,
            tc.tile_pool(name="fcp2", bufs=2, space="PSUM") as fpp,
        ):
            for c0, w in ch512:
                ps = fpp.tile([P, 512], F32, tag="fc3p")
                nc.tensor.matmul(
                    out=ps[:, :w], lhsT=fcw_s[3][:], rhs=hf_sb[:, c0 : c0 + w],
                    start=True, stop=True,
                )
                nc.scalar.activation(
                    out=hf_sb[:, c0 : c0 + w], in_=ps[:, :w], func=ACTF.Relu,
                    bias=fcb_s[3][:], scale=1.0,
                )
            for c0, w in ch512:
                ps = fpp.tile([P, 512], F32, tag="fc4p")
                nc.tensor.matmul(
                    out=ps[:, :w], lhsT=fcw_s[4][:], rhs=hf_sb[:, c0 : c0 + w],
                    start=True, stop=True,
                )
                tmp = fp.tile([P, 512], F32, tag="fc4t")
                nc.vector.tensor_tensor(
                    out=tmp[:, :w], in0=ps[:, :w], in1=hc_sb[:, c0 : c0 + w], op=ALU.add
                )
                nc.scalar.activation(
                    out=hf_sb[:, c0 : c0 + w], in_=tmp[:, :w], func=ACTF.Identity,
                    bias=fcb_s[4][:], scale=1.0,
                )
        graph_ln(hf_sb, g2_s, be2_s, ln_in2, ln_out2, hc_sb)  # hc_sb = h_ln2

        if debug:
            with tc.tile_pool(name="dbgp", bufs=2) as dbp:
                for c0, w in ch512:
                    dt_ = dbp.tile([P, 512], F32, tag="dbg")
                    nc.vector.tensor_copy(out=dt_[:, :w], in_=hc_sb[:, c0 : c0 + w])
                    nc.sync.dma_start(out=dbg_out[:, c0 : c0 + w], in_=dt_[:, :w])

        # fcf + per-core pooled slot sums
        with (
            tc.tile_pool(name="pl", bufs=6) as sp,
            tc.tile_pool(name="plp", bufs=4, space="PSUM") as pp,
            tc.tile_pool(name="plq", bufs=1, space="PSUM") as qq,
        ):
            pool_a = qq.tile([P, 1], F32, tag="pool_a")
            pool_b = qq.tile([P, 1], F32, tag="pool_b")
            for k in range(nb):
                c0 = k * P
                hv = pp.tile([P, 1], F32, tag="hv")
                nc.tensor.matmul(
                    out=hv[:], lhsT=hc_sb[:, c0 : c0 + P], rhs=fcf_s[:],
                    start=True, stop=True,
                )
                hvs = sp.tile([P, 1], F32, tag="hvs")
                nc.vector.tensor_copy(out=hvs[:], in_=hv[:])
                oh = sp.tile([P, 256], BF16, tag="oh")
                nc.vector.tensor_scalar(
                    out=oh[:], in0=io256_s[:], scalar1=slot_s[:, k : k + 1],
                    scalar2=None, op0=ALU.is_equal,
                )
                ohf = sp.tile([P, 256], F32, tag="ohf")
                nc.vector.tensor_copy(out=ohf[:], in_=oh[:])
                nc.tensor.matmul(
                    out=pool_a[:], lhsT=ohf[:, 0:P], rhs=hvs[:],
                    start=(k == 0), stop=(k == nb - 1),
                )
                nc.tensor.matmul(
                    out=pool_b[:], lhsT=ohf[:, P:256], rhs=hvs[:],
                    start=(k == 0), stop=(k == nb - 1),
                )
            pool_sb = sp.tile([P, 2], F32, tag="pool_sb")
            nc.vector.tensor_copy(out=pool_sb[:, 0:1], in_=pool_a[:])
            nc.vector.tensor_copy(out=pool_sb[:, 1:2], in_=pool_b[:])
            nc.sync.dma_start(out=pool_out[0:P, 0:1], in_=pool_sb[:, 0:1])
            nc.sync.dma_start(out=pool_out[P:256, 0:1], in_=pool_sb[:, 1:2])

    nc.compile()
    return nc


# ======================================================================
# driver
# ======================================================================
def _in_maps(meta, core_arrays, slots, consts, x):
    nl, npd = meta["nl"], meta["npd"]
    maps = []
    for c in range(meta["ncores"]):
        gidx_pi, drel_pi = core_arrays[c]
        xT = np.zeros((D_IN, npd), dtype=np.float32)
        xT[:, :nl] = np.asarray(x[c * nl : (c + 1) * nl], dtype=np.float32).T
        m = dict(
            xT=xT, gidx=gidx_pi, drel=drel_pi, slot=slots[c]
        )
        m.update(consts)
        maps.append(m)
    return maps


def _install_ntff_shim():
    """Provide antenv.axon_hooks via direct ctypes into libaxon_pjrt.so."""
    import types, contextlib, ctypes

    try:
        import antenv.axon_hooks  # noqa: F401

        return True
    except ImportError:
        pass
    so_path = "/opt/axon/libaxon_pjrt.so"
    try:
        lib = ctypes.CDLL(so_path)
    except OSError:
        return False
    if not hasattr(lib, "axon_start_nrt_profile"):
        return False
    lib.axon_start_nrt_profile.argtypes = [
        ctypes.POINTER(ctypes.c_int64),
        ctypes.c_size_t,
    ]
    lib.axon_start_nrt_profile.restype = ctypes.c_int64
    lib.axon_stop_nrt_profile.argtypes = [ctypes.c_char_p]
    lib.axon_stop_nrt_profile.restype = ctypes.c_int64

    @contextlib.contextmanager
    def _hook(output_dir, device_ids):
        import jax

        jax.devices()
        if device_ids:
            ids = (ctypes.c_int64 * len(device_ids))(*device_ids)
            rc = lib.axon_start_nrt_profile(ids, len(device_ids))
        else:
            rc = lib.axon_start_nrt_profile(None, 0)
        if rc != 0:
            raise RuntimeError(f"axon_start_nrt_profile rc={rc}")
        try:
            yield
        finally:
            nfiles = lib.axon_stop_nrt_profile(str(output_dir).encode())
            print(f"ntff profile: {nfiles} file(s) -> {output_dir}", file=sys.stderr)

    mod = types.ModuleType("antenv.axon_hooks")
    mod.get_axon_ntff_profile_hook = lambda: _hook
    mod.set_axon_ntff_profile_hook = lambda h: None
    import antenv

    antenv.axon_hooks = mod
    sys.modules["antenv.axon_hooks"] = mod
    return True


def run(inputs, debug=False, trace=False):
    if trace:
        trace = _install_ntff_shim()
    x = np.asarray(inputs["x"])
    edge_index = np.asarray(inputs["edge_index"])
    batch = np.asarray(inputs["batch"])
    meta, core_arrays, slots, g0s, counts = _prep(x, edge_index, batch)
    weights = {
        k: np.asarray(v)
        for k, v in inputs.items()
        if k not in ("x", "edge_index", "batch")
    }
    consts = _consts(weights, meta)
    nc = build_program(meta, debug=debug)
    maps = _in_maps(meta, core_arrays, slots, consts, x)

    hw = get_hw_module(nc.m)
    old = nc.m
    nc.m = hw
    try:
        res = bass_utils.run_bass_kernel_spmd(
            nc, maps, core_ids=list(range(meta["ncores"])), trace=trace
        )
    finally:
        nc.m = old

    # host unshard: assemble per-graph sums from per-core slot partials
    sums = np.zeros((G,), dtype=np.float64)
    for c in range(meta["ncores"]):
        part = np.asarray(res.results[c]["pool_out"], dtype=np.float64).reshape(256)
        g0 = g0s[c]
        hi = min(256, G - g0)
        sums[g0 : g0 + hi] += part[:hi]
    fcf_b = float(np.asarray(inputs["fcf_b"]).reshape(-1)[0])
    out = sums / np.maximum(counts, 1.0) + fcf_b
    return out.astype(np.float32).reshape(G, 1), res


def kernel(**inputs):
    out, _ = run(inputs)
    return out



# revision 58
# speedup vs baseline: 1.1393x; 1.0036x over previous
"""GAT (2-layer, PyG-style) + MLP + graph-LN + global mean pool on 8 Trainium2 cores.

Strategy (sharding_hint): nodes partitioned contiguously across the 8 cores;
edges partitioned by destination node (1-D graph partition, host-sorted by dst);
the per-layer node-feature table [h | a_src-logit] is AllGathered so each core
gathers h[src] rows for its local edges with indirect DMA; per-destination
softmax + weighted aggregation is done with one-hot scatter matmuls
accumulating in PSUM per 128-node block; graph-LayerNorm statistics and are
combined with a tiny AllReduce; the final global_mean_pool partial sums per
core are assembled on the host (unshard step).
"""

import os
import sys

sys.path.insert(0, "/opt/trn_rl_repo")

import math
from contextlib import ExitStack

QSPLIT = os.environ.get("QSPLIT", "0") == "1"  # alternate SWDGE queues for gathers

import numpy as np
import ml_dtypes

BF = ml_dtypes.bfloat16

import concourse.bass as bass
import concourse.bacc as bacc
import concourse.tile as tile
import concourse.mybir as mybir
from concourse import bass_utils
from concourse.bass import IndirectOffsetOnAxis
from concourse.bass_interp import get_hw_module

F32 = mybir.dt.float32
BF16 = mybir.dt.bfloat16
I32 = mybir.dt.int32
I16 = mybir.dt.int16
ALU = mybir.AluOpType
ACTF = mybir.ActivationFunctionType

# ---- problem constants (hardcoded per spec) ----
N = 100000
E_RAW = 1600000
D_IN = 9
HID = 16
HEADS = 8
D = 128
G = 1000
NCORES = 8
NEG_SLOPE = 0.2
LN_EPS = 1e-5
TBLW = 144  # table row width in bf16 (288B rows: h 128 | al_s f32-pairs)
WIN = 32  # edge tiles per gather window
P = 128

PAD_DREL = 300.0  # one-hot never matches (iota is 0..127)
PAD_SLOT = 300.0


# ======================================================================
# host-side prep: edge sort / partition / padding, index layouts, weights
# ======================================================================
def _prep(x, edge_index, batch, n=N, e_raw=E_RAW, ncores=NCORES, g=G):
    nl = n // ncores  # owned nodes per core
    npd = ((nl + P - 1) // P) * P
    nb = npd // P  # 128-node blocks per core

    # self-loops are handled by an on-chip diagonal fast path (h and al
    # are core-local), so only the raw edges go through the gather.
    src = edge_index[0].astype(np.int64)
    dst = edge_index[1].astype(np.int64)
    order = np.argsort(dst, kind="stable")
    srcs = src[order]
    dsts = dst[order]

    bounds = np.searchsorted(dsts, np.arange(ncores + 1) * nl)
    per_core = []
    cnt = np.zeros((ncores, nb), dtype=np.int64)
    for c in range(ncores):
        s_c = srcs[bounds[c] : bounds[c + 1]]
        d_c = dsts[bounds[c] : bounds[c + 1]] - c * nl
        per_core.append((s_c, d_c))
        cnt[c] = np.bincount(d_c // P, minlength=nb)
    maxe = cnt.max(axis=0)
    cap = ((maxe + P - 1) // P) * P  # edge slots per block
    cap = np.maximum(cap, P)
    ktiles = (cap // P).astype(np.int64)
    t_total = int(ktiles.sum())
    blk_starts = np.concatenate([[0], np.cumsum(ktiles)])
    tile2blk = []
    mcap = []  # valid gather rows per tile (max over cores, pads skipped)
    for b in range(nb):
        tile2blk += [b] * int(ktiles[b])
        for i in range(int(ktiles[b])):
            mcap.append(int(min(P, max(1, maxe[b] - i * P))))
    blk_first = {b: int(blk_starts[b]) for b in range(nb)}
    blk_last = {b: int(blk_starts[b + 1]) - 1 for b in range(nb)}

    core_arrays = []
    for c in range(ncores):
        s_c, d_c = per_core[c]
        gidx = np.zeros((t_total * P,), dtype=np.int32)
        drel = np.full((t_total * P,), PAD_DREL, dtype=np.float32)
        blk = d_c // P
        cstart = np.concatenate([[0], np.cumsum(np.bincount(blk, minlength=nb))])
        for b in range(nb):
            e0, e1 = cstart[b], cstart[b + 1]
            o0 = int(blk_starts[b]) * P
            m = e1 - e0
            sc = s_c[e0:e1]
            gidx[o0 : o0 + m] = ((sc // nl) * npd + (sc % nl)).astype(np.int32)
            drel[o0 : o0 + m] = (d_c[e0:e1] % P).astype(np.float32)
        core_arrays.append(
            (
                gidx.reshape(t_total, P).T.copy(),
                drel.reshape(t_total, P).T.copy(),
            )
        )

    # pool slots
    bsort = np.asarray(batch, dtype=np.int64)
    slots = []
    g0s = []
    counts = np.bincount(bsort, minlength=g).astype(np.float64)
    for c in range(ncores):
        bs = bsort[c * nl : (c + 1) * nl]
        g0 = int(bs[0])
        sl = np.full((npd,), PAD_SLOT, dtype=np.float32)
        sl[:nl] = (bs - g0).astype(np.float32)
        assert sl[:nl].max() < 256, "graph-slot overflow"
        slots.append(sl.reshape(nb, P).T.copy())
        g0s.append(g0)

    meta = dict(
        n=n, nl=nl, npd=npd, nb=nb, t=t_total, mcap=mcap,
        tile2blk=tile2blk, blk_first=blk_first, blk_last=blk_last,
        ncores=ncores, g=g,
    )
    return meta, core_arrays, slots, g0s, counts


def _blockdiag(a):  # a [H, C] -> [H*C, H]
    h, c = a.shape
    out = np.zeros((h * c, h), dtype=np.float32)
    for i in range(h):
        out[i * c : (i + 1) * c, i] = a[i]
    return out


def _headmap(heads, ch):  # [H, H*C] one-hot expansion map
    out = np.zeros((heads, heads * ch), dtype=np.float32)
    for i in range(heads):
        out[i, i * ch : (i + 1) * ch] = 1.0
    return out


def _consts(weights, meta):
    """Replicated (same every core) input arrays."""
    w = weights
    c = {}
    c["W1"] = w["W1"].astype(np.float32)  # [9, 128]
    c["a1blk"] = np.concatenate(
        [_blockdiag(w["a_src1"]), _blockdiag(w["a_dst1"])], axis=1
    ).astype(BF)  # [128, 16]
    c["b1c"] = w["b1"].reshape(D, 1).astype(np.float32)
    c["W2"] = w["W2"].astype(BF)  # [128,128] lhsT
    c["a2blk"] = np.concatenate(
        [w["a_src2"].reshape(D, 1), w["a_dst2"].reshape(D, 1)], axis=1
    ).astype(BF)  # [128, 2]
    c["b2c"] = w["b2"].reshape(D, 1).astype(np.float32)
    for i in (1, 2, 3, 4):
        c[f"fc{i}w"] = w[f"fc{i}_w"].astype(BF)
        c[f"fc{i}b"] = w[f"fc{i}_b"].reshape(D, 1).astype(np.float32)
    c["g1c"] = w["g1"].reshape(D, 1).astype(np.float32)
    c["be1c"] = w["beta1"].reshape(D, 1).astype(np.float32)
    c["g2c"] = w["g2"].reshape(D, 1).astype(np.float32)
    c["be2c"] = w["beta2"].reshape(D, 1).astype(np.float32)
    c["fcfw"] = w["fcf_w"].reshape(D, 1).astype(BF)
    # layer-2 attention on raw (pre-LN) h: zals = h_pre @ (g1*(W2@a_src2)),
    # als2 = rstd1*zals + ofs1@(W2@a_src2); same for dst
    a2s = w["a_src2"].reshape(D).astype(np.float32)
    a2d = w["a_dst2"].reshape(D).astype(np.float32)
    W2f = w["W2"].astype(np.float32)
    g1f = w["g1"].astype(np.float32)
    c["zvec"] = np.stack(
        [g1f * (W2f @ a2s), g1f * (W2f @ a2d)], axis=1
    ).astype(BF)  # [D, 2]
    c["w2asum"] = (W2f @ (a2s + a2d)).reshape(D, 1).astype(np.float32)
    c["iota256"] = np.tile(np.arange(256, dtype=np.float32), (P, 1)).astype(BF)
    c["iota128"] = np.tile(np.arange(P, dtype=np.float32), (P, 1)).astype(BF)
    c["eye128b"] = np.eye(P, dtype=np.float32).astype(BF)
    c["eye16f"] = np.eye(16, dtype=np.float32)
    c["hmap1"] = _headmap(HEADS, HID)  # [8, 128] f32
    c["ones1r"] = np.ones((1, P), dtype=np.float32)
    c["ones128c"] = np.ones((P, 1), dtype=np.float32)
    return c


# ======================================================================
# device program
# ======================================================================
def _chunks(total, width):
    out = []
    o = 0
    while o < total:
        w = min(width, total - o)
        out.append((o, w))
        o += w
    return out


def build_program(meta, debug=False):
    npd, nb, t = meta["npd"], meta["nb"], meta["t"]
    ncores = meta["ncores"]
    mcap = meta["mcap"]
    tile2blk = meta["tile2blk"]
    blk_first = meta["blk_first"]
    blk_last = meta["blk_last"]
    n_glob = meta["n"]
    nl = meta["nl"]
    rg = [list(range(ncores))]
    ch512 = _chunks(npd, 512)
    ln_cnt = float(n_glob * D)  # real elements for graph-LN stats

    nc = bacc.Bacc(
        "TRN2",
        target_bir_lowering=False,
        debug=False,
        enable_asserts=False,
        num_devices=ncores,
        num_swdge_queues=2 if QSPLIT else 1,
    )

    def inp(name, shape, dt):
        return nc.dram_tensor(name, shape, dt, kind="ExternalInput").ap()

    xT = inp("xT", [D_IN, npd], F32)
    gidx = inp("gidx", [P, t], I32)
    drel = inp("drel", [P, t], F32)
    slot = inp("slot", [P, nb], F32)
    iota128 = inp("iota128", [P, P], BF16)
    W1 = inp("W1", [D_IN, D], F32)
    a1blk = inp("a1blk", [D, 2 * HEADS], BF16)
    b1c = inp("b1c", [D, 1], F32)
    W2 = inp("W2", [D, D], BF16)
    a2blk = inp("a2blk", [D, 2], BF16)
    b2c = inp("b2c", [D, 1], F32)
    zvec = inp("zvec", [D, 2], BF16)
    w2asum = inp("w2asum", [D, 1], F32)
    fcw = {i: inp(f"fc{i}w", [D, D], BF16) for i in (1, 2, 3, 4)}
    fcb = {i: inp(f"fc{i}b", [D, 1], F32) for i in (1, 2, 3, 4)}
    g1c = inp("g1c", [D, 1], F32)
    be1c = inp("be1c", [D, 1], F32)
    g2c = inp("g2c", [D, 1], F32)
    be2c = inp("be2c", [D, 1], F32)
    fcfw = inp("fcfw", [D, 1], BF16)
    iota256 = inp("iota256", [P, 256], BF16)
    eye128b = inp("eye128b", [P, P], BF16)
    eye16f = inp("eye16f", [16, 16], F32)
    hmap1 = inp("hmap1", [HEADS, D], F32)
    ones1r = inp("ones1r", [1, P], F32)
    ones128c = inp("ones128c", [P, 1], F32)

    pool_out = nc.dram_tensor("pool_out", [256, 1], F32, kind="ExternalOutput").ap()
    dbg_out = None
    dbg1_out = None
    if debug:
        dbg_out = nc.dram_tensor("dbg_out", [D, npd], F32, kind="ExternalOutput").ap()
        dbg1_out = nc.dram_tensor("dbg1_out", [D, npd], F32, kind="ExternalOutput").ap()

    TW = TBLW  # 256 bf16 = 512B rows: h(128) | al_s f32-pairs | pad

    with tile.TileContext(nc) as tc, ExitStack() as top:
        dram = top.enter_context(tc.tile_pool(name="dram", bufs=1, space="DRAM"))
        persist = top.enter_context(tc.tile_pool(name="persist", bufs=1))
        cpool = top.enter_context(tc.tile_pool(name="consts", bufs=1))
        cbp = top.enter_context(tc.tile_pool(name="cbp", bufs=2, space="PSUM"))
        cbs = top.enter_context(tc.tile_pool(name="cbs", bufs=2))

        def scr():  # one-bank PSUM scratch, callers slice columns
            return cbp.tile([P, 512], F32, tag="scr", name="scr")

        tbl1_loc = dram.tile([npd, TW], BF16, tag="tbl1_loc")
        tbl1_full = dram.tile([ncores * npd, TW], BF16, tag="tbl1_full", addr_space="Shared")
        tbl2_loc = dram.tile([npd, TW], BF16, tag="tbl2_loc")
        tbl2_full = dram.tile([ncores * npd, TW], BF16, tag="tbl2_full", addr_space="Shared")

        ln_in = dram.tile([1, 2], F32, tag="ln_in")
        ln_out = dram.tile([1, 2], F32, tag="ln_out", addr_space="Shared")
        ln_in2 = dram.tile([1, 2], F32, tag="ln_in2")
        ln_out2 = dram.tile([1, 2], F32, tag="ln_out2", addr_space="Shared")

        # persistent activations (transposed [feat, node]) and edge-index tables
        hc_sb = persist.tile([P, npd], BF16, tag="hc_sb")  # residual
        hf_sb = persist.tile([P, npd], BF16, tag="hf_sb")  # working activation
        drel_sb = persist.tile([P, t], F32, tag="drel_sb")
        gidx_sb = persist.tile([P, t], I32, tag="gidx_sb")
        ald1_sb = persist.tile([P, nb * HEADS], BF16, tag="ald1_sb")
        ald2_sb = persist.tile([P, nb], BF16, tag="ald2_sb")
        als1_sb = persist.tile([P, nb * HEADS], BF16, tag="als1_sb")
        als2_sb = persist.tile([P, nb], BF16, tag="als2_sb")
        hnm_sb = persist.tile([P, npd], BF16, tag="hnm_sb")  # node-major h
        # LN1-derived affine terms (filled post-E1, consumed in E2)
        sc1_sb = persist.tile([P, 1], F32, tag="sc1_sb")
        ofs1_sb = persist.tile([P, 1], F32, tag="ofs1_sb")
        rstd1_sb = persist.tile([P, 1], F32, tag="rstd1_sb")
        cc_sb = persist.tile([P, 1], F32, tag="cc_sb")
        c2_sb = persist.tile([P, 1], F32, tag="c2_sb")
        ofsrow_sb = persist.tile([1, P], F32, tag="ofsrow_sb")

        # constants in SBUF
        def cload(ap_in, shape, dt, tag):
            s = cpool.tile(shape, dt, tag=tag)
            nc.sync.dma_start(out=s[:], in_=ap_in)
            return s

        W1_s = cload(W1, [D_IN, D], F32, "W1")
        a1_s = cload(a1blk, [D, 2 * HEADS], BF16, "a1")
        a2_s = cload(a2blk, [D, 2], BF16, "a2")
        b1_s = cload(b1c, [D, 1], F32, "b1")
        b2_s = cload(b2c, [D, 1], F32, "b2")
        zvec_s = cload(zvec, [D, 2], BF16, "zvec")
        w2asum_s = cload(w2asum, [D, 1], F32, "w2asum")
        fcw_s = {i: cload(fcw[i], [D, D], BF16, f"fw{i}") for i in (1, 2, 3, 4)}
        fcb_s = {i: cload(fcb[i], [D, 1], F32, f"fb{i}") for i in (1, 2, 3, 4)}
        g1_s = cload(g1c, [D, 1], F32, "g1")
        be1_s = cload(be1c, [D, 1], F32, "be1")
        g2_s = cload(g2c, [D, 1], F32, "g2")
        be2_s = cload(be2c, [D, 1], F32, "be2")
        W2_s = cload(W2, [D, D], BF16, "W2")
        fcf_s = cload(fcfw, [D, 1], BF16, "fcf")
        io256_s = cload(iota256, [P, 256], BF16, "io256")
        io128_s = cload(iota128, [P, P], BF16, "io128")
        eye_s = cload(eye128b, [P, P], BF16, "eye")
        eye16_s = cload(eye16f, [16, 16], F32, "eye16")
        hmap1_s = cload(hmap1, [HEADS, D], F32, "hmap1")
        ones1r_s = cload(ones1r, [1, P], F32, "ones1r")
        ones128c_s = cload(ones128c, [P, 1], F32, "ones128c")
        slot_s = cload(slot, [P, nb], F32, "slot")

        nc.sync.dma_start(out=drel_sb[:], in_=drel)
        nc.sync.dma_start(out=gidx_sb[:], in_=gidx)

        # ---------------- stage A for a GAT layer: build tables ----------------
        def stage_a(layer, src_hT_fn):
            """Write tbl{layer}_loc rows [h bf16 | al_s f32] and ald table; then AllGather.
            src_hT_fn(c0, w, ps): fills psum tile [128, w] with this layer's hT chunk."""
            heads = HEADS if layer == 1 else 1
            a_s = a1_s if layer == 1 else a2_s
            tbl_loc = tbl1_loc if layer == 1 else tbl2_loc
            tbl_full = tbl1_full if layer == 1 else tbl2_full
            ald_sb = ald1_sb if layer == 1 else ald2_sb
            als_sb = als1_sb if layer == 1 else als2_sb
            with (
                tc.tile_pool(name=f"sa{layer}", bufs=5) as sp,
                tc.tile_pool(name=f"sap{layer}", bufs=2, space="PSUM") as pp,
                tc.tile_pool(name=f"sat{layer}", bufs=2, space="PSUM") as tp2,
            ):
                for c0, w in ch512:
                    ps = pp.tile([P, 512], F32, tag="hps")
                    src_hT_fn(c0, w, ps)
                    # keep transposed activation for downstream dense chain
                    nc.vector.tensor_copy(out=hf_sb[:, c0 : c0 + w], in_=ps[:, :w])
                for k in range(nb):
                    c0 = k * P
                    al_t = tp2.tile([P, 256], F32, tag="al_t", name="al_t")
                    # al_s/al_d for this chunk: [2*heads, 128] = a^T @ hT
                    alps = al_t[0 : 2 * heads, 0:P]
                    nc.tensor.matmul(
                        out=alps, lhsT=a_s[:], rhs=hf_sb[:, c0 : c0 + P],
                        start=True, stop=True,
                    )
                    asb = sp.tile([2 * heads, P], F32, tag="asb")
                    nc.vector.tensor_copy(out=asb[:], in_=alps)
                    # transpose -> [128, 2*heads]
                    atp = al_t[:, P : P + 2 * heads]
                    nc.tensor.transpose(
                        out=atp, in_=asb[:], identity=eye16_s[: 2 * heads, : 2 * heads]
                    )
                    ats = sp.tile([P, 2 * heads], F32, tag="ats")
                    nc.vector.tensor_copy(out=ats[:], in_=atp)
                    # transpose h chunk -> node-major [128n, 128f]
                    ht_t = tp2.tile([P, P], BF16, tag="ht_t", name="ht_t")
                    htp = ht_t[:]
                    nc.tensor.transpose(
                        out=htp, in_=hf_sb[:, c0 : c0 + P], identity=eye_s[:]
                    )
                    nc.vector.tensor_copy(
                        out=hnm_sb[:, c0 : c0 + P], in_=htp
                    )
                    # table writes
                    nc.sync.dma_start(
                        out=tbl_loc[c0 : c0 + P, 0:D], in_=hnm_sb[:, c0 : c0 + P]
                    )
                    nc.sync.dma_start(
                        out=tbl_loc[c0 : c0 + P, D : D + 2 * heads],
                        in_=ats[:].bitcast(BF16)[:, 0 : 2 * heads],
                    )
                    nc.vector.tensor_copy(
                        out=ald_sb[:, k * heads : (k + 1) * heads],
                        in_=ats[:, heads : 2 * heads],
                    )
                    nc.vector.tensor_copy(
                        out=als_sb[:, k * heads : (k + 1) * heads],
                        in_=ats[:, 0:heads],
                    )
            nc.gpsimd.collective_compute(
                "AllGather",
                ALU.bypass,
                replica_groups=rg,
                ins=[tbl_loc[:].opt()],
                outs=[tbl_full[:].opt()],
            )

        # ---------------- edge phase for a GAT layer ----------------
        def edge_phase(layer, out_cb):
            """GAT aggregation; out_cb(b, osb) gets [128f, 128n] f32 sbuf tile.

            Per tile: one indirect row-gather (h|al_s), one-hot S on DVE,
            St = transpose(S) on PE feeds the al_d expansion matmul; agg/den
            matmuls accumulate into quad-packed PSUM banks (4 blocks/bank).
            """
            heads = HEADS if layer == 1 else 1
            tbl_full = tbl1_full if layer == 1 else tbl2_full
            ald_sb = ald1_sb if layer == 1 else ald2_sb
            als_sb = als1_sb if layer == 1 else als2_sb
            nwin = (t + WIN - 1) // WIN
            agg_banks = {}
            den_banks = {}
            with (
                tc.tile_pool(name=f"eg{layer}", bufs=4) as gp,
                tc.tile_pool(name=f"eb{layer}", bufs=2) as bigp,
                tc.tile_pool(name=f"es{layer}", bufs=3) as sp,
                tc.tile_pool(name=f"est{layer}", bufs=WIN + 4) as stpool,
                tc.tile_pool(name=f"ef{layer}", bufs=2) as fin,
                tc.tile_pool(name=f"ep{layer}", bufs=2, space="PSUM") as pp,
                tc.tile_pool(name=f"ed{layer}", bufs=1, space="PSUM") as dp,
                tc.tile_pool(name=f"ea{layer}", bufs=1, space="PSUM") as ap_,
                tc.tile_pool(name=f"et{layer}", bufs=1, space="PSUM") as tp_,
                tc.tile_pool(name=f"er{layer}", bufs=1, space="PSUM") as rp,
            ):
                # first-touch memset so never-gathered pad partitions stay finite
                for _ in range(4):
                    gm = gp.tile([P, WIN * TW], BF16, tag="gath")
                    nc.gpsimd.memset(gm[:], 0.0)
                for wi in range(nwin):
                    t0 = wi * WIN
                    L = min(WIN, t - t0)
                    gath = gp.tile([P, WIN * TW], BF16, tag="gath")
                    for i in range(L):
                        m = mcap[t0 + i]
                        bi = nc.gpsimd.indirect_dma_start(
                            out=gath[0:m, i * TW : (i + 1) * TW],
                            out_offset=None,
                            in_=tbl_full[:],
                            in_offset=IndirectOffsetOnAxis(
                                ap=gidx_sb[0:m, t0 + i : t0 + i + 1], axis=0
                            ),
                        )
                        if QSPLIT and (t0 + i) % 2 == 1:
                            bi.ins.queue = "qPoolDynamic1"
                    aldw = ap_.tile([P, WIN * heads], F32, tag="aldw")
                    st_list = []
                    for i in range(L):
                        ti = t0 + i
                        s_t = stpool.tile([P, P], BF16, tag="s_t")
                        nc.vector.tensor_scalar(
                            out=s_t[:], in0=io128_s[:],
                            scalar1=drel_sb[:, ti : ti + 1], scalar2=None,
                            op0=ALU.is_equal,
                        )
                        st_list.append(s_t)
                    # batched St: 4 transposes share one PSUM bank, one ACT copy
                    for g0 in range(0, L, 4):
                        gl = min(4, L - g0)
                        stp = tp_.tile([P, 4 * P], BF16, tag="stp")
                        for j in range(gl):
                            nc.tensor.matmul(
                                out=stp[:, j * P : (j + 1) * P],
                                lhsT=st_list[g0 + j][:],
                                rhs=eye_s[:],
                                is_transpose=True,
                                start=(j == 0),
                                stop=(j == gl - 1),
                                skip_group_check=True,
                            )
                        sts = sp.tile([P, 4 * P], BF16, tag="sts")
                        nc.scalar.activation(
                            out=sts[:, 0 : gl * P], in_=stp[:, 0 : gl * P],
                            func=ACTF.Copy,
                        )
                        for j in range(gl):
                            i = g0 + j
                            b = tile2blk[t0 + i]
                            nc.tensor.matmul(
                                out=aldw[:, i * heads : (i + 1) * heads],
                                lhsT=sts[:, j * P : (j + 1) * P],
                                rhs=ald_sb[:, b * heads : (b + 1) * heads],
                                start=(i == 0),
                                stop=(i == L - 1),
                                skip_group_check=True,
                            )
                    # e = al_s[src] + al_d[dst]
                    als_v = (
                        gath[:]
                        .bitcast(F32)
                        .rearrange("p (t w) -> p t w", w=TW // 2)[
                            :, 0:L, D // 2 : D // 2 + heads
                        ]
                    )
                    e_sb = sp.tile([P, WIN * heads], F32, tag="e_sb")
                    nc.vector.tensor_tensor(
                        out=e_sb[:, 0 : L * heads].rearrange(
                            "p (t h) -> p t h", h=heads
                        ),
                        in0=als_v,
                        in1=aldw[:, 0 : L * heads].rearrange(
                            "p (t h) -> p t h", h=heads
                        ),
                        op=ALU.add,
                    )
                    if layer == 2:
                        # raw z-logits -> true logits: e = rstd1*e + CC
                        nc.vector.tensor_scalar(
                            out=e_sb[:, 0 : L * heads],
                            in0=e_sb[:, 0 : L * heads],
                            scalar1=rstd1_sb[:], scalar2=cc_sb[:],
                            op0=ALU.mult, op1=ALU.add,
                        )
                    t02 = sp.tile([P, WIN * heads], F32, tag="t02")
                    nc.vector.tensor_scalar(
                        out=t02[:, 0 : L * heads], in0=e_sb[:, 0 : L * heads],
                        scalar1=NEG_SLOPE, scalar2=None, op0=ALU.mult,
                    )
                    lr_sb = sp.tile([P, WIN * heads], F32, tag="lr_sb")
                    nc.vector.tensor_tensor(
                        out=lr_sb[:, 0 : L * heads], in0=e_sb[:, 0 : L * heads],
                        in1=t02[:, 0 : L * heads], op=ALU.max,
                    )
                    pexp = bigp.tile([P, WIN * D], BF16, tag="pexp")
                    nc.scalar.activation(
                        out=pexp[:, 0 : L * D].rearrange(
                            "p (t h c) -> p t h c", h=heads, c=D // heads
                        ),
                        in_=lr_sb[:, 0 : L * heads]
                        .rearrange("p (t h) -> p t h", h=heads)
                        .broadcast_to((P, L, heads, D // heads)),
                        func=ACTF.Exp,
                    )
                    p_sb = sp.tile([P, WIN * heads], BF16, tag="p_sb")
                    nc.scalar.activation(
                        out=p_sb[:, 0 : L * heads], in_=lr_sb[:, 0 : L * heads],
                        func=ACTF.Exp,
                    )
                    wh = bigp.tile([P, WIN * D], BF16, tag="wh")
                    nc.vector.tensor_tensor(
                        out=wh[:, 0 : L * D].rearrange("p (t c) -> p t c", c=D),
                        in0=gath[:].rearrange("p (t w) -> p t w", w=TW)[:, 0:L, 0:D],
                        in1=pexp[:, 0 : L * D].rearrange("p (t c) -> p t c", c=D),
                        op=ALU.mult,
                    )
                    for i in range(L):
                        ti = t0 + i
                        b = tile2blk[ti]
                        q = b // 4  # quad id
                        qi = b % 4
                        if q not in agg_banks:
                            qblocks = [bb for bb in range(4 * q, min(4 * q + 4, nb))]
                            ab = pp.tile([P, 512], F32, tag="aggq", name="aggq")
                            db = dp.tile([8, 512], F32, tag="denq", name="denq")
                            agg_banks[q] = (
                                ab, blk_first[qblocks[0]], blk_last[qblocks[-1]]
                            )
                            den_banks[q] = (
                                db, blk_first[qblocks[0]], blk_last[qblocks[-1]]
                            )
                        ab, qf, ql = agg_banks[q]
                        db, _, _ = den_banks[q]
                        nc.tensor.matmul(
                            out=ab[:, qi * P : (qi + 1) * P],
                            lhsT=wh[:, i * D : (i + 1) * D],
                            rhs=st_list[i][:],
                            start=(ti == qf),
                            stop=False,
                            skip_group_check=True,
                        )
                        nc.tensor.matmul(
                            out=db[0:heads, qi * P : (qi + 1) * P],
                            lhsT=p_sb[:, i * heads : (i + 1) * heads],
                            rhs=st_list[i][:],
                            start=(ti == qf),
                            stop=False,
                            skip_group_check=True,
                        )
                        if ti == blk_last[b]:
                            # diagonal (self-loop) contribution: h and al are
                            # local, injected as one pseudo-tile (S = identity)
                            esf = fin.tile([P, HEADS], F32, tag="esf")
                            nc.vector.tensor_tensor(
                                out=esf[:, 0:heads],
                                in0=als_sb[:, b * heads : (b + 1) * heads],
                                in1=ald_sb[:, b * heads : (b + 1) * heads],
                                op=ALU.add,
                            )
                            if layer == 2:
                                nc.vector.tensor_scalar(
                                    out=esf[:, 0:heads], in0=esf[:, 0:heads],
                                    scalar1=rstd1_sb[:], scalar2=cc_sb[:],
                                    op0=ALU.mult, op1=ALU.add,
                                )
                            tsf = fin.tile([P, HEADS], F32, tag="tsf")
                            nc.vector.tensor_scalar(
                                out=tsf[:, 0:heads], in0=esf[:, 0:heads],
                                scalar1=NEG_SLOPE, scalar2=None, op0=ALU.mult,
                            )
                            lrs = fin.tile([P, HEADS], F32, tag="lrs")
                            nc.vector.tensor_tensor(
                                out=lrs[:, 0:heads], in0=esf[:, 0:heads],
                                in1=tsf[:, 0:heads], op=ALU.max,
                            )
                            pxs = fin.tile([P, D], BF16, tag="pxs")
                            nc.scalar.activation(
                                out=pxs[:].rearrange("p (h c) -> p h c", h=heads),
                                in_=lrs[:, 0:heads].broadcast_to(
                                    (P, heads, D // heads)
                                ),
                                func=ACTF.Exp,
                            )
                            psbs = fin.tile([P, HEADS], BF16, tag="psbs")
                            nc.scalar.activation(
                                out=psbs[:, 0:heads], in_=lrs[:, 0:heads],
                                func=ACTF.Exp,
                            )
                            whs = fin.tile([P, D], BF16, tag="whs")
                            nc.vector.tensor_tensor(
                                out=whs[:], in0=hnm_sb[:, b * P : (b + 1) * P],
                                in1=pxs[:], op=ALU.mult,
                            )
                            nc.tensor.matmul(
                                out=ab[:, qi * P : (qi + 1) * P],
                                lhsT=whs[:], rhs=eye_s[:],
                                start=False, stop=(ti == ql),
                                skip_group_check=True,
                            )
                            nc.tensor.matmul(
                                out=db[0:heads, qi * P : (qi + 1) * P],
                                lhsT=psbs[:, 0:heads], rhs=eye_s[:],
                                start=False, stop=(ti == ql),
                                skip_group_check=True,
                            )
                            dsb = fin.tile([heads, P], F32, tag="dsb")
                            nc.vector.tensor_scalar(
                                out=dsb[:], in0=db[0:heads, qi * P : (qi + 1) * P],
                                scalar1=1e-16, scalar2=None, op0=ALU.add,
                            )
                            rsb = fin.tile([heads, P], F32, tag="rsb")
                            nc.vector.reciprocal(out=rsb[:], in_=dsb[:])
                            rp_t = rp.tile([P, 512], F32, tag="rscr")
                            rex = rp_t[:, 0:P]
                            nc.tensor.matmul(
                                out=rex,
                                lhsT=(ones1r_s if heads == 1 else hmap1_s)[:],
                                rhs=rsb[:], start=True, stop=True,
                            )
                            rxs = fin.tile([P, P], F32, tag="rxs")
                            nc.scalar.activation(out=rxs[:], in_=rex, func=ACTF.Copy)
                            osb = fin.tile([P, P], F32, tag="osb")
                            if layer == 2:
                                # out2 = [sc1*ab + ofs1*den] @ W2 / den
                                tmul = fin.tile([P, P], F32, tag="tmul")
                                nc.vector.tensor_scalar(
                                    out=tmul[:],
                                    in0=ab[:, qi * P : (qi + 1) * P],
                                    scalar1=sc1_sb[:], scalar2=None,
                                    op0=ALU.mult,
                                )
                                t2l = fin.tile([P, P], BF16, tag="t2l")
                                nc.vector.tensor_copy(out=t2l[:], in_=tmul[:])
                                ps2 = rp_t[:, 2 * P : 3 * P]
                                nc.tensor.matmul(
                                    out=ps2, lhsT=W2_s[:], rhs=t2l[:],
                                    start=True, stop=True,
                                )
                                nc.vector.tensor_tensor(
                                    out=osb[:], in0=ps2, in1=rxs[:],
                                    op=ALU.mult,
                                )
                            else:
                                nc.vector.tensor_tensor(
                                    out=osb[:], in0=ab[:, qi * P : (qi + 1) * P],
                                    in1=rxs[:], op=ALU.mult,
                                )
                            out_cb(b, osb)
                            if b == 4 * q + 3 or b == nb - 1:
                                del agg_banks[q], den_banks[q]

        # ---------------- graph-LN over h_sb -> dst_sb (bf16) ----------------
        def graph_ln(src_sb, g_s, be_s, ln_i, ln_o, dst_sb, stash=False):
            with (
                tc.tile_pool(name="ln", bufs=2) as sp,
                tc.tile_pool(name="lnp", bufs=2, space="PSUM") as pp,
            ):
                nchunk = len(ch512)
                stats = sp.tile([P, nchunk * 6], F32, tag="stats")
                for ci, (c0, w) in enumerate(ch512):
                    nc.vector.bn_stats(
                        out=stats[:, ci * 6 : (ci + 1) * 6], in_=src_sb[:, c0 : c0 + w]
                    )
                mv = sp.tile([P, 2], F32, tag="mv")
                nc.vector.bn_aggr(
                    out=mv[:], in_=stats[:].rearrange("p (c s) -> p c s", s=6)
                )
                # per-partition sums: [sx, sxx] = [m, (v+m^2)] * npd
                sums = sp.tile([P, 2], F32, tag="sums")
                nc.vector.tensor_scalar(
                    out=sums[:, 0:1], in0=mv[:, 0:1], scalar1=float(npd),
                    scalar2=None, op0=ALU.mult,
                )
                m2 = sp.tile([P, 1], F32, tag="m2")
                nc.vector.tensor_tensor(
                    out=m2[:], in0=mv[:, 0:1], in1=mv[:, 0:1], op=ALU.mult
                )
                nc.vector.tensor_tensor(
                    out=sums[:, 1:2], in0=mv[:, 1:2], in1=m2[:], op=ALU.add
                )
                nc.vector.tensor_scalar(
                    out=sums[:, 1:2], in0=sums[:, 1:2], scalar1=float(npd),
                    scalar2=None, op0=ALU.mult,
                )
                red = pp.tile([1, 2], F32, tag="red")
                nc.tensor.matmul(
                    out=red[:], lhsT=ones128c_s[:], rhs=sums[:], start=True, stop=True
                )
                rsb = sp.tile([1, 2], F32, tag="rsb2")
                nc.vector.tensor_copy(out=rsb[:], in_=red[:])
                nc.gpsimd.dma_start(out=ln_i[:], in_=rsb[:])
                nc.gpsimd.collective_compute(
                    "AllReduce", ALU.add, replica_groups=rg,
                    ins=[ln_i[:].opt()], outs=[ln_o[:].opt()],
                )
                ar = sp.tile([1, 2], F32, tag="ar")
                nc.gpsimd.dma_start(out=ar[:], in_=ln_o[:])
                bc = pp.tile([P, 2], F32, tag="bc")
                nc.tensor.matmul(
                    out=bc[:], lhsT=ones1r_s[:], rhs=ar[:], start=True, stop=True
                )
                # mu = s1/cnt ; var = s2/cnt - mu^2 ; s = g * rsqrt(var+eps) ; b = be - mu*s
                mu = sp.tile([P, 1], F32, tag="mu")
                nc.vector.tensor_scalar(
                    out=mu[:], in0=bc[:, 0:1], scalar1=1.0 / ln_cnt, scalar2=None,
                    op0=ALU.mult,
                )
                var = sp.tile([P, 1], F32, tag="var")
                nc.vector.tensor_scalar(
                    out=var[:], in0=bc[:, 1:2], scalar1=1.0 / ln_cnt, scalar2=None,
                    op0=ALU.mult,
                )
                mu2 = sp.tile([P, 1], F32, tag="mu2")
                nc.vector.tensor_tensor(out=mu2[:], in0=mu[:], in1=mu[:], op=ALU.mult)
                nc.vector.tensor_tensor(
                    out=var[:], in0=var[:], in1=mu2[:], op=ALU.subtract
                )
                nc.vector.tensor_scalar(
                    out=var[:], in0=var[:], scalar1=LN_EPS, scalar2=None, op0=ALU.add
                )
                sd = sp.tile([P, 1], F32, tag="sd")
                nc.scalar.activation(out=sd[:], in_=var[:], func=ACTF.Sqrt)
                rstd = sp.tile([P, 1], F32, tag="rstd")
                nc.vector.reciprocal(out=rstd[:], in_=sd[:])
                sc = sp.tile([P, 1], F32, tag="sc")
                nc.vector.tensor_tensor(out=sc[:], in0=g_s[:], in1=rstd[:], op=ALU.mult)
                ofs = sp.tile([P, 1], F32, tag="ofs")
                nc.vector.tensor_tensor(out=ofs[:], in0=mu[:], in1=sc[:], op=ALU.mult)
                nc.vector.tensor_tensor(
                    out=ofs[:], in0=be_s[:], in1=ofs[:], op=ALU.subtract
                )
                if stash:
                    nc.vector.tensor_copy(out=rstd1_sb[:], in_=rstd[:])
                    nc.vector.tensor_copy(out=sc1_sb[:], in_=sc[:])
                    nc.vector.tensor_copy(out=ofs1_sb[:], in_=ofs[:])
                for c0, w in ch512:
                    nc.scalar.activation(
                        out=dst_sb[:, c0 : c0 + w], in_=src_sb[:, c0 : c0 + w],
                        func=ACTF.Identity, bias=ofs[:], scale=sc[:],
                    )
                # zero the padded node columns so they don't pollute later stats
                if npd > nl:
                    nc.gpsimd.memset(dst_sb[:, nl:npd], 0.0)

        # =========================== the network ===========================
        # ---- GAT layer 1 stage A: h1 = x @ W1 (transposed) ----
        xin_pool = top.enter_context(tc.tile_pool(name="xin", bufs=2))

        def h1_fn(c0, w, ps):
            xt = xin_pool.tile([D_IN, 512], F32, tag="xt")
            nc.sync.dma_start(out=xt[:, :w], in_=xT[:, c0 : c0 + w])
            nc.tensor.matmul(
                out=ps[:, :w], lhsT=W1_s[:], rhs=xt[:, :w], start=True, stop=True
            )

        stage_a(1, h1_fn)

        # ---- GAT layer 1 edge phase; per-block callback also runs fc1/fc2
        # and builds the raw (pre-LN) layer-2 table [h_pre | zals], so AG2
        # can start immediately when E1 drains and LN1 runs under it. ----
        def out1_cb(b, osb):
            c0 = b * P
            h1o = hc_sb[:, c0 : c0 + P]
            nc.scalar.activation(
                out=h1o, in_=osb[:], func=ACTF.Relu, bias=b1_s[:], scale=1.0,
            )
            s_ = scr()
            ps1 = s_[:, 0:P]
            nc.tensor.matmul(
                out=ps1, lhsT=fcw_s[1][:], rhs=h1o, start=True, stop=True
            )
            t1 = cbs.tile([P, P], BF16, tag="t1")
            nc.scalar.activation(
                out=t1[:], in_=ps1, func=ACTF.Relu, bias=fcb_s[1][:], scale=1.0
            )
            psf = s_[:, P : 2 * P]
            nc.tensor.matmul(
                out=psf, lhsT=fcw_s[2][:], rhs=t1[:], start=True, stop=True
            )
            t2f = cbs.tile([P, P], F32, tag="t2f")
            nc.vector.tensor_tensor(out=t2f[:], in0=psf, in1=h1o, op=ALU.add)
            nc.scalar.activation(
                out=hf_sb[:, c0 : c0 + P], in_=t2f[:], func=ACTF.Relu,
                bias=fcb_s[2][:], scale=1.0,
            )
            # raw layer-2 attention z-values
            zz = s_[0:2, 2 * P : 3 * P]
            nc.tensor.matmul(
                out=zz, lhsT=zvec_s[:], rhs=hf_sb[:, c0 : c0 + P],
                start=True, stop=True,
            )
            zzs = cbs.tile([2, P], F32, tag="zzs")
            nc.vector.tensor_copy(out=zzs[:], in_=zz)
            atp2 = s_[:, 3 * P : 3 * P + 2]
            nc.tensor.transpose(out=atp2, in_=zzs[:], identity=eye16_s[0:2, 0:2])
            ats2 = cbs.tile([P, 2], F32, tag="ats2")
            nc.vector.tensor_copy(out=ats2[:], in_=atp2)
            nc.vector.tensor_copy(out=ald2_sb[:, b : b + 1], in_=ats2[:, 1:2])
            nc.vector.tensor_copy(out=als2_sb[:, b : b + 1], in_=ats2[:, 0:1])
            htp2 = s_[:].bitcast(BF16)[:, 772:900]
            nc.tensor.transpose(
                out=htp2, in_=hf_sb[:, c0 : c0 + P], identity=eye_s[:]
            )
            nc.vector.tensor_copy(out=hnm_sb[:, c0 : c0 + P], in_=htp2)
            nc.sync.dma_start(
                out=tbl2_loc[c0 : c0 + P, 0:D], in_=hnm_sb[:, c0 : c0 + P]
            )
            nc.sync.dma_start(
                out=tbl2_loc[c0 : c0 + P, D : D + 2],
                in_=ats2[:].bitcast(BF16)[:, 0:2],
            )

        edge_phase(1, out1_cb)
        nc.gpsimd.collective_compute(
            "AllGather",
            ALU.bypass,
            replica_groups=rg,
            ins=[tbl2_loc[:].opt()],
            outs=[tbl2_full[:].opt()],
        )
        if debug:
            with tc.tile_pool(name="dbg1p", bufs=2) as dbp1:
                for c0, w in ch512:
                    dt1 = dbp1.tile([P, 512], F32, tag="dbg1")
                    nc.vector.tensor_copy(out=dt1[:, :w], in_=hc_sb[:, c0 : c0 + w])
                    nc.sync.dma_start(out=dbg1_out[:, c0 : c0 + w], in_=dt1[:, :w])

        # ---- LN1 (stats + AR + affine; runs under AG2), then the E2
        # affine constants derived from (rstd1, ofs1) ----
        graph_ln(hf_sb, g1_s, be1_s, ln_in, ln_out, hc_sb, stash=True)
        with (
            tc.tile_pool(name="epl", bufs=1) as epp,
            tc.tile_pool(name="eplp", bufs=1, space="PSUM") as epps,
        ):
            m1 = epps.tile([1, 1], F32, tag="m1")
            nc.tensor.matmul(
                out=m1[:], lhsT=ofs1_sb[:], rhs=w2asum_s[:], start=True, stop=True
            )
            m1s = epp.tile([1, 1], F32, tag="m1s")
            nc.vector.tensor_copy(out=m1s[:], in_=m1[:])
            ccp = epps.tile([P, 1], F32, tag="ccp")
            nc.tensor.matmul(
                out=ccp[:], lhsT=ones1r_s[:], rhs=m1s[:], start=True, stop=True
            )
            nc.vector.tensor_copy(out=cc_sb[:], in_=ccp[:])
            ofsb = epp.tile([P, 1], BF16, tag="ofsb")
            nc.vector.tensor_copy(out=ofsb[:], in_=ofs1_sb[:])
            c2p = epps.tile([P, 1], F32, tag="c2p")
            nc.tensor.matmul(
                out=c2p[:], lhsT=W2_s[:], rhs=ofsb[:], start=True, stop=True
            )
            c2t = epp.tile([P, 1], F32, tag="c2t")
            nc.vector.tensor_copy(out=c2t[:], in_=c2p[:])
            nc.vector.tensor_tensor(
                out=c2_sb[:], in0=c2t[:], in1=b2_s[:], op=ALU.add
            )
            orp = epps.tile([1, P], F32, tag="orp")
            nc.tensor.matmul(
                out=orp[:], lhsT=ofsb[:], rhs=eye_s[:], start=True, stop=True
            )
            nc.vector.tensor_copy(out=ofsrow_sb[:], in_=orp[:])

        # ---- GAT layer 2 edge phase (raw table; LN+W2 folded into the
        # finalize) -> hf_sb = out2 + (W2^T ofs1 + b2) ----
        def out2_cb(b, osb):
            nc.scalar.activation(
                out=hf_sb[:, b * P : (b + 1) * P], in_=osb[:], func=ACTF.Identity,
                bias=c2_sb[:], scale=1.0,
            )

        edge_phase(2, out2_cb)

        # ---- fc3 (relu), fc4 (+residual h_ln1), LN2, fcf, pool ----
        with (
            tc.tile_pool(name="fc2", bufs=3) as fp,
            tc.tile_pool(name="fcp2", bufs=2, space="PSUM") as fpp,
        ):
            for c0, w in ch512:
                ps = fpp.tile([P, 512], F32, tag="fc3p")
                nc.tensor.matmul(
                    out=ps[:, :w], lhsT=fcw_s[3][:], rhs=hf_sb[:, c0 : c0 + w],
                    start=True, stop=True,
                )
                nc.scalar.activation(
                    out=hf_sb[:, c0 : c0 + w], in_=ps[:, :w], func=ACTF.Relu,
                    bias=fcb_s[3][:], scale=1.0,
                )
            for c0, w in ch512:
                ps = fpp.tile([P, 512], F32, tag="fc4p")
                nc.tensor.matmul(
                    out=ps[:, :w], lhsT=fcw_s[4][:], rhs=hf_sb[:, c0 : c0 + w],
                    start=True, stop=True,
                )
                tmp = fp.tile([P, 512], F32, tag="fc4t")
                nc.vector.tensor_tensor(
                    out=tmp[:, :w], in0=ps[:, :w], in1=hc_sb[:, c0 : c0 + w], op=ALU.add
                )
                nc.scalar.activation(
                    out=hf_sb[:, c0 : c0 + w], in_=tmp[:, :w], func=ACTF.Identity,
                    bias=fcb_s[4][:], scale=1.0,
                )
        graph_ln(hf_sb, g2_s, be2_s, ln_in2, ln_out2, hc_sb)  # hc_sb = h_ln2

        if debug:
            with tc.tile_pool(name="dbgp", bufs=2) as dbp:
                for c0, w in ch512:
                    dt_ = dbp.tile([P, 512], F32, tag="dbg")
                    nc.vector.tensor_copy(out=dt_[:, :w], in_=hc_sb[:, c0 : c0 + w])
                    nc.sync.dma_start(out=dbg_out[:, c0 : c0 + w], in_=dt_[:, :w])

        # fcf + per-core pooled slot sums
        with (
            tc.tile_pool(name="pl", bufs=6) as sp,
            tc.tile_pool(name="plp", bufs=4, space="PSUM") as pp,
            tc.tile_pool(name="plq", bufs=1, space="PSUM") as qq,
        ):
            pool_a = qq.tile([P, 1], F32, tag="pool_a")
            pool_b = qq.tile([P, 1], F32, tag="pool_b")
            for k in range(nb):
                c0 = k * P
                hv = pp.tile([P, 1], F32, tag="hv")
                nc.tensor.matmul(
                    out=hv[:], lhsT=hc_sb[:, c0 : c0 + P], rhs=fcf_s[:],
                    start=True, stop=True,
                )
                hvs = sp.tile([P, 1], F32, tag="hvs")
                nc.vector.tensor_copy(out=hvs[:], in_=hv[:])
                oh = sp.tile([P, 256], BF16, tag="oh")
                nc.vector.tensor_scalar(
                    out=oh[:], in0=io256_s[:], scalar1=slot_s[:, k : k + 1],
                    scalar2=None, op0=ALU.is_equal,
                )
                ohf = sp.tile([P, 256], F32, tag="ohf")
                nc.vector.tensor_copy(out=ohf[:], in_=oh[:])
                nc.tensor.matmul(
                    out=pool_a[:], lhsT=ohf[:, 0:P], rhs=hvs[:],
                    start=(k == 0), stop=(k == nb - 1),
                )
                nc.tensor.matmul(
                    out=pool_b[:], lhsT=ohf[:, P:256], rhs=hvs[:],
                    start=(k == 0), stop=(k == nb - 1),
                )
            pool_sb = sp.tile([P, 2], F32, tag="pool_sb")
            nc.vector.tensor_copy(out=pool_sb[:, 0:1], in_=pool_a[:])
            nc.vector.tensor_copy(out=pool_sb[:, 1:2], in_=pool_b[:])
            nc.sync.dma_start(out=pool_out[0:P, 0:1], in_=pool_sb[:, 0:1])
            nc.sync.dma_start(out=pool_out[P:256, 0:1], in_=pool_sb[:, 1:2])

    nc.compile()
    return nc


# ======================================================================
# driver
# ======================================================================
def _in_maps(meta, core_arrays, slots, consts, x):
    nl, npd = meta["nl"], meta["npd"]
    maps = []
    for c in range(meta["ncores"]):
        gidx_pi, drel_pi = core_arrays[c]
        xT = np.zeros((D_IN, npd), dtype=np.float32)
        xT[:, :nl] = np.asarray(x[c * nl : (c + 1) * nl], dtype=np.float32).T
        m = dict(
            xT=xT, gidx=gidx_pi, drel=drel_pi, slot=slots[c]
        )
        m.update(consts)
        maps.append(m)
    return maps


def _install_ntff_shim():
    """Provide antenv.axon_hooks via direct ctypes into libaxon_pjrt.so."""
    import types, contextlib, ctypes

    try:
        import antenv.axon_hooks  # noqa: F401

        return True
    except ImportError:
        pass
    so_path = "/opt/axon/libaxon_pjrt.so"
    try:
        lib = ctypes.CDLL(so_path)
    except OSError:
        return False
    if not hasattr(lib, "axon_start_nrt_profile"):
        return False
    lib.axon_start_nrt_profile.argtypes = [
        ctypes.POINTER(ctypes.c_int64),
        ctypes.c_size_t,
    ]
    lib.axon_start_nrt_profile.restype = ctypes.c_int64
    lib.axon_stop_nrt_profile.argtypes = [ctypes.c_char_p]
    lib.axon_stop_nrt_profile.restype = ctypes.c_int64

    @contextlib.contextmanager
    def _hook(output_dir, device_ids):
        import jax

        jax.devices()
        if device_ids:
            ids = (ctypes.c_int64 * len(device_ids))(*device_ids)
            rc = lib.axon_start_nrt_profile(ids, len(device_ids))
        else:
            rc = lib.axon_start_nrt_profile(None, 0)
        if rc != 0:
            raise RuntimeError(f"axon_start_nrt_profile rc={rc}")
        try:
            yield
        finally:
            nfiles = lib.axon_stop_nrt_profile(str(output_dir).encode())
            print(f"ntff profile: {nfiles} file(s) -> {output_dir}", file=sys.stderr)

    mod = types.ModuleType("antenv.axon_hooks")
    mod.get_axon_ntff_profile_hook = lambda: _hook
    mod.set_axon_ntff_profile_hook = lambda h: None
    import antenv

    antenv.axon_hooks = mod
    sys.modules["antenv.axon_hooks"] = mod
    return True


def run(inputs, debug=False, trace=False):
    if trace:
        trace = _install_ntff_shim()
    x = np.asarray(inputs["x"])
    edge_index = np.asarray(inputs["edge_index"])
    batch = np.asarray(inputs["batch"])
    meta, core_arrays, slots, g0s, counts = _prep(x, edge_index, batch)
    weights = {
        k: np.asarray(v)
        for k, v in inputs.items()
        if k not in ("x", "edge_index", "batch")
    }
    consts = _consts(weights, meta)
    nc = build_program(meta, debug=debug)
    maps = _in_maps(meta, core_arrays, slots, consts, x)

    hw = get_hw_module(nc.m)
    old = nc.m
    nc.m = hw
    try:
        res = bass_utils.run_bass_kernel_spmd(
            nc, maps, core_ids=list(range(meta["ncores"])), trace=trace
        )
    finally:
        nc.m = old

    # host unshard: assemble per-graph sums from per-core slot partials
    sums = np.zeros((G,), dtype=np.float64)
    for c in range(meta["ncores"]):
        part = np.asarray(res.results[c]["pool_out"], dtype=np.float64).reshape(256)
        g0 = g0s[c]
        hi = min(256, G - g0)
        sums[g0 : g0 + hi] += part[:hi]
    fcf_b = float(np.asarray(inputs["fcf_b"]).reshape(-1)[0])
    out = sums / np.maximum(counts, 1.0) + fcf_b
    return out.astype(np.float32).reshape(G, 1), res


def kernel(**inputs):
    out, _ = run(inputs)
    return out



# revision 59
# speedup vs baseline: 1.1480x; 1.0076x over previous
"""GAT (2-layer, PyG-style) + MLP + graph-LN + global mean pool on 8 Trainium2 cores.

Strategy (sharding_hint): nodes partitioned contiguously across the 8 cores;
edges partitioned by destination node (1-D graph partition, host-sorted by dst);
the per-layer node-feature table [h | a_src-logit] is AllGathered so each core
gathers h[src] rows for its local edges with indirect DMA; per-destination
softmax + weighted aggregation is done with one-hot scatter matmuls
accumulating in PSUM per 128-node block; graph-LayerNorm statistics and are
combined with a tiny AllReduce; the final global_mean_pool partial sums per
core are assembled on the host (unshard step).
"""

import os
import sys

sys.path.insert(0, "/opt/trn_rl_repo")

import math
from contextlib import ExitStack

QSPLIT = os.environ.get("QSPLIT", "0") == "1"  # alternate SWDGE queues for gathers

import numpy as np
import ml_dtypes

BF = ml_dtypes.bfloat16

import concourse.bass as bass
import concourse.bacc as bacc
import concourse.tile as tile
import concourse.mybir as mybir
from concourse import bass_utils
from concourse.bass import IndirectOffsetOnAxis
from concourse.bass_interp import get_hw_module

F32 = mybir.dt.float32
BF16 = mybir.dt.bfloat16
I32 = mybir.dt.int32
I16 = mybir.dt.int16
ALU = mybir.AluOpType
ACTF = mybir.ActivationFunctionType

# ---- problem constants (hardcoded per spec) ----
N = 100000
E_RAW = 1600000
D_IN = 9
HID = 16
HEADS = 8
D = 128
G = 1000
NCORES = 8
NEG_SLOPE = 0.2
LN_EPS = 1e-5
TBLW = 144  # table row width in bf16 (288B rows: h 128 | al_s f32-pairs)
WIN = 32  # edge tiles per gather window
P = 128

PAD_DREL = 300.0  # one-hot never matches (iota is 0..127)
PAD_SLOT = 300.0


# ======================================================================
# host-side prep: edge sort / partition / padding, index layouts, weights
# ======================================================================
def _prep(x, edge_index, batch, n=N, e_raw=E_RAW, ncores=NCORES, g=G):
    nl = n // ncores  # owned nodes per core
    npd = ((nl + P - 1) // P) * P
    nb = npd // P  # 128-node blocks per core

    # self-loops are handled by an on-chip diagonal fast path (h and al
    # are core-local), so only the raw edges go through the gather.
    src = edge_index[0].astype(np.int64)
    dst = edge_index[1].astype(np.int64)
    order = np.argsort(dst, kind="stable")
    srcs = src[order]
    dsts = dst[order]

    bounds = np.searchsorted(dsts, np.arange(ncores + 1) * nl)
    per_core = []
    cnt = np.zeros((ncores, nb), dtype=np.int64)
    for c in range(ncores):
        s_c = srcs[bounds[c] : bounds[c + 1]]
        d_c = dsts[bounds[c] : bounds[c + 1]] - c * nl
        per_core.append((s_c, d_c))
        cnt[c] = np.bincount(d_c // P, minlength=nb)
    maxe = cnt.max(axis=0)
    cap = ((maxe + P - 1) // P) * P  # edge slots per block
    cap = np.maximum(cap, P)
    ktiles = (cap // P).astype(np.int64)
    t_total = int(ktiles.sum())
    blk_starts = np.concatenate([[0], np.cumsum(ktiles)])
    tile2blk = []
    mcap = []  # valid gather rows per tile (max over cores, pads skipped)
    for b in range(nb):
        tile2blk += [b] * int(ktiles[b])
        for i in range(int(ktiles[b])):
            mcap.append(int(min(P, max(1, maxe[b] - i * P))))
    blk_first = {b: int(blk_starts[b]) for b in range(nb)}
    blk_last = {b: int(blk_starts[b + 1]) - 1 for b in range(nb)}

    core_arrays = []
    for c in range(ncores):
        s_c, d_c = per_core[c]
        gidx = np.zeros((t_total * P,), dtype=np.int32)
        drel = np.full((t_total * P,), PAD_DREL, dtype=np.float32)
        blk = d_c // P
        cstart = np.concatenate([[0], np.cumsum(np.bincount(blk, minlength=nb))])
        for b in range(nb):
            e0, e1 = cstart[b], cstart[b + 1]
            o0 = int(blk_starts[b]) * P
            m = e1 - e0
            sc = s_c[e0:e1]
            gidx[o0 : o0 + m] = ((sc // nl) * npd + (sc % nl)).astype(np.int32)
            drel[o0 : o0 + m] = (d_c[e0:e1] % P).astype(np.float32)
        core_arrays.append(
            (
                gidx.reshape(t_total, P).T.copy(),
                drel.reshape(t_total, P).T.copy(),
            )
        )

    # pool slots
    bsort = np.asarray(batch, dtype=np.int64)
    slots = []
    g0s = []
    counts = np.bincount(bsort, minlength=g).astype(np.float64)
    for c in range(ncores):
        bs = bsort[c * nl : (c + 1) * nl]
        g0 = int(bs[0])
        sl = np.full((npd,), PAD_SLOT, dtype=np.float32)
        sl[:nl] = (bs - g0).astype(np.float32)
        assert sl[:nl].max() < 256, "graph-slot overflow"
        slots.append(sl.reshape(nb, P).T.copy())
        g0s.append(g0)

    meta = dict(
        n=n, nl=nl, npd=npd, nb=nb, t=t_total, mcap=mcap,
        tile2blk=tile2blk, blk_first=blk_first, blk_last=blk_last,
        ncores=ncores, g=g,
    )
    return meta, core_arrays, slots, g0s, counts


def _blockdiag(a):  # a [H, C] -> [H*C, H]
    h, c = a.shape
    out = np.zeros((h * c, h), dtype=np.float32)
    for i in range(h):
        out[i * c : (i + 1) * c, i] = a[i]
    return out


def _headmap(heads, ch):  # [H, H*C] one-hot expansion map
    out = np.zeros((heads, heads * ch), dtype=np.float32)
    for i in range(heads):
        out[i, i * ch : (i + 1) * ch] = 1.0
    return out


def _consts(weights, meta):
    """Replicated (same every core) input arrays."""
    w = weights
    c = {}
    c["W1"] = w["W1"].astype(np.float32)  # [9, 128]
    c["a1blk"] = np.concatenate(
        [_blockdiag(w["a_src1"]), _blockdiag(w["a_dst1"])], axis=1
    ).astype(BF)  # [128, 16]
    c["b1c"] = w["b1"].reshape(D, 1).astype(np.float32)
    c["W2"] = w["W2"].astype(BF)  # [128,128] lhsT
    c["a2blk"] = np.concatenate(
        [w["a_src2"].reshape(D, 1), w["a_dst2"].reshape(D, 1)], axis=1
    ).astype(BF)  # [128, 2]
    c["b2c"] = w["b2"].reshape(D, 1).astype(np.float32)
    for i in (1, 2, 3, 4):
        c[f"fc{i}w"] = w[f"fc{i}_w"].astype(BF)
        c[f"fc{i}b"] = w[f"fc{i}_b"].reshape(D, 1).astype(np.float32)
    c["g1c"] = w["g1"].reshape(D, 1).astype(np.float32)
    c["be1c"] = w["beta1"].reshape(D, 1).astype(np.float32)
    c["g2c"] = w["g2"].reshape(D, 1).astype(np.float32)
    c["be2c"] = w["beta2"].reshape(D, 1).astype(np.float32)
    c["fcfw"] = w["fcf_w"].reshape(D, 1).astype(BF)
    # layer-2 attention on raw (pre-LN) h: zals = h_pre @ (g1*(W2@a_src2)),
    # als2 = rstd1*zals + ofs1@(W2@a_src2); same for dst
    a2s = w["a_src2"].reshape(D).astype(np.float32)
    a2d = w["a_dst2"].reshape(D).astype(np.float32)
    W2f = w["W2"].astype(np.float32)
    g1f = w["g1"].astype(np.float32)
    c["zvec"] = np.stack(
        [g1f * (W2f @ a2s), g1f * (W2f @ a2d)], axis=1
    ).astype(BF)  # [D, 2]
    c["w2asum"] = (W2f @ (a2s + a2d)).reshape(D, 1).astype(np.float32)
    c["iota256"] = np.tile(np.arange(256, dtype=np.float32), (P, 1)).astype(BF)
    c["iota128"] = np.tile(np.arange(P, dtype=np.float32), (P, 1)).astype(BF)
    c["eye128b"] = np.eye(P, dtype=np.float32).astype(BF)
    c["eye16f"] = np.eye(16, dtype=np.float32)
    c["hmap1"] = _headmap(HEADS, HID)  # [8, 128] f32
    c["ones1r"] = np.ones((1, P), dtype=np.float32)
    c["ones128c"] = np.ones((P, 1), dtype=np.float32)
    return c


# ======================================================================
# device program
# ======================================================================
def _chunks(total, width):
    out = []
    o = 0
    while o < total:
        w = min(width, total - o)
        out.append((o, w))
        o += w
    return out


def build_program(meta, debug=False):
    npd, nb, t = meta["npd"], meta["nb"], meta["t"]
    ncores = meta["ncores"]
    mcap = meta["mcap"]
    tile2blk = meta["tile2blk"]
    blk_first = meta["blk_first"]
    blk_last = meta["blk_last"]
    n_glob = meta["n"]
    nl = meta["nl"]
    rg = [list(range(ncores))]
    ch512 = _chunks(npd, 512)
    ln_cnt = float(n_glob * D)  # real elements for graph-LN stats

    nc = bacc.Bacc(
        "TRN2",
        target_bir_lowering=False,
        debug=False,
        enable_asserts=False,
        num_devices=ncores,
        num_swdge_queues=2 if QSPLIT else 1,
    )

    def inp(name, shape, dt):
        return nc.dram_tensor(name, shape, dt, kind="ExternalInput").ap()

    xT = inp("xT", [D_IN, npd], F32)
    gidx = inp("gidx", [P, t], I32)
    drel = inp("drel", [P, t], F32)
    slot = inp("slot", [P, nb], F32)
    iota128 = inp("iota128", [P, P], BF16)
    W1 = inp("W1", [D_IN, D], F32)
    a1blk = inp("a1blk", [D, 2 * HEADS], BF16)
    b1c = inp("b1c", [D, 1], F32)
    W2 = inp("W2", [D, D], BF16)
    a2blk = inp("a2blk", [D, 2], BF16)
    b2c = inp("b2c", [D, 1], F32)
    zvec = inp("zvec", [D, 2], BF16)
    w2asum = inp("w2asum", [D, 1], F32)
    fcw = {i: inp(f"fc{i}w", [D, D], BF16) for i in (1, 2, 3, 4)}
    fcb = {i: inp(f"fc{i}b", [D, 1], F32) for i in (1, 2, 3, 4)}
    g1c = inp("g1c", [D, 1], F32)
    be1c = inp("be1c", [D, 1], F32)
    g2c = inp("g2c", [D, 1], F32)
    be2c = inp("be2c", [D, 1], F32)
    fcfw = inp("fcfw", [D, 1], BF16)
    iota256 = inp("iota256", [P, 256], BF16)
    eye128b = inp("eye128b", [P, P], BF16)
    eye16f = inp("eye16f", [16, 16], F32)
    hmap1 = inp("hmap1", [HEADS, D], F32)
    ones1r = inp("ones1r", [1, P], F32)
    ones128c = inp("ones128c", [P, 1], F32)

    pool_out = nc.dram_tensor("pool_out", [256, 1], F32, kind="ExternalOutput").ap()
    dbg_out = None
    dbg1_out = None
    if debug:
        dbg_out = nc.dram_tensor("dbg_out", [D, npd], F32, kind="ExternalOutput").ap()
        dbg1_out = nc.dram_tensor("dbg1_out", [D, npd], F32, kind="ExternalOutput").ap()

    TW = TBLW  # 256 bf16 = 512B rows: h(128) | al_s f32-pairs | pad

    with tile.TileContext(nc) as tc, ExitStack() as top:
        dram = top.enter_context(tc.tile_pool(name="dram", bufs=1, space="DRAM"))
        persist = top.enter_context(tc.tile_pool(name="persist", bufs=1))
        cpool = top.enter_context(tc.tile_pool(name="consts", bufs=1))
        cbp = top.enter_context(tc.tile_pool(name="cbp", bufs=2, space="PSUM"))
        cbs = top.enter_context(tc.tile_pool(name="cbs", bufs=2))

        def scr():  # one-bank PSUM scratch, callers slice columns
            return cbp.tile([P, 512], F32, tag="scr", name="scr")

        tbl1_loc = dram.tile([npd, TW], BF16, tag="tbl1_loc")
        tbl1_full = dram.tile([ncores * npd, TW], BF16, tag="tbl1_full", addr_space="Shared")
        tbl2_loc = dram.tile([npd, TW], BF16, tag="tbl2_loc")
        tbl2_full = dram.tile([ncores * npd, TW], BF16, tag="tbl2_full", addr_space="Shared")

        ln_in = dram.tile([1, 2], F32, tag="ln_in")
        ln_out = dram.tile([1, 2], F32, tag="ln_out", addr_space="Shared")
        ln_in2 = dram.tile([1, 2], F32, tag="ln_in2")
        ln_out2 = dram.tile([1, 2], F32, tag="ln_out2", addr_space="Shared")

        # persistent activations (transposed [feat, node]) and edge-index tables
        hc_sb = persist.tile([P, npd], BF16, tag="hc_sb")  # residual
        hf_sb = persist.tile([P, npd], BF16, tag="hf_sb")  # working activation
        drel_sb = persist.tile([P, t], F32, tag="drel_sb")
        gidx_sb = persist.tile([P, t], I32, tag="gidx_sb")
        ald1_sb = persist.tile([P, nb * HEADS], BF16, tag="ald1_sb")
        ald2_sb = persist.tile([P, nb], BF16, tag="ald2_sb")
        als1_sb = persist.tile([P, nb * HEADS], BF16, tag="als1_sb")
        als2_sb = persist.tile([P, nb], BF16, tag="als2_sb")
        hnm_sb = persist.tile([P, npd], BF16, tag="hnm_sb")  # node-major h
        # LN1-derived affine terms (filled post-E1, consumed in E2)
        sc1_sb = persist.tile([P, 1], F32, tag="sc1_sb")
        ofs1_sb = persist.tile([P, 1], F32, tag="ofs1_sb")
        rstd1_sb = persist.tile([P, 1], F32, tag="rstd1_sb")
        cc_sb = persist.tile([P, 1], F32, tag="cc_sb")
        c2_sb = persist.tile([P, 1], F32, tag="c2_sb")
        ofsrow_sb = persist.tile([1, P], F32, tag="ofsrow_sb")

        # constants in SBUF
        def cload(ap_in, shape, dt, tag):
            s = cpool.tile(shape, dt, tag=tag)
            nc.sync.dma_start(out=s[:], in_=ap_in)
            return s

        W1_s = cload(W1, [D_IN, D], F32, "W1")
        a1_s = cload(a1blk, [D, 2 * HEADS], BF16, "a1")
        a2_s = cload(a2blk, [D, 2], BF16, "a2")
        b1_s = cload(b1c, [D, 1], F32, "b1")
        b2_s = cload(b2c, [D, 1], F32, "b2")
        zvec_s = cload(zvec, [D, 2], BF16, "zvec")
        w2asum_s = cload(w2asum, [D, 1], F32, "w2asum")
        fcw_s = {i: cload(fcw[i], [D, D], BF16, f"fw{i}") for i in (1, 2, 3, 4)}
        fcb_s = {i: cload(fcb[i], [D, 1], F32, f"fb{i}") for i in (1, 2, 3, 4)}
        g1_s = cload(g1c, [D, 1], F32, "g1")
        be1_s = cload(be1c, [D, 1], F32, "be1")
        g2_s = cload(g2c, [D, 1], F32, "g2")
        be2_s = cload(be2c, [D, 1], F32, "be2")
        W2_s = cload(W2, [D, D], BF16, "W2")
        fcf_s = cload(fcfw, [D, 1], BF16, "fcf")
        io256_s = cload(iota256, [P, 256], BF16, "io256")
        io128_s = cload(iota128, [P, P], BF16, "io128")
        eye_s = cload(eye128b, [P, P], BF16, "eye")
        eye16_s = cload(eye16f, [16, 16], F32, "eye16")
        hmap1_s = cload(hmap1, [HEADS, D], F32, "hmap1")
        ones1r_s = cload(ones1r, [1, P], F32, "ones1r")
        ones128c_s = cload(ones128c, [P, 1], F32, "ones128c")
        slot_s = cload(slot, [P, nb], F32, "slot")

        nc.sync.dma_start(out=drel_sb[:], in_=drel)
        nc.sync.dma_start(out=gidx_sb[:], in_=gidx)

        # ---------------- stage A for a GAT layer: build tables ----------------
        def stage_a(layer, src_hT_fn):
            """Write tbl{layer}_loc rows [h bf16 | al_s f32] and ald table; then AllGather.
            src_hT_fn(c0, w, ps): fills psum tile [128, w] with this layer's hT chunk."""
            heads = HEADS if layer == 1 else 1
            a_s = a1_s if layer == 1 else a2_s
            tbl_loc = tbl1_loc if layer == 1 else tbl2_loc
            tbl_full = tbl1_full if layer == 1 else tbl2_full
            ald_sb = ald1_sb if layer == 1 else ald2_sb
            als_sb = als1_sb if layer == 1 else als2_sb
            with (
                tc.tile_pool(name=f"sa{layer}", bufs=5) as sp,
                tc.tile_pool(name=f"sap{layer}", bufs=2, space="PSUM") as pp,
                tc.tile_pool(name=f"sat{layer}", bufs=2, space="PSUM") as tp2,
            ):
                for c0, w in ch512:
                    ps = pp.tile([P, 512], F32, tag="hps")
                    src_hT_fn(c0, w, ps)
                    # keep transposed activation for downstream dense chain
                    nc.vector.tensor_copy(out=hf_sb[:, c0 : c0 + w], in_=ps[:, :w])
                for k in range(nb):
                    c0 = k * P
                    al_t = tp2.tile([P, 256], F32, tag="al_t", name="al_t")
                    # al_s/al_d for this chunk: [2*heads, 128] = a^T @ hT
                    alps = al_t[0 : 2 * heads, 0:P]
                    nc.tensor.matmul(
                        out=alps, lhsT=a_s[:], rhs=hf_sb[:, c0 : c0 + P],
                        start=True, stop=True,
                    )
                    asb = sp.tile([2 * heads, P], F32, tag="asb")
                    nc.vector.tensor_copy(out=asb[:], in_=alps)
                    # transpose -> [128, 2*heads]
                    atp = al_t[:, P : P + 2 * heads]
                    nc.tensor.transpose(
                        out=atp, in_=asb[:], identity=eye16_s[: 2 * heads, : 2 * heads]
                    )
                    ats = sp.tile([P, 2 * heads], F32, tag="ats")
                    nc.vector.tensor_copy(out=ats[:], in_=atp)
                    # transpose h chunk -> node-major [128n, 128f]
                    ht_t = tp2.tile([P, P], BF16, tag="ht_t", name="ht_t")
                    htp = ht_t[:]
                    nc.tensor.transpose(
                        out=htp, in_=hf_sb[:, c0 : c0 + P], identity=eye_s[:]
                    )
                    nc.vector.tensor_copy(
                        out=hnm_sb[:, c0 : c0 + P], in_=htp
                    )
                    # table writes
                    nc.sync.dma_start(
                        out=tbl_loc[c0 : c0 + P, 0:D], in_=hnm_sb[:, c0 : c0 + P]
                    )
                    nc.sync.dma_start(
                        out=tbl_loc[c0 : c0 + P, D : D + 2 * heads],
                        in_=ats[:].bitcast(BF16)[:, 0 : 2 * heads],
                    )
                    nc.vector.tensor_copy(
                        out=ald_sb[:, k * heads : (k + 1) * heads],
                        in_=ats[:, heads : 2 * heads],
                    )
                    nc.vector.tensor_copy(
                        out=als_sb[:, k * heads : (k + 1) * heads],
                        in_=ats[:, 0:heads],
                    )
            nc.gpsimd.collective_compute(
                "AllGather",
                ALU.bypass,
                replica_groups=rg,
                ins=[tbl_loc[:].opt()],
                outs=[tbl_full[:].opt()],
            )

        # ---------------- edge phase for a GAT layer ----------------
        def edge_phase(layer, out_cb):
            """GAT aggregation; out_cb(b, osb) gets [128f, 128n] f32 sbuf tile.

            Per tile: one indirect row-gather (h|al_s), one-hot S on DVE,
            St = transpose(S) on PE feeds the al_d expansion matmul; agg/den
            matmuls accumulate into quad-packed PSUM banks (4 blocks/bank).
            """
            heads = HEADS if layer == 1 else 1
            tbl_full = tbl1_full if layer == 1 else tbl2_full
            ald_sb = ald1_sb if layer == 1 else ald2_sb
            als_sb = als1_sb if layer == 1 else als2_sb
            nwin = (t + WIN - 1) // WIN
            agg_banks = {}
            den_banks = {}
            with (
                tc.tile_pool(name=f"eg{layer}", bufs=4) as gp,
                tc.tile_pool(name=f"eb{layer}", bufs=2) as bigp,
                tc.tile_pool(name=f"es{layer}", bufs=3) as sp,
                tc.tile_pool(name=f"est{layer}", bufs=WIN + 4) as stpool,
                tc.tile_pool(name=f"ef{layer}", bufs=2) as fin,
                tc.tile_pool(name=f"ep{layer}", bufs=2, space="PSUM") as pp,
                tc.tile_pool(name=f"ed{layer}", bufs=1, space="PSUM") as dp,
                tc.tile_pool(name=f"ea{layer}", bufs=1, space="PSUM") as ap_,
                tc.tile_pool(name=f"et{layer}", bufs=1, space="PSUM") as tp_,
                tc.tile_pool(name=f"er{layer}", bufs=1, space="PSUM") as rp,
            ):
                # first-touch memset so never-gathered pad partitions stay finite
                for _ in range(4):
                    gm = gp.tile([P, WIN * TW], BF16, tag="gath")
                    nc.gpsimd.memset(gm[:], 0.0)
                for wi in range(nwin):
                    t0 = wi * WIN
                    L = min(WIN, t - t0)
                    gath = gp.tile([P, WIN * TW], BF16, tag="gath")
                    for i in range(L):
                        m = mcap[t0 + i]
                        bi = nc.gpsimd.indirect_dma_start(
                            out=gath[0:m, i * TW : (i + 1) * TW],
                            out_offset=None,
                            in_=tbl_full[:],
                            in_offset=IndirectOffsetOnAxis(
                                ap=gidx_sb[0:m, t0 + i : t0 + i + 1], axis=0
                            ),
                        )
                        if QSPLIT and (t0 + i) % 2 == 1:
                            bi.ins.queue = "qPoolDynamic1"
                    aldw = ap_.tile([P, WIN * heads], F32, tag="aldw")
                    st_list = []
                    for i in range(L):
                        ti = t0 + i
                        s_t = stpool.tile([P, P], BF16, tag="s_t")
                        nc.vector.tensor_scalar(
                            out=s_t[:], in0=io128_s[:],
                            scalar1=drel_sb[:, ti : ti + 1], scalar2=None,
                            op0=ALU.is_equal,
                        )
                        st_list.append(s_t)
                    # batched St: 4 transposes share one PSUM bank, one ACT copy
                    for g0 in range(0, L, 4):
                        gl = min(4, L - g0)
                        stp = tp_.tile([P, 4 * P], BF16, tag="stp")
                        for j in range(gl):
                            nc.tensor.matmul(
                                out=stp[:, j * P : (j + 1) * P],
                                lhsT=st_list[g0 + j][:],
                                rhs=eye_s[:],
                                is_transpose=True,
                                start=(j == 0),
                                stop=(j == gl - 1),
                                skip_group_check=True,
                            )
                        sts = sp.tile([P, 4 * P], BF16, tag="sts")
                        nc.scalar.activation(
                            out=sts[:, 0 : gl * P], in_=stp[:, 0 : gl * P],
                            func=ACTF.Copy,
                        )
                        for j in range(gl):
                            i = g0 + j
                            b = tile2blk[t0 + i]
                            nc.tensor.matmul(
                                out=aldw[:, i * heads : (i + 1) * heads],
                                lhsT=sts[:, j * P : (j + 1) * P],
                                rhs=ald_sb[:, b * heads : (b + 1) * heads],
                                start=(i == 0),
                                stop=(i == L - 1),
                                skip_group_check=True,
                            )
                    # e = al_s[src] + al_d[dst]
                    als_v = (
                        gath[:]
                        .bitcast(F32)
                        .rearrange("p (t w) -> p t w", w=TW // 2)[
                            :, 0:L, D // 2 : D // 2 + heads
                        ]
                    )
                    e_sb = sp.tile([P, WIN * heads], F32, tag="e_sb")
                    nc.vector.tensor_tensor(
                        out=e_sb[:, 0 : L * heads].rearrange(
                            "p (t h) -> p t h", h=heads
                        ),
                        in0=als_v,
                        in1=aldw[:, 0 : L * heads].rearrange(
                            "p (t h) -> p t h", h=heads
                        ),
                        op=ALU.add,
                    )
                    if layer == 2:
                        # raw z-logits -> true logits: e = rstd1*e + CC
                        nc.vector.tensor_scalar(
                            out=e_sb[:, 0 : L * heads],
                            in0=e_sb[:, 0 : L * heads],
                            scalar1=rstd1_sb[:], scalar2=cc_sb[:],
                            op0=ALU.mult, op1=ALU.add,
                        )
                    t02 = sp.tile([P, WIN * heads], F32, tag="t02")
                    nc.vector.tensor_scalar(
                        out=t02[:, 0 : L * heads], in0=e_sb[:, 0 : L * heads],
                        scalar1=NEG_SLOPE, scalar2=None, op0=ALU.mult,
                    )
                    lr_sb = sp.tile([P, WIN * heads], F32, tag="lr_sb")
                    nc.vector.tensor_tensor(
                        out=lr_sb[:, 0 : L * heads], in0=e_sb[:, 0 : L * heads],
                        in1=t02[:, 0 : L * heads], op=ALU.max,
                    )
                    pexp = bigp.tile([P, WIN * D], BF16, tag="pexp")
                    nc.scalar.activation(
                        out=pexp[:, 0 : L * D].rearrange(
                            "p (t h c) -> p t h c", h=heads, c=D // heads
                        ),
                        in_=lr_sb[:, 0 : L * heads]
                        .rearrange("p (t h) -> p t h", h=heads)
                        .broadcast_to((P, L, heads, D // heads)),
                        func=ACTF.Exp,
                    )
                    p_sb = sp.tile([P, WIN * heads], BF16, tag="p_sb")
                    nc.scalar.activation(
                        out=p_sb[:, 0 : L * heads], in_=lr_sb[:, 0 : L * heads],
                        func=ACTF.Exp,
                    )
                    wh = bigp.tile([P, WIN * D], BF16, tag="wh")
                    nc.vector.tensor_tensor(
                        out=wh[:, 0 : L * D].rearrange("p (t c) -> p t c", c=D),
                        in0=gath[:].rearrange("p (t w) -> p t w", w=TW)[:, 0:L, 0:D],
                        in1=pexp[:, 0 : L * D].rearrange("p (t c) -> p t c", c=D),
                        op=ALU.mult,
                    )
                    for i in range(L):
                        ti = t0 + i
                        b = tile2blk[ti]
                        q = b // 4  # quad id
                        qi = b % 4
                        if q not in agg_banks:
                            qblocks = [bb for bb in range(4 * q, min(4 * q + 4, nb))]
                            ab = pp.tile([P, 512], F32, tag="aggq", name="aggq")
                            db = dp.tile([8, 512], F32, tag="denq", name="denq")
                            agg_banks[q] = (
                                ab, blk_first[qblocks[0]], blk_last[qblocks[-1]]
                            )
                            den_banks[q] = (
                                db, blk_first[qblocks[0]], blk_last[qblocks[-1]]
                            )
                        ab, qf, ql = agg_banks[q]
                        db, _, _ = den_banks[q]
                        nc.tensor.matmul(
                            out=ab[:, qi * P : (qi + 1) * P],
                            lhsT=wh[:, i * D : (i + 1) * D],
                            rhs=st_list[i][:],
                            start=(ti == qf),
                            stop=False,
                            skip_group_check=True,
                        )
                        nc.tensor.matmul(
                            out=db[0:heads, qi * P : (qi + 1) * P],
                            lhsT=p_sb[:, i * heads : (i + 1) * heads],
                            rhs=st_list[i][:],
                            start=(ti == qf),
                            stop=False,
                            skip_group_check=True,
                        )
                        if ti == blk_last[b]:
                            # diagonal (self-loop) contribution: h and al are
                            # local, injected as one pseudo-tile (S = identity)
                            esf = fin.tile([P, HEADS], F32, tag="esf")
                            nc.vector.tensor_tensor(
                                out=esf[:, 0:heads],
                                in0=als_sb[:, b * heads : (b + 1) * heads],
                                in1=ald_sb[:, b * heads : (b + 1) * heads],
                                op=ALU.add,
                            )
                            if layer == 2:
                                nc.vector.tensor_scalar(
                                    out=esf[:, 0:heads], in0=esf[:, 0:heads],
                                    scalar1=rstd1_sb[:], scalar2=cc_sb[:],
                                    op0=ALU.mult, op1=ALU.add,
                                )
                            tsf = fin.tile([P, HEADS], F32, tag="tsf")
                            nc.vector.tensor_scalar(
                                out=tsf[:, 0:heads], in0=esf[:, 0:heads],
                                scalar1=NEG_SLOPE, scalar2=None, op0=ALU.mult,
                            )
                            lrs = fin.tile([P, HEADS], F32, tag="lrs")
                            nc.vector.tensor_tensor(
                                out=lrs[:, 0:heads], in0=esf[:, 0:heads],
                                in1=tsf[:, 0:heads], op=ALU.max,
                            )
                            pxs = fin.tile([P, D], BF16, tag="pxs")
                            nc.scalar.activation(
                                out=pxs[:].rearrange("p (h c) -> p h c", h=heads),
                                in_=lrs[:, 0:heads].broadcast_to(
                                    (P, heads, D // heads)
                                ),
                                func=ACTF.Exp,
                            )
                            psbs = fin.tile([P, HEADS], BF16, tag="psbs")
                            nc.scalar.activation(
                                out=psbs[:, 0:heads], in_=lrs[:, 0:heads],
                                func=ACTF.Exp,
                            )
                            whs = fin.tile([P, D], BF16, tag="whs")
                            nc.vector.tensor_tensor(
                                out=whs[:], in0=hnm_sb[:, b * P : (b + 1) * P],
                                in1=pxs[:], op=ALU.mult,
                            )
                            nc.tensor.matmul(
                                out=ab[:, qi * P : (qi + 1) * P],
                                lhsT=whs[:], rhs=eye_s[:],
                                start=False, stop=(ti == ql),
                                skip_group_check=True,
                            )
                            nc.tensor.matmul(
                                out=db[0:heads, qi * P : (qi + 1) * P],
                                lhsT=psbs[:, 0:heads], rhs=eye_s[:],
                                start=False, stop=(ti == ql),
                                skip_group_check=True,
                            )
                            dsb = fin.tile([heads, P], F32, tag="dsb")
                            nc.vector.tensor_scalar(
                                out=dsb[:], in0=db[0:heads, qi * P : (qi + 1) * P],
                                scalar1=1e-16, scalar2=None, op0=ALU.add,
                            )
                            rsb = fin.tile([heads, P], F32, tag="rsb")
                            nc.vector.reciprocal(out=rsb[:], in_=dsb[:])
                            rp_t = rp.tile([P, 512], F32, tag="rscr")
                            rex = rp_t[:, 0:P]
                            nc.tensor.matmul(
                                out=rex,
                                lhsT=(ones1r_s if heads == 1 else hmap1_s)[:],
                                rhs=rsb[:], start=True, stop=True,
                            )
                            rxs = fin.tile([P, P], F32, tag="rxs")
                            nc.scalar.activation(out=rxs[:], in_=rex, func=ACTF.Copy)
                            osb = fin.tile([P, P], F32, tag="osb")
                            if layer == 2:
                                # out2 = [sc1*ab + ofs1*den] @ W2 / den
                                tmul = fin.tile([P, P], F32, tag="tmul")
                                nc.vector.tensor_scalar(
                                    out=tmul[:],
                                    in0=ab[:, qi * P : (qi + 1) * P],
                                    scalar1=sc1_sb[:], scalar2=None,
                                    op0=ALU.mult,
                                )
                                t2l = fin.tile([P, P], BF16, tag="t2l")
                                nc.vector.tensor_copy(out=t2l[:], in_=tmul[:])
                                ps2 = rp_t[:, 2 * P : 3 * P]
                                nc.tensor.matmul(
                                    out=ps2, lhsT=W2_s[:], rhs=t2l[:],
                                    start=True, stop=True,
                                )
                                nc.vector.tensor_tensor(
                                    out=osb[:], in0=ps2, in1=rxs[:],
                                    op=ALU.mult,
                                )
                            else:
                                nc.vector.tensor_tensor(
                                    out=osb[:], in0=ab[:, qi * P : (qi + 1) * P],
                                    in1=rxs[:], op=ALU.mult,
                                )
                            out_cb(b, osb)
                            if b == 4 * q + 3 or b == nb - 1:
                                del agg_banks[q], den_banks[q]

        # ---------------- graph-LN over h_sb -> dst_sb (bf16) ----------------
        def graph_ln(src_sb, g_s, be_s, ln_i, ln_o, dst_sb, stash=False):
            with (
                tc.tile_pool(name="ln", bufs=2) as sp,
                tc.tile_pool(name="lnp", bufs=2, space="PSUM") as pp,
            ):
                nchunk = len(ch512)
                stats = sp.tile([P, nchunk * 6], F32, tag="stats")
                for ci, (c0, w) in enumerate(ch512):
                    nc.vector.bn_stats(
                        out=stats[:, ci * 6 : (ci + 1) * 6], in_=src_sb[:, c0 : c0 + w]
                    )
                mv = sp.tile([P, 2], F32, tag="mv")
                nc.vector.bn_aggr(
                    out=mv[:], in_=stats[:].rearrange("p (c s) -> p c s", s=6)
                )
                # per-partition sums: [sx, sxx] = [m, (v+m^2)] * npd
                sums = sp.tile([P, 2], F32, tag="sums")
                nc.vector.tensor_scalar(
                    out=sums[:, 0:1], in0=mv[:, 0:1], scalar1=float(npd),
                    scalar2=None, op0=ALU.mult,
                )
                m2 = sp.tile([P, 1], F32, tag="m2")
                nc.vector.tensor_tensor(
                    out=m2[:], in0=mv[:, 0:1], in1=mv[:, 0:1], op=ALU.mult
                )
                nc.vector.tensor_tensor(
                    out=sums[:, 1:2], in0=mv[:, 1:2], in1=m2[:], op=ALU.add
                )
                nc.vector.tensor_scalar(
                    out=sums[:, 1:2], in0=sums[:, 1:2], scalar1=float(npd),
                    scalar2=None, op0=ALU.mult,
                )
                red = pp.tile([1, 2], F32, tag="red")
                nc.tensor.matmul(
                    out=red[:], lhsT=ones128c_s[:], rhs=sums[:], start=True, stop=True
                )
                rsb = sp.tile([1, 2], F32, tag="rsb2")
                nc.vector.tensor_copy(out=rsb[:], in_=red[:])
                nc.gpsimd.dma_start(out=ln_i[:], in_=rsb[:])
                nc.gpsimd.collective_compute(
                    "AllReduce", ALU.add, replica_groups=rg,
                    ins=[ln_i[:].opt()], outs=[ln_o[:].opt()],
                )
                ar = sp.tile([1, 2], F32, tag="ar")
                nc.gpsimd.dma_start(out=ar[:], in_=ln_o[:])
                bc = pp.tile([P, 2], F32, tag="bc")
                nc.tensor.matmul(
                    out=bc[:], lhsT=ones1r_s[:], rhs=ar[:], start=True, stop=True
                )
                # mu = s1/cnt ; var = s2/cnt - mu^2 ; s = g * rsqrt(var+eps) ; b = be - mu*s
                mu = sp.tile([P, 1], F32, tag="mu")
                nc.vector.tensor_scalar(
                    out=mu[:], in0=bc[:, 0:1], scalar1=1.0 / ln_cnt, scalar2=None,
                    op0=ALU.mult,
                )
                var = sp.tile([P, 1], F32, tag="var")
                nc.vector.tensor_scalar(
                    out=var[:], in0=bc[:, 1:2], scalar1=1.0 / ln_cnt, scalar2=None,
                    op0=ALU.mult,
                )
                mu2 = sp.tile([P, 1], F32, tag="mu2")
                nc.vector.tensor_tensor(out=mu2[:], in0=mu[:], in1=mu[:], op=ALU.mult)
                nc.vector.tensor_tensor(
                    out=var[:], in0=var[:], in1=mu2[:], op=ALU.subtract
                )
                nc.vector.tensor_scalar(
                    out=var[:], in0=var[:], scalar1=LN_EPS, scalar2=None, op0=ALU.add
                )
                sd = sp.tile([P, 1], F32, tag="sd")
                nc.scalar.activation(out=sd[:], in_=var[:], func=ACTF.Sqrt)
                rstd = sp.tile([P, 1], F32, tag="rstd")
                nc.vector.reciprocal(out=rstd[:], in_=sd[:])
                sc = sp.tile([P, 1], F32, tag="sc")
                nc.vector.tensor_tensor(out=sc[:], in0=g_s[:], in1=rstd[:], op=ALU.mult)
                ofs = sp.tile([P, 1], F32, tag="ofs")
                nc.vector.tensor_tensor(out=ofs[:], in0=mu[:], in1=sc[:], op=ALU.mult)
                nc.vector.tensor_tensor(
                    out=ofs[:], in0=be_s[:], in1=ofs[:], op=ALU.subtract
                )
                if stash:
                    nc.vector.tensor_copy(out=rstd1_sb[:], in_=rstd[:])
                    nc.vector.tensor_copy(out=sc1_sb[:], in_=sc[:])
                    nc.vector.tensor_copy(out=ofs1_sb[:], in_=ofs[:])
                for c0, w in ch512:
                    nc.scalar.activation(
                        out=dst_sb[:, c0 : c0 + w], in_=src_sb[:, c0 : c0 + w],
                        func=ACTF.Identity, bias=ofs[:], scale=sc[:],
                    )
                # zero the padded node columns so they don't pollute later stats
                if npd > nl:
                    nc.gpsimd.memset(dst_sb[:, nl:npd], 0.0)

        # =========================== the network ===========================
        # ---- GAT layer 1 stage A: h1 = x @ W1 (transposed) ----
        xin_pool = top.enter_context(tc.tile_pool(name="xin", bufs=2))

        def h1_fn(c0, w, ps):
            xt = xin_pool.tile([D_IN, 512], F32, tag="xt")
            nc.sync.dma_start(out=xt[:, :w], in_=xT[:, c0 : c0 + w])
            nc.tensor.matmul(
                out=ps[:, :w], lhsT=W1_s[:], rhs=xt[:, :w], start=True, stop=True
            )

        stage_a(1, h1_fn)

        # ---- GAT layer 1 edge phase; per-block callback also runs fc1/fc2
        # and builds the raw (pre-LN) layer-2 table [h_pre | zals], so AG2
        # can start immediately when E1 drains and LN1 runs under it. ----
        def out1_cb(b, osb):
            c0 = b * P
            h1o = hc_sb[:, c0 : c0 + P]
            nc.scalar.activation(
                out=h1o, in_=osb[:], func=ACTF.Relu, bias=b1_s[:], scale=1.0,
            )
            s_ = scr()
            ps1 = s_[:, 0:P]
            nc.tensor.matmul(
                out=ps1, lhsT=fcw_s[1][:], rhs=h1o, start=True, stop=True
            )
            t1 = cbs.tile([P, P], BF16, tag="t1")
            nc.scalar.activation(
                out=t1[:], in_=ps1, func=ACTF.Relu, bias=fcb_s[1][:], scale=1.0
            )
            psf = s_[:, P : 2 * P]
            nc.tensor.matmul(
                out=psf, lhsT=fcw_s[2][:], rhs=t1[:], start=True, stop=True
            )
            t2f = cbs.tile([P, P], F32, tag="t2f")
            nc.vector.tensor_tensor(out=t2f[:], in0=psf, in1=h1o, op=ALU.add)
            nc.scalar.activation(
                out=hf_sb[:, c0 : c0 + P], in_=t2f[:], func=ACTF.Relu,
                bias=fcb_s[2][:], scale=1.0,
            )
            # raw layer-2 attention z-values
            zz = s_[0:2, 2 * P : 3 * P]
            nc.tensor.matmul(
                out=zz, lhsT=zvec_s[:], rhs=hf_sb[:, c0 : c0 + P],
                start=True, stop=True,
            )
            zzs = cbs.tile([2, P], F32, tag="zzs")
            nc.vector.tensor_copy(out=zzs[:], in_=zz)
            atp2 = s_[:, 3 * P : 3 * P + 2]
            nc.tensor.transpose(out=atp2, in_=zzs[:], identity=eye16_s[0:2, 0:2])
            ats2 = cbs.tile([P, 2], F32, tag="ats2")
            nc.vector.tensor_copy(out=ats2[:], in_=atp2)
            nc.vector.tensor_copy(out=ald2_sb[:, b : b + 1], in_=ats2[:, 1:2])
            nc.vector.tensor_copy(out=als2_sb[:, b : b + 1], in_=ats2[:, 0:1])
            htp2 = s_[:].bitcast(BF16)[:, 772:900]
            nc.tensor.transpose(
                out=htp2, in_=hf_sb[:, c0 : c0 + P], identity=eye_s[:]
            )
            nc.vector.tensor_copy(out=hnm_sb[:, c0 : c0 + P], in_=htp2)
            nc.sync.dma_start(
                out=tbl2_loc[c0 : c0 + P, 0:D], in_=hnm_sb[:, c0 : c0 + P]
            )
            nc.sync.dma_start(
                out=tbl2_loc[c0 : c0 + P, D : D + 2],
                in_=ats2[:].bitcast(BF16)[:, 0:2],
            )

        edge_phase(1, out1_cb)
        nc.gpsimd.collective_compute(
            "AllGather",
            ALU.bypass,
            replica_groups=rg,
            ins=[tbl2_loc[:].opt()],
            outs=[tbl2_full[:].opt()],
        )
        if debug:
            with tc.tile_pool(name="dbg1p", bufs=2) as dbp1:
                for c0, w in ch512:
                    dt1 = dbp1.tile([P, 512], F32, tag="dbg1")
                    nc.vector.tensor_copy(out=dt1[:, :w], in_=hc_sb[:, c0 : c0 + w])
                    nc.sync.dma_start(out=dbg1_out[:, c0 : c0 + w], in_=dt1[:, :w])

        # ---- LN1 (stats + AR + affine; runs under AG2), then the E2
        # affine constants derived from (rstd1, ofs1) ----
        graph_ln(hf_sb, g1_s, be1_s, ln_in, ln_out, hc_sb, stash=True)
        with (
            tc.tile_pool(name="epl", bufs=1) as epp,
            tc.tile_pool(name="eplp", bufs=1, space="PSUM") as epps,
        ):
            m1 = epps.tile([1, 1], F32, tag="m1")
            nc.tensor.matmul(
                out=m1[:], lhsT=ofs1_sb[:], rhs=w2asum_s[:], start=True, stop=True
            )
            m1s = epp.tile([1, 1], F32, tag="m1s")
            nc.vector.tensor_copy(out=m1s[:], in_=m1[:])
            ccp = epps.tile([P, 1], F32, tag="ccp")
            nc.tensor.matmul(
                out=ccp[:], lhsT=ones1r_s[:], rhs=m1s[:], start=True, stop=True
            )
            nc.vector.tensor_copy(out=cc_sb[:], in_=ccp[:])
            ofsb = epp.tile([P, 1], BF16, tag="ofsb")
            nc.vector.tensor_copy(out=ofsb[:], in_=ofs1_sb[:])
            c2p = epps.tile([P, 1], F32, tag="c2p")
            nc.tensor.matmul(
                out=c2p[:], lhsT=W2_s[:], rhs=ofsb[:], start=True, stop=True
            )
            c2t = epp.tile([P, 1], F32, tag="c2t")
            nc.vector.tensor_copy(out=c2t[:], in_=c2p[:])
            nc.vector.tensor_tensor(
                out=c2_sb[:], in0=c2t[:], in1=b2_s[:], op=ALU.add
            )
            orp = epps.tile([1, P], F32, tag="orp")
            nc.tensor.matmul(
                out=orp[:], lhsT=ofsb[:], rhs=eye_s[:], start=True, stop=True
            )
            nc.vector.tensor_copy(out=ofsrow_sb[:], in_=orp[:])

        # ---- GAT layer 2 edge phase (raw table; LN+W2 folded into the
        # finalize) -> hf_sb = out2 + (W2^T ofs1 + b2) ----
        def out2_cb(b, osb):
            nc.scalar.activation(
                out=hf_sb[:, b * P : (b + 1) * P], in_=osb[:], func=ACTF.Identity,
                bias=c2_sb[:], scale=1.0,
            )

        edge_phase(2, out2_cb)

        # ---- fc3 (relu), fc4 (+residual h_ln1), LN2, fcf, pool ----
        with (
            tc.tile_pool(name="fc2", bufs=3) as fp,
            tc.tile_pool(name="fcp2", bufs=2, space="PSUM") as fpp,
        ):
            for c0, w in ch512:
                ps = fpp.tile([P, 512], F32, tag="fc3p")
                nc.tensor.matmul(
                    out=ps[:, :w], lhsT=fcw_s[3][:], rhs=hf_sb[:, c0 : c0 + w],
                    start=True, stop=True,
                )
                nc.scalar.activation(
                    out=hf_sb[:, c0 : c0 + w], in_=ps[:, :w], func=ACTF.Relu,
                    bias=fcb_s[3][:], scale=1.0,
                )
            for c0, w in ch512:
                ps = fpp.tile([P, 512], F32, tag="fc4p")
                nc.tensor.matmul(
                    out=ps[:, :w], lhsT=fcw_s[4][:], rhs=hf_sb[:, c0 : c0 + w],
                    start=True, stop=True,
                )
                tmp = fp.tile([P, 512], F32, tag="fc4t")
                nc.vector.tensor_tensor(
                    out=tmp[:, :w], in0=ps[:, :w], in1=hc_sb[:, c0 : c0 + w], op=ALU.add
                )
                nc.scalar.activation(
                    out=hf_sb[:, c0 : c0 + w], in_=tmp[:, :w], func=ACTF.Identity,
                    bias=fcb_s[4][:], scale=1.0,
                )
        graph_ln(hf_sb, g2_s, be2_s, ln_in2, ln_out2, hc_sb)  # hc_sb = h_ln2

        if debug:
            with tc.tile_pool(name="dbgp", bufs=2) as dbp:
                for c0, w in ch512:
                    dt_ = dbp.tile([P, 512], F32, tag="dbg")
                    nc.vector.tensor_copy(out=dt_[:, :w], in_=hc_sb[:, c0 : c0 + w])
                    nc.sync.dma_start(out=dbg_out[:, c0 : c0 + w], in_=dt_[:, :w])

        # fcf + per-core pooled slot sums
        with (
            tc.tile_pool(name="pl", bufs=6) as sp,
            tc.tile_pool(name="plp", bufs=4, space="PSUM") as pp,
            tc.tile_pool(name="plq", bufs=1, space="PSUM") as qq,
        ):
            pool_a = qq.tile([P, 1], F32, tag="pool_a")
            pool_b = qq.tile([P, 1], F32, tag="pool_b")
            for k in range(nb):
                c0 = k * P
                hv = pp.tile([P, 1], F32, tag="hv")
                nc.tensor.matmul(
                    out=hv[:], lhsT=hc_sb[:, c0 : c0 + P], rhs=fcf_s[:],
                    start=True, stop=True,
                )
                hvs = sp.tile([P, 1], BF16, tag="hvs")
                nc.vector.tensor_copy(out=hvs[:], in_=hv[:])
                oh = sp.tile([P, 256], BF16, tag="oh")
                nc.vector.tensor_scalar(
                    out=oh[:], in0=io256_s[:], scalar1=slot_s[:, k : k + 1],
                    scalar2=None, op0=ALU.is_equal,
                )
                nc.tensor.matmul(
                    out=pool_a[:], lhsT=oh[:, 0:P], rhs=hvs[:],
                    start=(k == 0), stop=(k == nb - 1),
                )
                nc.tensor.matmul(
                    out=pool_b[:], lhsT=oh[:, P:256], rhs=hvs[:],
                    start=(k == 0), stop=(k == nb - 1),
                )
            pool_sb = sp.tile([P, 2], F32, tag="pool_sb")
            nc.vector.tensor_copy(out=pool_sb[:, 0:1], in_=pool_a[:])
            nc.vector.tensor_copy(out=pool_sb[:, 1:2], in_=pool_b[:])
            nc.sync.dma_start(out=pool_out[0:P, 0:1], in_=pool_sb[:, 0:1])
            nc.sync.dma_start(out=pool_out[P:256, 0:1], in_=pool_sb[:, 1:2])

    nc.compile()
    return nc


# ======================================================================
# driver
# ======================================================================
def _in_maps(meta, core_arrays, slots, consts, x):
    nl, npd = meta["nl"], meta["npd"]
    maps = []
    for c in range(meta["ncores"]):
        gidx_pi, drel_pi = core_arrays[c]
        xT = np.zeros((D_IN, npd), dtype=np.float32)
        xT[:, :nl] = np.asarray(x[c * nl : (c + 1) * nl], dtype=np.float32).T
        m = dict(
            xT=xT, gidx=gidx_pi, drel=drel_pi, slot=slots[c]
        )
        m.update(consts)
        maps.append(m)
    return maps


def _install_ntff_shim():
    """Provide antenv.axon_hooks via direct ctypes into libaxon_pjrt.so."""
    import types, contextlib, ctypes

    try:
        import antenv.axon_hooks  # noqa: F401

        return True
    except ImportError:
        pass
    so_path = "/opt/axon/libaxon_pjrt.so"
    try:
        lib = ctypes.CDLL(so_path)
    except OSError:
        return False
    if not hasattr(lib, "axon_start_nrt_profile"):
        return False
    lib.axon_start_nrt_profile.argtypes = [
        ctypes.POINTER(ctypes.c_int64),
        ctypes.c_size_t,
    ]
    lib.axon_start_nrt_profile.restype = ctypes.c_int64
    lib.axon_stop_nrt_profile.argtypes = [ctypes.c_char_p]
    lib.axon_stop_nrt_profile.restype = ctypes.c_int64

    @contextlib.contextmanager
    def _hook(output_dir, device_ids):
        import jax

        jax.devices()
        if device_ids:
            ids = (ctypes.c_int64 * len(device_ids))(*device_ids)
            rc = lib.axon_start_nrt_profile(ids, len(device_ids))
        else:
            rc = lib.axon_start_nrt_profile(None, 0)
        if rc != 0:
            raise RuntimeError(f"axon_start_nrt_profile rc={rc}")
        try:
            yield
        finally:
            nfiles = lib.axon_stop_nrt_profile(str(output_dir).encode())
            print(f"ntff profile: {nfiles} file(s) -> {output_dir}", file=sys.stderr)

    mod = types.ModuleType("antenv.axon_hooks")
    mod.get_axon_ntff_profile_hook = lambda: _hook
    mod.set_axon_ntff_profile_hook = lambda h: None
    import antenv

    antenv.axon_hooks = mod
    sys.modules["antenv.axon_hooks"] = mod
    return True


def run(inputs, debug=False, trace=False):
    if trace:
        trace = _install_ntff_shim()
    x = np.asarray(inputs["x"])
    edge_index = np.asarray(inputs["edge_index"])
    batch = np.asarray(inputs["batch"])
    meta, core_arrays, slots, g0s, counts = _prep(x, edge_index, batch)
    weights = {
        k: np.asarray(v)
        for k, v in inputs.items()
        if k not in ("x", "edge_index", "batch")
    }
    consts = _consts(weights, meta)
    nc = build_program(meta, debug=debug)
    maps = _in_maps(meta, core_arrays, slots, consts, x)

    hw = get_hw_module(nc.m)
    old = nc.m
    nc.m = hw
    try:
        res = bass_utils.run_bass_kernel_spmd(
            nc, maps, core_ids=list(range(meta["ncores"])), trace=trace
        )
    finally:
        nc.m = old

    # host unshard: assemble per-graph sums from per-core slot partials
    sums = np.zeros((G,), dtype=np.float64)
    for c in range(meta["ncores"]):
        part = np.asarray(res.results[c]["pool_out"], dtype=np.float64).reshape(256)
        g0 = g0s[c]
        hi = min(256, G - g0)
        sums[g0 : g0 + hi] += part[:hi]
    fcf_b = float(np.asarray(inputs["fcf_b"]).reshape(-1)[0])
    out = sums / np.maximum(counts, 1.0) + fcf_b
    return out.astype(np.float32).reshape(G, 1), res


def kernel(**inputs):
    out, _ = run(inputs)
    return out



# revision 60
# speedup vs baseline: 1.1490x; 1.0009x over previous
"""GAT (2-layer, PyG-style) + MLP + graph-LN + global mean pool on 8 Trainium2 cores.

Strategy (sharding_hint): nodes partitioned contiguously across the 8 cores;
edges partitioned by destination node (1-D graph partition, host-sorted by dst);
the per-layer node-feature table [h | a_src-logit] is AllGathered so each core
gathers h[src] rows for its local edges with indirect DMA; per-destination
softmax + weighted aggregation is done with one-hot scatter matmuls
accumulating in PSUM per 128-node block; graph-LayerNorm statistics and are
combined with a tiny AllReduce; the final global_mean_pool partial sums per
core are assembled on the host (unshard step).
"""

import os
import sys

sys.path.insert(0, "/opt/trn_rl_repo")

import math
from contextlib import ExitStack

QSPLIT = os.environ.get("QSPLIT", "0") == "1"  # alternate SWDGE queues for gathers

import numpy as np
import ml_dtypes

BF = ml_dtypes.bfloat16

import concourse.bass as bass
import concourse.bacc as bacc
import concourse.tile as tile
import concourse.mybir as mybir
from concourse import bass_utils
from concourse.bass import IndirectOffsetOnAxis
from concourse.bass_interp import get_hw_module

F32 = mybir.dt.float32
BF16 = mybir.dt.bfloat16
I32 = mybir.dt.int32
I16 = mybir.dt.int16
ALU = mybir.AluOpType
ACTF = mybir.ActivationFunctionType

# ---- problem constants (hardcoded per spec) ----
N = 100000
E_RAW = 1600000
D_IN = 9
HID = 16
HEADS = 8
D = 128
G = 1000
NCORES = 8
NEG_SLOPE = 0.2
LN_EPS = 1e-5
TBLW = 144  # table row width in bf16 (288B rows: h 128 | al_s f32-pairs)
WIN = 32  # edge tiles per gather window
P = 128

PAD_DREL = 300.0  # one-hot never matches (iota is 0..127)
PAD_SLOT = 300.0


# ======================================================================
# host-side prep: edge sort / partition / padding, index layouts, weights
# ======================================================================
def _prep(x, edge_index, batch, n=N, e_raw=E_RAW, ncores=NCORES, g=G):
    nl = n // ncores  # owned nodes per core
    npd = ((nl + P - 1) // P) * P
    nb = npd // P  # 128-node blocks per core

    # self-loops are handled by an on-chip diagonal fast path (h and al
    # are core-local), so only the raw edges go through the gather.
    src = edge_index[0].astype(np.int64)
    dst = edge_index[1].astype(np.int64)
    order = np.argsort(dst, kind="stable")
    srcs = src[order]
    dsts = dst[order]

    bounds = np.searchsorted(dsts, np.arange(ncores + 1) * nl)
    per_core = []
    cnt = np.zeros((ncores, nb), dtype=np.int64)
    for c in range(ncores):
        s_c = srcs[bounds[c] : bounds[c + 1]]
        d_c = dsts[bounds[c] : bounds[c + 1]] - c * nl
        per_core.append((s_c, d_c))
        cnt[c] = np.bincount(d_c // P, minlength=nb)
    maxe = cnt.max(axis=0)
    cap = ((maxe + P - 1) // P) * P  # edge slots per block
    cap = np.maximum(cap, P)
    ktiles = (cap // P).astype(np.int64)
    t_total = int(ktiles.sum())
    blk_starts = np.concatenate([[0], np.cumsum(ktiles)])
    tile2blk = []
    mcap = []  # valid gather rows per tile (max over cores, pads skipped)
    for b in range(nb):
        tile2blk += [b] * int(ktiles[b])
        for i in range(int(ktiles[b])):
            mcap.append(int(min(P, max(1, maxe[b] - i * P))))
    blk_first = {b: int(blk_starts[b]) for b in range(nb)}
    blk_last = {b: int(blk_starts[b + 1]) - 1 for b in range(nb)}

    core_arrays = []
    for c in range(ncores):
        s_c, d_c = per_core[c]
        gidx = np.zeros((t_total * P,), dtype=np.int32)
        drel = np.full((t_total * P,), PAD_DREL, dtype=np.float32)
        blk = d_c // P
        cstart = np.concatenate([[0], np.cumsum(np.bincount(blk, minlength=nb))])
        for b in range(nb):
            e0, e1 = cstart[b], cstart[b + 1]
            o0 = int(blk_starts[b]) * P
            m = e1 - e0
            sc = s_c[e0:e1]
            gidx[o0 : o0 + m] = ((sc // nl) * npd + (sc % nl)).astype(np.int32)
            drel[o0 : o0 + m] = (d_c[e0:e1] % P).astype(np.float32)
        core_arrays.append(
            (
                gidx.reshape(t_total, P).T.copy(),
                drel.reshape(t_total, P).T.copy(),
            )
        )

    # pool slots
    bsort = np.asarray(batch, dtype=np.int64)
    slots = []
    g0s = []
    counts = np.bincount(bsort, minlength=g).astype(np.float64)
    for c in range(ncores):
        bs = bsort[c * nl : (c + 1) * nl]
        g0 = int(bs[0])
        sl = np.full((npd,), PAD_SLOT, dtype=np.float32)
        sl[:nl] = (bs - g0).astype(np.float32)
        assert sl[:nl].max() < 256, "graph-slot overflow"
        slots.append(sl.reshape(nb, P).T.copy())
        g0s.append(g0)

    meta = dict(
        n=n, nl=nl, npd=npd, nb=nb, t=t_total, mcap=mcap,
        tile2blk=tile2blk, blk_first=blk_first, blk_last=blk_last,
        ncores=ncores, g=g,
    )
    return meta, core_arrays, slots, g0s, counts


def _blockdiag(a):  # a [H, C] -> [H*C, H]
    h, c = a.shape
    out = np.zeros((h * c, h), dtype=np.float32)
    for i in range(h):
        out[i * c : (i + 1) * c, i] = a[i]
    return out


def _headmap(heads, ch):  # [H, H*C] one-hot expansion map
    out = np.zeros((heads, heads * ch), dtype=np.float32)
    for i in range(heads):
        out[i, i * ch : (i + 1) * ch] = 1.0
    return out


def _consts(weights, meta):
    """Replicated (same every core) input arrays."""
    w = weights
    c = {}
    c["W1"] = w["W1"].astype(np.float32)  # [9, 128]
    c["a1blk"] = np.concatenate(
        [_blockdiag(w["a_src1"]), _blockdiag(w["a_dst1"])], axis=1
    ).astype(BF)  # [128, 16]
    c["b1c"] = w["b1"].reshape(D, 1).astype(np.float32)
    c["W2"] = w["W2"].astype(BF)  # [128,128] lhsT
    c["a2blk"] = np.concatenate(
        [w["a_src2"].reshape(D, 1), w["a_dst2"].reshape(D, 1)], axis=1
    ).astype(BF)  # [128, 2]
    c["b2c"] = w["b2"].reshape(D, 1).astype(np.float32)
    for i in (1, 2, 3, 4):
        c[f"fc{i}w"] = w[f"fc{i}_w"].astype(BF)
        c[f"fc{i}b"] = w[f"fc{i}_b"].reshape(D, 1).astype(np.float32)
    c["g1c"] = w["g1"].reshape(D, 1).astype(np.float32)
    c["be1c"] = w["beta1"].reshape(D, 1).astype(np.float32)
    c["g2c"] = w["g2"].reshape(D, 1).astype(np.float32)
    c["be2c"] = w["beta2"].reshape(D, 1).astype(np.float32)
    c["fcfw"] = w["fcf_w"].reshape(D, 1).astype(BF)
    # layer-2 attention on raw (pre-LN) h: zals = h_pre @ (g1*(W2@a_src2)),
    # als2 = rstd1*zals + ofs1@(W2@a_src2); same for dst
    a2s = w["a_src2"].reshape(D).astype(np.float32)
    a2d = w["a_dst2"].reshape(D).astype(np.float32)
    W2f = w["W2"].astype(np.float32)
    g1f = w["g1"].astype(np.float32)
    c["zvec"] = np.stack(
        [g1f * (W2f @ a2s), g1f * (W2f @ a2d)], axis=1
    ).astype(BF)  # [D, 2]
    c["w2asum"] = (W2f @ (a2s + a2d)).reshape(D, 1).astype(np.float32)
    c["iota256"] = np.tile(np.arange(256, dtype=np.float32), (P, 1)).astype(BF)
    c["iota128"] = np.tile(np.arange(P, dtype=np.float32), (P, 1)).astype(BF)
    c["eye128b"] = np.eye(P, dtype=np.float32).astype(BF)
    c["eye16f"] = np.eye(16, dtype=np.float32)
    c["hmap1"] = _headmap(HEADS, HID)  # [8, 128] f32
    c["ones1r"] = np.ones((1, P), dtype=np.float32)
    c["ones128c"] = np.ones((P, 1), dtype=np.float32)
    return c


# ======================================================================
# device program
# ======================================================================
def _chunks(total, width):
    out = []
    o = 0
    while o < total:
        w = min(width, total - o)
        out.append((o, w))
        o += w
    return out


def build_program(meta, debug=False):
    npd, nb, t = meta["npd"], meta["nb"], meta["t"]
    ncores = meta["ncores"]
    mcap = meta["mcap"]
    tile2blk = meta["tile2blk"]
    blk_first = meta["blk_first"]
    blk_last = meta["blk_last"]
    n_glob = meta["n"]
    nl = meta["nl"]
    rg = [list(range(ncores))]
    ch512 = _chunks(npd, 512)
    ln_cnt = float(n_glob * D)  # real elements for graph-LN stats

    nc = bacc.Bacc(
        "TRN2",
        target_bir_lowering=False,
        debug=False,
        enable_asserts=False,
        num_devices=ncores,
        num_swdge_queues=2 if QSPLIT else 1,
    )

    def inp(name, shape, dt):
        return nc.dram_tensor(name, shape, dt, kind="ExternalInput").ap()

    xT = inp("xT", [D_IN, npd], F32)
    gidx = inp("gidx", [P, t], I32)
    drel = inp("drel", [P, t], F32)
    slot = inp("slot", [P, nb], F32)
    iota128 = inp("iota128", [P, P], BF16)
    W1 = inp("W1", [D_IN, D], F32)
    a1blk = inp("a1blk", [D, 2 * HEADS], BF16)
    b1c = inp("b1c", [D, 1], F32)
    W2 = inp("W2", [D, D], BF16)
    a2blk = inp("a2blk", [D, 2], BF16)
    b2c = inp("b2c", [D, 1], F32)
    zvec = inp("zvec", [D, 2], BF16)
    w2asum = inp("w2asum", [D, 1], F32)
    fcw = {i: inp(f"fc{i}w", [D, D], BF16) for i in (1, 2, 3, 4)}
    fcb = {i: inp(f"fc{i}b", [D, 1], F32) for i in (1, 2, 3, 4)}
    g1c = inp("g1c", [D, 1], F32)
    be1c = inp("be1c", [D, 1], F32)
    g2c = inp("g2c", [D, 1], F32)
    be2c = inp("be2c", [D, 1], F32)
    fcfw = inp("fcfw", [D, 1], BF16)
    iota256 = inp("iota256", [P, 256], BF16)
    eye128b = inp("eye128b", [P, P], BF16)
    eye16f = inp("eye16f", [16, 16], F32)
    hmap1 = inp("hmap1", [HEADS, D], F32)
    ones1r = inp("ones1r", [1, P], F32)
    ones128c = inp("ones128c", [P, 1], F32)

    pool_out = nc.dram_tensor("pool_out", [256, 1], F32, kind="ExternalOutput").ap()
    dbg_out = None
    dbg1_out = None
    if debug:
        dbg_out = nc.dram_tensor("dbg_out", [D, npd], F32, kind="ExternalOutput").ap()
        dbg1_out = nc.dram_tensor("dbg1_out", [D, npd], F32, kind="ExternalOutput").ap()

    TW = TBLW  # 256 bf16 = 512B rows: h(128) | al_s f32-pairs | pad

    with tile.TileContext(nc) as tc, ExitStack() as top:
        dram = top.enter_context(tc.tile_pool(name="dram", bufs=1, space="DRAM"))
        persist = top.enter_context(tc.tile_pool(name="persist", bufs=1))
        cpool = top.enter_context(tc.tile_pool(name="consts", bufs=1))
        cbp = top.enter_context(tc.tile_pool(name="cbp", bufs=2, space="PSUM"))
        cbs = top.enter_context(tc.tile_pool(name="cbs", bufs=2))

        def scr():  # one-bank PSUM scratch, callers slice columns
            return cbp.tile([P, 512], F32, tag="scr", name="scr")

        tbl1_loc = dram.tile([npd, TW], BF16, tag="tbl1_loc")
        tbl1_full = dram.tile([ncores * npd, TW], BF16, tag="tbl1_full", addr_space="Shared")
        tbl2_loc = dram.tile([npd, TW], BF16, tag="tbl2_loc")
        tbl2_full = dram.tile([ncores * npd, TW], BF16, tag="tbl2_full", addr_space="Shared")

        ln_in = dram.tile([1, 2], F32, tag="ln_in")
        ln_out = dram.tile([1, 2], F32, tag="ln_out", addr_space="Shared")
        ln_in2 = dram.tile([1, 2], F32, tag="ln_in2")
        ln_out2 = dram.tile([1, 2], F32, tag="ln_out2", addr_space="Shared")

        # persistent activations (transposed [feat, node]) and edge-index tables
        hc_sb = persist.tile([P, npd], BF16, tag="hc_sb")  # residual
        hf_sb = persist.tile([P, npd], BF16, tag="hf_sb")  # working activation
        drel_sb = persist.tile([P, t], F32, tag="drel_sb")
        gidx_sb = persist.tile([P, t], I32, tag="gidx_sb")
        ald1_sb = persist.tile([P, nb * HEADS], BF16, tag="ald1_sb")
        ald2_sb = persist.tile([P, nb], BF16, tag="ald2_sb")
        als1_sb = persist.tile([P, nb * HEADS], BF16, tag="als1_sb")
        als2_sb = persist.tile([P, nb], BF16, tag="als2_sb")
        hnm_sb = persist.tile([P, npd], BF16, tag="hnm_sb")  # node-major h
        # LN1-derived affine terms (filled post-E1, consumed in E2)
        sc1_sb = persist.tile([P, 1], F32, tag="sc1_sb")
        ofs1_sb = persist.tile([P, 1], F32, tag="ofs1_sb")
        rstd1_sb = persist.tile([P, 1], F32, tag="rstd1_sb")
        cc_sb = persist.tile([P, 1], F32, tag="cc_sb")
        c2_sb = persist.tile([P, 1], F32, tag="c2_sb")
        ofsrow_sb = persist.tile([1, P], F32, tag="ofsrow_sb")

        # constants in SBUF
        def cload(ap_in, shape, dt, tag):
            s = cpool.tile(shape, dt, tag=tag)
            nc.sync.dma_start(out=s[:], in_=ap_in)
            return s

        W1_s = cload(W1, [D_IN, D], F32, "W1")
        a1_s = cload(a1blk, [D, 2 * HEADS], BF16, "a1")
        a2_s = cload(a2blk, [D, 2], BF16, "a2")
        b1_s = cload(b1c, [D, 1], F32, "b1")
        b2_s = cload(b2c, [D, 1], F32, "b2")
        zvec_s = cload(zvec, [D, 2], BF16, "zvec")
        w2asum_s = cload(w2asum, [D, 1], F32, "w2asum")
        fcw_s = {i: cload(fcw[i], [D, D], BF16, f"fw{i}") for i in (1, 2, 3, 4)}
        fcb_s = {i: cload(fcb[i], [D, 1], F32, f"fb{i}") for i in (1, 2, 3, 4)}
        g1_s = cload(g1c, [D, 1], F32, "g1")
        be1_s = cload(be1c, [D, 1], F32, "be1")
        g2_s = cload(g2c, [D, 1], F32, "g2")
        be2_s = cload(be2c, [D, 1], F32, "be2")
        W2_s = cload(W2, [D, D], BF16, "W2")
        fcf_s = cload(fcfw, [D, 1], BF16, "fcf")
        io256_s = cload(iota256, [P, 256], BF16, "io256")
        io128_s = cload(iota128, [P, P], BF16, "io128")
        eye_s = cload(eye128b, [P, P], BF16, "eye")
        eye16_s = cload(eye16f, [16, 16], F32, "eye16")
        hmap1_s = cload(hmap1, [HEADS, D], F32, "hmap1")
        ones1r_s = cload(ones1r, [1, P], F32, "ones1r")
        ones128c_s = cload(ones128c, [P, 1], F32, "ones128c")
        slot_s = cload(slot, [P, nb], F32, "slot")

        nc.sync.dma_start(out=drel_sb[:], in_=drel)
        nc.sync.dma_start(out=gidx_sb[:], in_=gidx)

        # ---------------- stage A for a GAT layer: build tables ----------------
        def stage_a(layer, src_hT_fn):
            """Write tbl{layer}_loc rows [h bf16 | al_s f32] and ald table; then AllGather.
            src_hT_fn(c0, w, ps): fills psum tile [128, w] with this layer's hT chunk."""
            heads = HEADS if layer == 1 else 1
            a_s = a1_s if layer == 1 else a2_s
            tbl_loc = tbl1_loc if layer == 1 else tbl2_loc
            tbl_full = tbl1_full if layer == 1 else tbl2_full
            ald_sb = ald1_sb if layer == 1 else ald2_sb
            als_sb = als1_sb if layer == 1 else als2_sb
            with (
                tc.tile_pool(name=f"sa{layer}", bufs=5) as sp,
                tc.tile_pool(name=f"sap{layer}", bufs=2, space="PSUM") as pp,
                tc.tile_pool(name=f"sat{layer}", bufs=2, space="PSUM") as tp2,
            ):
                for c0, w in ch512:
                    ps = pp.tile([P, 512], F32, tag="hps")
                    src_hT_fn(c0, w, ps)
                    # keep transposed activation for downstream dense chain
                    nc.vector.tensor_copy(out=hf_sb[:, c0 : c0 + w], in_=ps[:, :w])
                for k in range(nb):
                    c0 = k * P
                    al_t = tp2.tile([P, 256], F32, tag="al_t", name="al_t")
                    # al_s/al_d for this chunk: [2*heads, 128] = a^T @ hT
                    alps = al_t[0 : 2 * heads, 0:P]
                    nc.tensor.matmul(
                        out=alps, lhsT=a_s[:], rhs=hf_sb[:, c0 : c0 + P],
                        start=True, stop=True,
                    )
                    asb = sp.tile([2 * heads, P], F32, tag="asb")
                    nc.vector.tensor_copy(out=asb[:], in_=alps)
                    # transpose -> [128, 2*heads]
                    atp = al_t[:, P : P + 2 * heads]
                    nc.tensor.transpose(
                        out=atp, in_=asb[:], identity=eye16_s[: 2 * heads, : 2 * heads]
                    )
                    ats = sp.tile([P, 2 * heads], F32, tag="ats")
                    nc.vector.tensor_copy(out=ats[:], in_=atp)
                    # transpose h chunk -> node-major [128n, 128f]
                    ht_t = tp2.tile([P, P], BF16, tag="ht_t", name="ht_t")
                    htp = ht_t[:]
                    nc.tensor.transpose(
                        out=htp, in_=hf_sb[:, c0 : c0 + P], identity=eye_s[:]
                    )
                    nc.vector.tensor_copy(
                        out=hnm_sb[:, c0 : c0 + P], in_=htp
                    )
                    # table writes
                    nc.sync.dma_start(
                        out=tbl_loc[c0 : c0 + P, 0:D], in_=hnm_sb[:, c0 : c0 + P]
                    )
                    nc.sync.dma_start(
                        out=tbl_loc[c0 : c0 + P, D : D + 2 * heads],
                        in_=ats[:].bitcast(BF16)[:, 0 : 2 * heads],
                    )
                    nc.vector.tensor_copy(
                        out=ald_sb[:, k * heads : (k + 1) * heads],
                        in_=ats[:, heads : 2 * heads],
                    )
                    nc.vector.tensor_copy(
                        out=als_sb[:, k * heads : (k + 1) * heads],
                        in_=ats[:, 0:heads],
                    )
            nc.gpsimd.collective_compute(
                "AllGather",
                ALU.bypass,
                replica_groups=rg,
                ins=[tbl_loc[:].opt()],
                outs=[tbl_full[:].opt()],
            )

        # ---------------- edge phase for a GAT layer ----------------
        def edge_phase(layer, out_cb):
            """GAT aggregation; out_cb(b, osb) gets [128f, 128n] f32 sbuf tile.

            Per tile: one indirect row-gather (h|al_s), one-hot S on DVE,
            St = transpose(S) on PE feeds the al_d expansion matmul; agg/den
            matmuls accumulate into quad-packed PSUM banks (4 blocks/bank).
            """
            heads = HEADS if layer == 1 else 1
            tbl_full = tbl1_full if layer == 1 else tbl2_full
            ald_sb = ald1_sb if layer == 1 else ald2_sb
            als_sb = als1_sb if layer == 1 else als2_sb
            nwin = (t + WIN - 1) // WIN
            agg_banks = {}
            den_banks = {}
            with (
                tc.tile_pool(name=f"eg{layer}", bufs=4) as gp,
                tc.tile_pool(name=f"eb{layer}", bufs=2) as bigp,
                tc.tile_pool(name=f"es{layer}", bufs=3) as sp,
                tc.tile_pool(name=f"est{layer}", bufs=WIN + 4) as stpool,
                tc.tile_pool(name=f"ef{layer}", bufs=3) as fin,
                tc.tile_pool(name=f"ep{layer}", bufs=2, space="PSUM") as pp,
                tc.tile_pool(name=f"ed{layer}", bufs=1, space="PSUM") as dp,
                tc.tile_pool(name=f"ea{layer}", bufs=1, space="PSUM") as ap_,
                tc.tile_pool(name=f"et{layer}", bufs=1, space="PSUM") as tp_,
                tc.tile_pool(name=f"er{layer}", bufs=1, space="PSUM") as rp,
            ):
                # first-touch memset so never-gathered pad partitions stay finite
                for _ in range(4):
                    gm = gp.tile([P, WIN * TW], BF16, tag="gath")
                    nc.gpsimd.memset(gm[:], 0.0)
                for wi in range(nwin):
                    t0 = wi * WIN
                    L = min(WIN, t - t0)
                    gath = gp.tile([P, WIN * TW], BF16, tag="gath")
                    for i in range(L):
                        m = mcap[t0 + i]
                        bi = nc.gpsimd.indirect_dma_start(
                            out=gath[0:m, i * TW : (i + 1) * TW],
                            out_offset=None,
                            in_=tbl_full[:],
                            in_offset=IndirectOffsetOnAxis(
                                ap=gidx_sb[0:m, t0 + i : t0 + i + 1], axis=0
                            ),
                        )
                        if QSPLIT and (t0 + i) % 2 == 1:
                            bi.ins.queue = "qPoolDynamic1"
                    aldw = ap_.tile([P, WIN * heads], F32, tag="aldw")
                    st_list = []
                    for i in range(L):
                        ti = t0 + i
                        s_t = stpool.tile([P, P], BF16, tag="s_t")
                        nc.vector.tensor_scalar(
                            out=s_t[:], in0=io128_s[:],
                            scalar1=drel_sb[:, ti : ti + 1], scalar2=None,
                            op0=ALU.is_equal,
                        )
                        st_list.append(s_t)
                    # batched St: 4 transposes share one PSUM bank, one ACT copy
                    for g0 in range(0, L, 4):
                        gl = min(4, L - g0)
                        stp = tp_.tile([P, 4 * P], BF16, tag="stp")
                        for j in range(gl):
                            nc.tensor.matmul(
                                out=stp[:, j * P : (j + 1) * P],
                                lhsT=st_list[g0 + j][:],
                                rhs=eye_s[:],
                                is_transpose=True,
                                start=(j == 0),
                                stop=(j == gl - 1),
                                skip_group_check=True,
                            )
                        sts = sp.tile([P, 4 * P], BF16, tag="sts")
                        nc.scalar.activation(
                            out=sts[:, 0 : gl * P], in_=stp[:, 0 : gl * P],
                            func=ACTF.Copy,
                        )
                        for j in range(gl):
                            i = g0 + j
                            b = tile2blk[t0 + i]
                            nc.tensor.matmul(
                                out=aldw[:, i * heads : (i + 1) * heads],
                                lhsT=sts[:, j * P : (j + 1) * P],
                                rhs=ald_sb[:, b * heads : (b + 1) * heads],
                                start=(i == 0),
                                stop=(i == L - 1),
                                skip_group_check=True,
                            )
                    # e = al_s[src] + al_d[dst]
                    als_v = (
                        gath[:]
                        .bitcast(F32)
                        .rearrange("p (t w) -> p t w", w=TW // 2)[
                            :, 0:L, D // 2 : D // 2 + heads
                        ]
                    )
                    e_sb = sp.tile([P, WIN * heads], F32, tag="e_sb")
                    nc.vector.tensor_tensor(
                        out=e_sb[:, 0 : L * heads].rearrange(
                            "p (t h) -> p t h", h=heads
                        ),
                        in0=als_v,
                        in1=aldw[:, 0 : L * heads].rearrange(
                            "p (t h) -> p t h", h=heads
                        ),
                        op=ALU.add,
                    )
                    if layer == 2:
                        # raw z-logits -> true logits: e = rstd1*e + CC
                        nc.vector.tensor_scalar(
                            out=e_sb[:, 0 : L * heads],
                            in0=e_sb[:, 0 : L * heads],
                            scalar1=rstd1_sb[:], scalar2=cc_sb[:],
                            op0=ALU.mult, op1=ALU.add,
                        )
                    t02 = sp.tile([P, WIN * heads], F32, tag="t02")
                    nc.vector.tensor_scalar(
                        out=t02[:, 0 : L * heads], in0=e_sb[:, 0 : L * heads],
                        scalar1=NEG_SLOPE, scalar2=None, op0=ALU.mult,
                    )
                    lr_sb = sp.tile([P, WIN * heads], F32, tag="lr_sb")
                    nc.vector.tensor_tensor(
                        out=lr_sb[:, 0 : L * heads], in0=e_sb[:, 0 : L * heads],
                        in1=t02[:, 0 : L * heads], op=ALU.max,
                    )
                    pexp = bigp.tile([P, WIN * D], BF16, tag="pexp")
                    nc.scalar.activation(
                        out=pexp[:, 0 : L * D].rearrange(
                            "p (t h c) -> p t h c", h=heads, c=D // heads
                        ),
                        in_=lr_sb[:, 0 : L * heads]
                        .rearrange("p (t h) -> p t h", h=heads)
                        .broadcast_to((P, L, heads, D // heads)),
                        func=ACTF.Exp,
                    )
                    p_sb = sp.tile([P, WIN * heads], BF16, tag="p_sb")
                    nc.scalar.activation(
                        out=p_sb[:, 0 : L * heads], in_=lr_sb[:, 0 : L * heads],
                        func=ACTF.Exp,
                    )
                    wh = bigp.tile([P, WIN * D], BF16, tag="wh")
                    nc.vector.tensor_tensor(
                        out=wh[:, 0 : L * D].rearrange("p (t c) -> p t c", c=D),
                        in0=gath[:].rearrange("p (t w) -> p t w", w=TW)[:, 0:L, 0:D],
                        in1=pexp[:, 0 : L * D].rearrange("p (t c) -> p t c", c=D),
                        op=ALU.mult,
                    )
                    for i in range(L):
                        ti = t0 + i
                        b = tile2blk[ti]
                        q = b // 4  # quad id
                        qi = b % 4
                        if q not in agg_banks:
                            qblocks = [bb for bb in range(4 * q, min(4 * q + 4, nb))]
                            ab = pp.tile([P, 512], F32, tag="aggq", name="aggq")
                            db = dp.tile([8, 512], F32, tag="denq", name="denq")
                            agg_banks[q] = (
                                ab, blk_first[qblocks[0]], blk_last[qblocks[-1]]
                            )
                            den_banks[q] = (
                                db, blk_first[qblocks[0]], blk_last[qblocks[-1]]
                            )
                        ab, qf, ql = agg_banks[q]
                        db, _, _ = den_banks[q]
                        nc.tensor.matmul(
                            out=ab[:, qi * P : (qi + 1) * P],
                            lhsT=wh[:, i * D : (i + 1) * D],
                            rhs=st_list[i][:],
                            start=(ti == qf),
                            stop=False,
                            skip_group_check=True,
                        )
                        nc.tensor.matmul(
                            out=db[0:heads, qi * P : (qi + 1) * P],
                            lhsT=p_sb[:, i * heads : (i + 1) * heads],
                            rhs=st_list[i][:],
                            start=(ti == qf),
                            stop=False,
                            skip_group_check=True,
                        )
                        if ti == blk_last[b]:
                            # diagonal (self-loop) contribution: h and al are
                            # local, injected as one pseudo-tile (S = identity)
                            esf = fin.tile([P, HEADS], F32, tag="esf")
                            nc.vector.tensor_tensor(
                                out=esf[:, 0:heads],
                                in0=als_sb[:, b * heads : (b + 1) * heads],
                                in1=ald_sb[:, b * heads : (b + 1) * heads],
                                op=ALU.add,
                            )
                            if layer == 2:
                                nc.vector.tensor_scalar(
                                    out=esf[:, 0:heads], in0=esf[:, 0:heads],
                                    scalar1=rstd1_sb[:], scalar2=cc_sb[:],
                                    op0=ALU.mult, op1=ALU.add,
                                )
                            tsf = fin.tile([P, HEADS], F32, tag="tsf")
                            nc.vector.tensor_scalar(
                                out=tsf[:, 0:heads], in0=esf[:, 0:heads],
                                scalar1=NEG_SLOPE, scalar2=None, op0=ALU.mult,
                            )
                            lrs = fin.tile([P, HEADS], F32, tag="lrs")
                            nc.vector.tensor_tensor(
                                out=lrs[:, 0:heads], in0=esf[:, 0:heads],
                                in1=tsf[:, 0:heads], op=ALU.max,
                            )
                            pxs = fin.tile([P, D], BF16, tag="pxs")
                            nc.scalar.activation(
                                out=pxs[:].rearrange("p (h c) -> p h c", h=heads),
                                in_=lrs[:, 0:heads].broadcast_to(
                                    (P, heads, D // heads)
                                ),
                                func=ACTF.Exp,
                            )
                            psbs = fin.tile([P, HEADS], BF16, tag="psbs")
                            nc.scalar.activation(
                                out=psbs[:, 0:heads], in_=lrs[:, 0:heads],
                                func=ACTF.Exp,
                            )
                            whs = fin.tile([P, D], BF16, tag="whs")
                            nc.vector.tensor_tensor(
                                out=whs[:], in0=hnm_sb[:, b * P : (b + 1) * P],
                                in1=pxs[:], op=ALU.mult,
                            )
                            nc.tensor.matmul(
                                out=ab[:, qi * P : (qi + 1) * P],
                                lhsT=whs[:], rhs=eye_s[:],
                                start=False, stop=(ti == ql),
                                skip_group_check=True,
                            )
                            nc.tensor.matmul(
                                out=db[0:heads, qi * P : (qi + 1) * P],
                                lhsT=psbs[:, 0:heads], rhs=eye_s[:],
                                start=False, stop=(ti == ql),
                                skip_group_check=True,
                            )
                            dsb = fin.tile([heads, P], F32, tag="dsb")
                            nc.vector.tensor_scalar(
                                out=dsb[:], in0=db[0:heads, qi * P : (qi + 1) * P],
                                scalar1=1e-16, scalar2=None, op0=ALU.add,
                            )
                            rsb = fin.tile([heads, P], F32, tag="rsb")
                            nc.vector.reciprocal(out=rsb[:], in_=dsb[:])
                            rp_t = rp.tile([P, 512], F32, tag="rscr")
                            rex = rp_t[:, 0:P]
                            nc.tensor.matmul(
                                out=rex,
                                lhsT=(ones1r_s if heads == 1 else hmap1_s)[:],
                                rhs=rsb[:], start=True, stop=True,
                            )
                            rxs = fin.tile([P, P], F32, tag="rxs")
                            nc.scalar.activation(out=rxs[:], in_=rex, func=ACTF.Copy)
                            osb = fin.tile([P, P], F32, tag="osb")
                            if layer == 2:
                                # out2 = [sc1*ab + ofs1*den] @ W2 / den
                                tmul = fin.tile([P, P], F32, tag="tmul")
                                nc.vector.tensor_scalar(
                                    out=tmul[:],
                                    in0=ab[:, qi * P : (qi + 1) * P],
                                    scalar1=sc1_sb[:], scalar2=None,
                                    op0=ALU.mult,
                                )
                                t2l = fin.tile([P, P], BF16, tag="t2l")
                                nc.vector.tensor_copy(out=t2l[:], in_=tmul[:])
                                ps2 = rp_t[:, 2 * P : 3 * P]
                                nc.tensor.matmul(
                                    out=ps2, lhsT=W2_s[:], rhs=t2l[:],
                                    start=True, stop=True,
                                )
                                nc.vector.tensor_tensor(
                                    out=osb[:], in0=ps2, in1=rxs[:],
                                    op=ALU.mult,
                                )
                            else:
                                nc.vector.tensor_tensor(
                                    out=osb[:], in0=ab[:, qi * P : (qi + 1) * P],
                                    in1=rxs[:], op=ALU.mult,
                                )
                            out_cb(b, osb)
                            if b == 4 * q + 3 or b == nb - 1:
                                del agg_banks[q], den_banks[q]

        # ---------------- graph-LN over h_sb -> dst_sb (bf16) ----------------
        def graph_ln(src_sb, g_s, be_s, ln_i, ln_o, dst_sb, stash=False):
            with (
                tc.tile_pool(name="ln", bufs=2) as sp,
                tc.tile_pool(name="lnp", bufs=2, space="PSUM") as pp,
            ):
                nchunk = len(ch512)
                stats = sp.tile([P, nchunk * 6], F32, tag="stats")
                for ci, (c0, w) in enumerate(ch512):
                    nc.vector.bn_stats(
                        out=stats[:, ci * 6 : (ci + 1) * 6], in_=src_sb[:, c0 : c0 + w]
                    )
                mv = sp.tile([P, 2], F32, tag="mv")
                nc.vector.bn_aggr(
                    out=mv[:], in_=stats[:].rearrange("p (c s) -> p c s", s=6)
                )
                # per-partition sums: [sx, sxx] = [m, (v+m^2)] * npd
                sums = sp.tile([P, 2], F32, tag="sums")
                nc.vector.tensor_scalar(
                    out=sums[:, 0:1], in0=mv[:, 0:1], scalar1=float(npd),
                    scalar2=None, op0=ALU.mult,
                )
                m2 = sp.tile([P, 1], F32, tag="m2")
                nc.vector.tensor_tensor(
                    out=m2[:], in0=mv[:, 0:1], in1=mv[:, 0:1], op=ALU.mult
                )
                nc.vector.tensor_tensor(
                    out=sums[:, 1:2], in0=mv[:, 1:2], in1=m2[:], op=ALU.add
                )
                nc.vector.tensor_scalar(
                    out=sums[:, 1:2], in0=sums[:, 1:2], scalar1=float(npd),
                    scalar2=None, op0=ALU.mult,
                )
                red = pp.tile([1, 2], F32, tag="red")
                nc.tensor.matmul(
                    out=red[:], lhsT=ones128c_s[:], rhs=sums[:], start=True, stop=True
                )
                rsb = sp.tile([1, 2], F32, tag="rsb2")
                nc.vector.tensor_copy(out=rsb[:], in_=red[:])
                nc.gpsimd.dma_start(out=ln_i[:], in_=rsb[:])
                nc.gpsimd.collective_compute(
                    "AllReduce", ALU.add, replica_groups=rg,
                    ins=[ln_i[:].opt()], outs=[ln_o[:].opt()],
                )
                ar = sp.tile([1, 2], F32, tag="ar")
                nc.gpsimd.dma_start(out=ar[:], in_=ln_o[:])
                bc = pp.tile([P, 2], F32, tag="bc")
                nc.tensor.matmul(
                    out=bc[:], lhsT=ones1r_s[:], rhs=ar[:], start=True, stop=True
                )
                # mu = s1/cnt ; var = s2/cnt - mu^2 ; s = g * rsqrt(var+eps) ; b = be - mu*s
                mu = sp.tile([P, 1], F32, tag="mu")
                nc.vector.tensor_scalar(
                    out=mu[:], in0=bc[:, 0:1], scalar1=1.0 / ln_cnt, scalar2=None,
                    op0=ALU.mult,
                )
                var = sp.tile([P, 1], F32, tag="var")
                nc.vector.tensor_scalar(
                    out=var[:], in0=bc[:, 1:2], scalar1=1.0 / ln_cnt, scalar2=None,
                    op0=ALU.mult,
                )
                mu2 = sp.tile([P, 1], F32, tag="mu2")
                nc.vector.tensor_tensor(out=mu2[:], in0=mu[:], in1=mu[:], op=ALU.mult)
                nc.vector.tensor_tensor(
                    out=var[:], in0=var[:], in1=mu2[:], op=ALU.subtract
                )
                nc.vector.tensor_scalar(
                    out=var[:], in0=var[:], scalar1=LN_EPS, scalar2=None, op0=ALU.add
                )
                sd = sp.tile([P, 1], F32, tag="sd")
                nc.scalar.activation(out=sd[:], in_=var[:], func=ACTF.Sqrt)
                rstd = sp.tile([P, 1], F32, tag="rstd")
                nc.vector.reciprocal(out=rstd[:], in_=sd[:])
                sc = sp.tile([P, 1], F32, tag="sc")
                nc.vector.tensor_tensor(out=sc[:], in0=g_s[:], in1=rstd[:], op=ALU.mult)
                ofs = sp.tile([P, 1], F32, tag="ofs")
                nc.vector.tensor_tensor(out=ofs[:], in0=mu[:], in1=sc[:], op=ALU.mult)
                nc.vector.tensor_tensor(
                    out=ofs[:], in0=be_s[:], in1=ofs[:], op=ALU.subtract
                )
                if stash:
                    nc.vector.tensor_copy(out=rstd1_sb[:], in_=rstd[:])
                    nc.vector.tensor_copy(out=sc1_sb[:], in_=sc[:])
                    nc.vector.tensor_copy(out=ofs1_sb[:], in_=ofs[:])
                for c0, w in ch512:
                    nc.scalar.activation(
                        out=dst_sb[:, c0 : c0 + w], in_=src_sb[:, c0 : c0 + w],
                        func=ACTF.Identity, bias=ofs[:], scale=sc[:],
                    )
                # zero the padded node columns so they don't pollute later stats
                if npd > nl:
                    nc.gpsimd.memset(dst_sb[:, nl:npd], 0.0)

        # =========================== the network ===========================
        # ---- GAT layer 1 stage A: h1 = x @ W1 (transposed) ----
        xin_pool = top.enter_context(tc.tile_pool(name="xin", bufs=4))

        def h1_fn(c0, w, ps):
            xt = xin_pool.tile([D_IN, 512], F32, tag="xt")
            nc.sync.dma_start(out=xt[:, :w], in_=xT[:, c0 : c0 + w])
            nc.tensor.matmul(
                out=ps[:, :w], lhsT=W1_s[:], rhs=xt[:, :w], start=True, stop=True
            )

        stage_a(1, h1_fn)

        # ---- GAT layer 1 edge phase; per-block callback also runs fc1/fc2
        # and builds the raw (pre-LN) layer-2 table [h_pre | zals], so AG2
        # can start immediately when E1 drains and LN1 runs under it. ----
        def out1_cb(b, osb):
            c0 = b * P
            h1o = hc_sb[:, c0 : c0 + P]
            nc.scalar.activation(
                out=h1o, in_=osb[:], func=ACTF.Relu, bias=b1_s[:], scale=1.0,
            )
            s_ = scr()
            ps1 = s_[:, 0:P]
            nc.tensor.matmul(
                out=ps1, lhsT=fcw_s[1][:], rhs=h1o, start=True, stop=True
            )
            t1 = cbs.tile([P, P], BF16, tag="t1")
            nc.scalar.activation(
                out=t1[:], in_=ps1, func=ACTF.Relu, bias=fcb_s[1][:], scale=1.0
            )
            psf = s_[:, P : 2 * P]
            nc.tensor.matmul(
                out=psf, lhsT=fcw_s[2][:], rhs=t1[:], start=True, stop=True
            )
            t2f = cbs.tile([P, P], F32, tag="t2f")
            nc.vector.tensor_tensor(out=t2f[:], in0=psf, in1=h1o, op=ALU.add)
            nc.scalar.activation(
                out=hf_sb[:, c0 : c0 + P], in_=t2f[:], func=ACTF.Relu,
                bias=fcb_s[2][:], scale=1.0,
            )
            # raw layer-2 attention z-values
            zz = s_[0:2, 2 * P : 3 * P]
            nc.tensor.matmul(
                out=zz, lhsT=zvec_s[:], rhs=hf_sb[:, c0 : c0 + P],
                start=True, stop=True,
            )
            zzs = cbs.tile([2, P], F32, tag="zzs")
            nc.vector.tensor_copy(out=zzs[:], in_=zz)
            atp2 = s_[:, 3 * P : 3 * P + 2]
            nc.tensor.transpose(out=atp2, in_=zzs[:], identity=eye16_s[0:2, 0:2])
            ats2 = cbs.tile([P, 2], F32, tag="ats2")
            nc.vector.tensor_copy(out=ats2[:], in_=atp2)
            nc.vector.tensor_copy(out=ald2_sb[:, b : b + 1], in_=ats2[:, 1:2])
            nc.vector.tensor_copy(out=als2_sb[:, b : b + 1], in_=ats2[:, 0:1])
            htp2 = s_[:].bitcast(BF16)[:, 772:900]
            nc.tensor.transpose(
                out=htp2, in_=hf_sb[:, c0 : c0 + P], identity=eye_s[:]
            )
            nc.vector.tensor_copy(out=hnm_sb[:, c0 : c0 + P], in_=htp2)
            nc.sync.dma_start(
                out=tbl2_loc[c0 : c0 + P, 0:D], in_=hnm_sb[:, c0 : c0 + P]
            )
            nc.sync.dma_start(
                out=tbl2_loc[c0 : c0 + P, D : D + 2],
                in_=ats2[:].bitcast(BF16)[:, 0:2],
            )

        edge_phase(1, out1_cb)
        nc.gpsimd.collective_compute(
            "AllGather",
            ALU.bypass,
            replica_groups=rg,
            ins=[tbl2_loc[:].opt()],
            outs=[tbl2_full[:].opt()],
        )
        if debug:
            with tc.tile_pool(name="dbg1p", bufs=2) as dbp1:
                for c0, w in ch512:
                    dt1 = dbp1.tile([P, 512], F32, tag="dbg1")
                    nc.vector.tensor_copy(out=dt1[:, :w], in_=hc_sb[:, c0 : c0 + w])
                    nc.sync.dma_start(out=dbg1_out[:, c0 : c0 + w], in_=dt1[:, :w])

        # ---- LN1 (stats + AR + affine; runs under AG2), then the E2
        # affine constants derived from (rstd1, ofs1) ----
        graph_ln(hf_sb, g1_s, be1_s, ln_in, ln_out, hc_sb, stash=True)
        with (
            tc.tile_pool(name="epl", bufs=1) as epp,
            tc.tile_pool(name="eplp", bufs=1, space="PSUM") as epps,
        ):
            m1 = epps.tile([1, 1], F32, tag="m1")
            nc.tensor.matmul(
                out=m1[:], lhsT=ofs1_sb[:], rhs=w2asum_s[:], start=True, stop=True
            )
            m1s = epp.tile([1, 1], F32, tag="m1s")
            nc.vector.tensor_copy(out=m1s[:], in_=m1[:])
            ccp = epps.tile([P, 1], F32, tag="ccp")
            nc.tensor.matmul(
                out=ccp[:], lhsT=ones1r_s[:], rhs=m1s[:], start=True, stop=True
            )
            nc.vector.tensor_copy(out=cc_sb[:], in_=ccp[:])
            ofsb = epp.tile([P, 1], BF16, tag="ofsb")
            nc.vector.tensor_copy(out=ofsb[:], in_=ofs1_sb[:])
            c2p = epps.tile([P, 1], F32, tag="c2p")
            nc.tensor.matmul(
                out=c2p[:], lhsT=W2_s[:], rhs=ofsb[:], start=True, stop=True
            )
            c2t = epp.tile([P, 1], F32, tag="c2t")
            nc.vector.tensor_copy(out=c2t[:], in_=c2p[:])
            nc.vector.tensor_tensor(
                out=c2_sb[:], in0=c2t[:], in1=b2_s[:], op=ALU.add
            )
            orp = epps.tile([1, P], F32, tag="orp")
            nc.tensor.matmul(
                out=orp[:], lhsT=ofsb[:], rhs=eye_s[:], start=True, stop=True
            )
            nc.vector.tensor_copy(out=ofsrow_sb[:], in_=orp[:])

        # ---- GAT layer 2 edge phase (raw table; LN+W2 folded into the
        # finalize) -> hf_sb = out2 + (W2^T ofs1 + b2) ----
        def out2_cb(b, osb):
            nc.scalar.activation(
                out=hf_sb[:, b * P : (b + 1) * P], in_=osb[:], func=ACTF.Identity,
                bias=c2_sb[:], scale=1.0,
            )

        edge_phase(2, out2_cb)

        # ---- fc3 (relu), fc4 (+residual h_ln1), LN2, fcf, pool ----
        with (
            tc.tile_pool(name="fc2", bufs=3) as fp,
            tc.tile_pool(name="fcp2", bufs=2, space="PSUM") as fpp,
        ):
            for c0, w in ch512:
                ps = fpp.tile([P, 512], F32, tag="fc3p")
                nc.tensor.matmul(
                    out=ps[:, :w], lhsT=fcw_s[3][:], rhs=hf_sb[:, c0 : c0 + w],
                    start=True, stop=True,
                )
                nc.scalar.activation(
                    out=hf_sb[:, c0 : c0 + w], in_=ps[:, :w], func=ACTF.Relu,
                    bias=fcb_s[3][:], scale=1.0,
                )
            for c0, w in ch512:
                ps = fpp.tile([P, 512], F32, tag="fc4p")
                nc.tensor.matmul(
                    out=ps[:, :w], lhsT=fcw_s[4][:], rhs=hf_sb[:, c0 : c0 + w],
                    start=True, stop=True,
                )
                tmp = fp.tile([P, 512], F32, tag="fc4t")
                nc.vector.tensor_tensor(
                    out=tmp[:, :w], in0=ps[:, :w], in1=hc_sb[:, c0 : c0 + w], op=ALU.add
                )
                nc.scalar.activation(
                    out=hf_sb[:, c0 : c0 + w], in_=tmp[:, :w], func=ACTF.Identity,
                    bias=fcb_s[4][:], scale=1.0,
                )
        graph_ln(hf_sb, g2_s, be2_s, ln_in2, ln_out2, hc_sb)  # hc_sb = h_ln2

        if debug:
            with tc.tile_pool(name="dbgp", bufs=2) as dbp:
                for c0, w in ch512:
                    dt_ = dbp.tile([P, 512], F32, tag="dbg")
                    nc.vector.tensor_copy(out=dt_[:, :w], in_=hc_sb[:, c0 : c0 + w])
                    nc.sync.dma_start(out=dbg_out[:, c0 : c0 + w], in_=dt_[:, :w])

        # fcf + per-core pooled slot sums
        with (
            tc.tile_pool(name="pl", bufs=6) as sp,
            tc.tile_pool(name="plp", bufs=4, space="PSUM") as pp,
            tc.tile_pool(name="plq", bufs=1, space="PSUM") as qq,
        ):
            pool_a = qq.tile([P, 1], F32, tag="pool_a")
            pool_b = qq.tile([P, 1], F32, tag="pool_b")
            for k in range(nb):
                c0 = k * P
                hv = pp.tile([P, 1], F32, tag="hv")
                nc.tensor.matmul(
                    out=hv[:], lhsT=hc_sb[:, c0 : c0 + P], rhs=fcf_s[:],
                    start=True, stop=True,
                )
                hvs = sp.tile([P, 1], BF16, tag="hvs")
                nc.vector.tensor_copy(out=hvs[:], in_=hv[:])
                oh = sp.tile([P, 256], BF16, tag="oh")
                nc.vector.tensor_scalar(
                    out=oh[:], in0=io256_s[:], scalar1=slot_s[:, k : k + 1],
                    scalar2=None, op0=ALU.is_equal,
                )
                nc.tensor.matmul(
                    out=pool_a[:], lhsT=oh[:, 0:P], rhs=hvs[:],
                    start=(k == 0), stop=(k == nb - 1),
                )
                nc.tensor.matmul(
                    out=pool_b[:], lhsT=oh[:, P:256], rhs=hvs[:],
                    start=(k == 0), stop=(k == nb - 1),
                )
            pool_sb = sp.tile([P, 2], F32, tag="pool_sb")
            nc.vector.tensor_copy(out=pool_sb[:, 0:1], in_=pool_a[:])
            nc.vector.tensor_copy(out=pool_sb[:, 1:2], in_=pool_b[:])
            nc.sync.dma_start(out=pool_out[0:P, 0:1], in_=pool_sb[:, 0:1])
            nc.sync.dma_start(out=pool_out[P:256, 0:1], in_=pool_sb[:, 1:2])

    nc.compile()
    return nc


# ======================================================================
# driver
# ======================================================================
def _in_maps(meta, core_arrays, slots, consts, x):
    nl, npd = meta["nl"], meta["npd"]
    maps = []
    for c in range(meta["ncores"]):
        gidx_pi, drel_pi = core_arrays[c]
        xT = np.zeros((D_IN, npd), dtype=np.float32)
        xT[:, :nl] = np.asarray(x[c * nl : (c + 1) * nl], dtype=np.float32).T
        m = dict(
            xT=xT, gidx=gidx_pi, drel=drel_pi, slot=slots[c]
        )
        m.update(consts)
        maps.append(m)
    return maps


def _install_ntff_shim():
    """Provide antenv.axon_hooks via direct ctypes into libaxon_pjrt.so."""
    import types, contextlib, ctypes

    try:
        import antenv.axon_hooks  # noqa: F401

        return True
    except ImportError:
        pass
    so_path = "/opt/axon/libaxon_pjrt.so"
    try:
        lib = ctypes.CDLL(so_path)
    except OSError:
        return False
    if not hasattr(lib, "axon_start_nrt_profile"):
        return False
    lib.axon_start_nrt_profile.argtypes = [
        ctypes.POINTER(ctypes.c_int64),
        ctypes.c_size_t,
    ]
    lib.axon_start_nrt_profile.restype = ctypes.c_int64
    lib.axon_stop_nrt_profile.argtypes = [ctypes.c_char_p]
    lib.axon_stop_nrt_profile.restype = ctypes.c_int64

    @contextlib.contextmanager
    def _hook(output_dir, device_ids):
        import jax

        jax.devices()
        if device_ids:
            ids = (ctypes.c_int64 * len(device_ids))(*device_ids)
            rc = lib.axon_start_nrt_profile(ids, len(device_ids))
        else:
            rc = lib.axon_start_nrt_profile(None, 0)
        if rc != 0:
            raise RuntimeError(f"axon_start_nrt_profile rc={rc}")
        try:
            yield
        finally:
            nfiles = lib.axon_stop_nrt_profile(str(output_dir).encode())
            print(f"ntff profile: {nfiles} file(s) -> {output_dir}", file=sys.stderr)

    mod = types.ModuleType("antenv.axon_hooks")
    mod.get_axon_ntff_profile_hook = lambda: _hook
    mod.set_axon_ntff_profile_hook = lambda h: None
    import antenv

    antenv.axon_hooks = mod
    sys.modules["antenv.axon_hooks"] = mod
    return True


def run(inputs, debug=False, trace=False):
    if trace:
        trace = _install_ntff_shim()
    x = np.asarray(inputs["x"])
    edge_index = np.asarray(inputs["edge_index"])
    batch = np.asarray(inputs["batch"])
    meta, core_arrays, slots, g0s, counts = _prep(x, edge_index, batch)
    weights = {
        k: np.asarray(v)
        for k, v in inputs.items()
        if k not in ("x", "edge_index", "batch")
    }
    consts = _consts(weights, meta)
    nc = build_program(meta, debug=debug)
    maps = _in_maps(meta, core_arrays, slots, consts, x)

    hw = get_hw_module(nc.m)
    old = nc.m
    nc.m = hw
    try:
        res = bass_utils.run_bass_kernel_spmd(
            nc, maps, core_ids=list(range(meta["ncores"])), trace=trace
        )
    finally:
        nc.m = old

    # host unshard: assemble per-graph sums from per-core slot partials
    sums = np.zeros((G,), dtype=np.float64)
    for c in range(meta["ncores"]):
        part = np.asarray(res.results[c]["pool_out"], dtype=np.float64).reshape(256)
        g0 = g0s[c]
        hi = min(256, G - g0)
        sums[g0 : g0 + hi] += part[:hi]
    fcf_b = float(np.asarray(inputs["fcf_b"]).reshape(-1)[0])
    out = sums / np.maximum(counts, 1.0) + fcf_b
    return out.astype(np.float32).reshape(G, 1), res


def kernel(**inputs):
    out, _ = run(inputs)
    return out

